# revision 15
# baseline (speedup 1.0000x reference)
"""Trainium2 Bass kernel for nn_DiscreteTimeNeuralGraph.

Strategy (8 NeuronCores, batch-parallel, engine-balanced):
  - Shard the batch of 32 across 8 cores (4 samples each); weights replicated.
  - Downsample path on-device; BatchNorm batch stats via per-core partial
    sums + one tiny AllReduce each.
  - Main loop in fp16 storage (X, D, weights; fp32 PSUM accumulation):
    depthwise 3x3 conv groups 0-2 as rect-clipped diagonal matmuls on PE;
    group 3 computed on the Vector engine as tensor_scalar(mul, 4x mode) +
    tensor_tensor(add, 2x mode) chains writing fp16 SBUF directly.
    PSUM->SBUF depthwise results copied (and cast to fp16) on the Pool
    engine, freeing ACT for the instnorm applies.
    Channel mix as fp16 blocked matmuls; instance-norm stats on VectorE;
    instnorm+ReLU fused into one ScalarE activation producing fp16 X.
  - Pad-column zeroing via engine memsets (not DMA).
  - Readout: center 2x2 mean (folded into fc weights) + fc matmul (f32r).

Top-k threshold for the pruned graph weight is computed on host
(np.partition) -- it is weight preprocessing of a replicated input.
"""
import numpy as np

import concourse.bass as bass
import concourse.tile as tile
from concourse import bacc, mybir
from concourse.bass_utils import run_bass_kernel_spmd

F32 = mybir.dt.float32
F32R = mybir.dt.float32r
F16 = mybir.dt.float16
AF = mybir.ActivationFunctionType
ALU = mybir.AluOpType

N_CORES = 8
B = 32
BPC = B // N_CORES          # 4 samples per core
DIM = 512
DS = 128
FEAT = 256
LAYERS = 8
IMG = 128
OUT = 1000
EPS = 1e-5
HALF = IMG // 4 // 2 - 1    # 15
PRUNE = 0.9

# f32 mega-weight column layout ([128, WCOLS])
W1X_OFF = 0                  # 3 dx-taps x [128,128] for conv1
W2D_OFF = W1X_OFF + 3 * 128  # 9 taps x [128,128] diag-dup for conv2
W3_OFF = W2D_OFF + 9 * 128   # [128,128] conv3 (w3 stacked twice on K)
FCW_OFF = W3_OFF + 128       # 2 kg x [128, 1000] fc lhsT (x0.25 pooled)
FCB_OFF = FCW_OFF + 2 * 1000  # [128, 8] fc bias chunks
BN1_OFF = FCB_OFF + 8          # [128, 2] bn1 gamma/beta (dup across halves)
BN2_OFF = BN1_OFF + 2          # [128, 2]
NGB_OFF = BN2_OFF + 2          # [128, 8] instnorm gamma/beta per group
KV_OFF = NGB_OFF + 8           # [128, 36] dw k vectors (g*9+t)
WCOLS = KV_OFF + 36

# fp16 weight layout ([128, W16COLS])
DW16_OFF = 0                   # 36 taps x [128,128] diag (g*9+t)
MIX16_OFF = DW16_OFF + 36 * 128  # 4 kg x [128, 512] = w_eff.T blocks
W16COLS = MIX16_OFF + 4 * 512

XP_BUFS = 20

# engine split for the main-loop depthwise conv: per group, per tap-index
# 'P' = whole group on PE (psum); otherwise per-tap: 'V' = DVE mul+add pair,
# 'A' = ACT product + DVE add, 'G' = Pool fused scalar_tensor_tensor.
# Tap 0 (the full-coverage (1,1) tap) of a non-PE group always inits on DVE.
DW_PLAN = {
    0: 'P',
    1: 'P',
    2: ['V', 'A', 'A', 'A', 'A', 'A', 'A', 'A', 'A'],
    3: ['V', 'V', 'V', 'V', 'V', 'G', 'G', 'G', 'G'],
}

# tap order: full-coverage tap first (start=True zeroes the psum region)
TAPS = [(1, 1), (0, 0), (0, 1), (0, 2), (1, 0), (1, 2), (2, 0), (2, 1), (2, 2)]


def _clip(lo, hi, lo2, hi2):
    return max(lo, lo2), min(hi, hi2)


def build_nc():
    nc = bacc.Bacc(num_devices=N_CORES)
    x4 = nc.dram_tensor("x4", [BPC, 3, IMG, IMG], F32R, kind="ExternalInput").ap()
    wts = nc.dram_tensor("wts", [128, WCOLS], F32R, kind="ExternalInput").ap()
    wts16 = nc.dram_tensor("wts16", [128, W16COLS], F16,
                           kind="ExternalInput").ap()
    y4 = nc.dram_tensor("y4", [BPC, OUT], F32, kind="ExternalOutput").ap()

    with tile.TileContext(nc) as tc:
        with (
            tc.tile_pool(name="wp", bufs=1) as wp,
            tc.tile_pool(name="wp16", bufs=1) as wp16,
            tc.tile_pool(name="small", bufs=1) as small,
            tc.tile_pool(name="psA", bufs=2, space="PSUM") as psA,
            tc.tile_pool(name="psB", bufs=3, space="PSUM") as psB,
            tc.tile_pool(name="dram", bufs=1, space="DRAM") as dram,
        ):
            w_sb = wp.tile([128, WCOLS], F32R)
            nc.sync.dma_start(out=w_sb, in_=wts)
            w32 = w_sb.bitcast(F32)
            w16 = wp16.tile([128, W16COLS], F16)
            nc.sync.dma_start(out=w16, in_=wts16)

            def wcols(off, n):
                return w_sb[:, off:off + n]

            def w16cols(off, n):
                return w16[:, off:off + n]

            def kvec(g, t):
                return w32[:, KV_OFF + g * 9 + t:KV_OFF + g * 9 + t + 1]

            eps_t = small.tile([128, 1], F32)
            nc.vector.memset(eps_t, EPS)
            z32 = small.tile([128, 64], F32)
            nc.vector.memset(z32, 0.0)
            z16 = small.tile([128, 32], F16)
            nc.vector.memset(z16, 0.0)

            # ---------------- downsample ----------------
            with tc.tile_pool(name="ds1", bufs=1) as ds1:
                # im2col9: partition p = 32*s + 3*dy + c ; free = (oy 64, ix' 130)
                # ix' = ix + 1 (x padded by 1 on both sides)
                im9 = ds1.tile([128, 64 * 130], F32R)
                im9r = im9.rearrange("p (y x) -> p y x", y=64, x=130)
                # zero the x pads (cols 0 and 129)
                for xc in (0, 129):
                    im9_pads = bass.AP(tensor=im9.tensor,
                                       offset=im9.offset + xc,
                                       ap=[im9.ap[0], [130, 64]])
                    nc.vector.tensor_copy(out=im9_pads, in_=z32[:, 0:64])
                # row oy=0 is out of range for dy=0 taps: zero it everywhere
                # first (dy=1/2 loads overwrite their row 0 afterwards; cols
                # 0/129 are the x-pads zeroed above)
                nc.vector.tensor_copy(out=im9[:, 1:65], in_=z32[:, 0:64])
                nc.vector.tensor_copy(out=im9[:, 65:129], in_=z32[:, 0:64])
                # x rows: iy = 2*oy + dy - 1
                # partition base: sample s -> 64*(s%2) + 27*(s//2)
                x4r = x4.rearrange("s c (y2 two) x -> s c y2 two x", two=2)
                for s in range(BPC):
                    for dy in range(3):
                        p0 = 64 * (s % 2) + 27 * (s // 2) + 3 * dy
                        if dy == 0:
                            # oy in [1,64): iy = 2*(oy-1)+1
                            nc.sync.dma_start(
                                out=im9r[p0:p0 + 3, 1:64, 1:129],
                                in_=x4r[s, :, 0:63, 1, :])
                        elif dy == 1:
                            nc.sync.dma_start(
                                out=im9r[p0:p0 + 3, :, 1:129],
                                in_=x4r[s, :, :, 0, :])
                        else:
                            nc.sync.dma_start(
                                out=im9r[p0:p0 + 3, :, 1:129],
                                in_=x4r[s, :, :, 1, :])

                # conv1: out h1 [128 = 64*(s//2)+ch, (s%2, oy 64, ox 64)]
                h1 = ds1.tile([128, 8192], F32)
                h1r = h1.rearrange("p (sh y x) -> p sh y x", sh=2, y=64, x=64)
                # im2col x-read: ix' = 2*ox + dx (x2 = ox + dx//2, tx = dx%2)
                # paired matmul: K=54 block-diag covers samples (q, q+2):
                # out partitions 0-63 <- sample q, 64-127 <- sample q+2.
                im9x = im9.rearrange("p (y x2 two) -> p y x2 two", x2=65, two=2)
                for q in range(2):
                    for yb in range(4):           # 16-oy blocks
                        for h in range(2):
                            pc1 = psA.tile([128, 512], F32, tag="a",
                                           name="pc1")
                            pc1r = pc1.rearrange("p (y x) -> p y x", y=8, x=64)
                            oy0 = yb * 16 + h * 8
                            for di, dx in enumerate([1, 0, 2]):
                                rhs = im9x[64 * q:64 * q + 54, oy0:oy0 + 8,
                                           dx // 2:dx // 2 + 64, dx % 2]
                                lhsT = w_sb[64 * q:64 * q + 54,
                                            W1X_OFF + di_col(dx) * 128:
                                            W1X_OFF + di_col(dx) * 128 + 128]
                                nc.tensor.matmul(pc1r, lhsT, rhs,
                                                 start=(di == 0), stop=(di == 2),
                                                 tile_position=(64 * q, 0))
                            if (q + yb + h) % 2 == 0:
                                nc.scalar.copy(
                                    out=h1r[:, q, oy0:oy0 + 8, :], in_=pc1)
                            else:
                                nc.vector.tensor_copy(
                                    out=h1r[:, q, oy0:oy0 + 8, :], in_=pc1)

                # BN1 partial stats
                st1 = small.tile([128, 16, 6], F32)
                for i in range(16):
                    nc.vector.bn_stats(out=st1[:, i, :],
                                       in_=h1[:, i * 512:(i + 1) * 512])
                mv1 = small.tile([128, 2], F32)
                nc.vector.bn_aggr(out=mv1, in_=st1)
                sums1 = small.tile([128, 2], F32)
                tmp1 = small.tile([128, 1], F32)
                nc.vector.tensor_scalar_mul(out=sums1[:, 0:1], in0=mv1[:, 0:1],
                                            scalar1=8192.0)
                nc.vector.tensor_mul(out=tmp1, in0=mv1[:, 0:1], in1=mv1[:, 0:1])
                nc.vector.tensor_add(out=tmp1, in0=tmp1, in1=mv1[:, 1:2])
                nc.vector.tensor_scalar_mul(out=sums1[:, 1:2], in0=tmp1,
                                            scalar1=8192.0)
                bn1_in = dram.tile([128, 2], F32)
                bn1_out = dram.tile([128, 2], F32)
                nc.gpsimd.dma_start(out=bn1_in, in_=sums1)
                nc.gpsimd.collective_compute(
                    "AllReduce", mybir.AluOpType.add,
                    replica_groups=[list(range(N_CORES))],
                    ins=[bn1_in.opt()], outs=[bn1_out.opt()])
                red1 = small.tile([128, 2], F32)
                nc.gpsimd.dma_start(out=red1, in_=bn1_out)
                comb1 = small.tile([128, 2], F32)
                nc.gpsimd.dma_start(out=comb1[0:64, :], in_=red1[0:64, :])
                nc.gpsimd.dma_start(out=comb1[0:64, :], in_=red1[64:128, :],
                                    accum_op=mybir.AluOpType.add)
                # scale/bias on rows 0:64, then duplicate
                s1t1 = small.tile([128, 2], F32)
                _bn_scale_bias(nc, s1t1, comb1, w32, BN1_OFF, 131072.0,
                               eps_t, small, rows=64)
                nc.gpsimd.dma_start(out=s1t1[64:128, :], in_=s1t1[0:64, :])

                # apply BN1 + relu -> h1n (f32r), x padded to 66 (ix' = ix+1)
                h1n = ds1.tile([128, 2 * 64 * 66], F32R)
                h1nr3 = h1n.rearrange("p (sh y x) -> p sh y x",
                                      sh=2, y=64, x=66)
                for sh in range(2):
                    for xc in (0, 65):
                        h1n_pads = bass.AP(tensor=h1n.tensor,
                                           offset=h1n.offset + 4224 * sh + xc,
                                           ap=[h1n.ap[0], [66, 64]])
                        nc.vector.tensor_copy(out=h1n_pads, in_=z32[:, 0:64])
                h1r4 = h1.rearrange("p (sh y x) -> p sh y x", sh=2, y=64, x=64)
                for sh in range(2):
                    nc.scalar.activation(out=h1nr3[:, sh, :, 1:65],
                                         in_=h1r4[:, sh, :, :], func=AF.Relu,
                                         scale=s1t1[:, 0:1], bias=s1t1[:, 1:2])

                # conv2: depthwise 3x3 stride 2 -> d2 [128, (sh, 32, 32)]
                # row iy = 2*oy + dy - 1 (unpadded), col ix' = 2*ox + dx (padded)
                h1nr = h1n.rearrange(
                    "p (sh y2 ty x2 tx) -> p sh y2 ty x2 tx",
                    sh=2, y2=32, ty=2, x2=33, tx=2)
                d2 = ds1.tile([128, 2048], F32R)
                for sh in range(2):
                    pd2 = psB.tile([128, 1024], F32, tag="b", name="pd2")
                    pd2r = pd2.rearrange("p (h y x) -> p h y x", h=2, y=16, x=32)
                    for h in range(2):
                        for ti, (dy, dx) in enumerate(TAPS):
                            oy0, oy1 = _clip(h * 16, h * 16 + 16,
                                             1 if dy == 0 else 0, 32)
                            if dy == 1:
                                ys, par = oy0, 0
                            elif dy == 0:
                                ys, par = oy0 - 1, 1
                            else:
                                ys, par = oy0, 1
                            rhs = h1nr[:, sh, ys:ys + (oy1 - oy0), par,
                                       dx // 2:dx // 2 + 32, dx % 2]
                            outp = pd2r[:, h, oy0 - h * 16:oy1 - h * 16, :]
                            t = TAPS.index((dy, dx))
                            nc.tensor.matmul(
                                outp, wcols(W2D_OFF + t * 128, 128), rhs,
                                start=(ti == 0), stop=(ti == len(TAPS) - 1))
                    nc.scalar.copy(out=d2[:, sh * 1024:(sh + 1) * 1024], in_=pd2)

                # conv3: 1x1, 64 -> 128 ; h3 [128=outc, (s, 1024px)]
                h3 = small.tile([128, 4096], F32)
                for a in range(2):
                    for nb in range(4):
                        pc3 = psA.tile([128, 512], F32, tag="a",
                                       name=f"pc3_{a}_{nb}")
                        nc.tensor.matmul(
                            pc3,
                            w_sb[64 * a:64 * a + 64, W3_OFF:W3_OFF + 128],
                            d2[64 * a:64 * a + 64, nb * 512:(nb + 1) * 512],
                            start=True, stop=True)
                        s_full = 2 * a + nb // 2
                        dst = h3[:, s_full * 1024 + (nb % 2) * 512:
                                 s_full * 1024 + (nb % 2) * 512 + 512]
                        if nb % 2 == 0:
                            nc.scalar.copy(out=dst, in_=pc3)
                        else:
                            nc.vector.tensor_copy(out=dst, in_=pc3)

                # BN2 stats + allreduce
                st2 = small.tile([128, 8, 6], F32)
                for i in range(8):
                    nc.vector.bn_stats(out=st2[:, i, :],
                                       in_=h3[:, i * 512:(i + 1) * 512])
                mv2 = small.tile([128, 2], F32)
                nc.vector.bn_aggr(out=mv2, in_=st2)
                sums2 = small.tile([128, 2], F32)
                tmp2 = small.tile([128, 1], F32)
                nc.vector.tensor_scalar_mul(out=sums2[:, 0:1], in0=mv2[:, 0:1],
                                            scalar1=4096.0)
                nc.vector.tensor_mul(out=tmp2, in0=mv2[:, 0:1], in1=mv2[:, 0:1])
                nc.vector.tensor_add(out=tmp2, in0=tmp2, in1=mv2[:, 1:2])
                nc.vector.tensor_scalar_mul(out=sums2[:, 1:2], in0=tmp2,
                                            scalar1=4096.0)
                bn2_in = dram.tile([128, 2], F32)
                bn2_out = dram.tile([128, 2], F32)
                nc.gpsimd.dma_start(out=bn2_in, in_=sums2)
                nc.gpsimd.collective_compute(
                    "AllReduce", mybir.AluOpType.add,
                    replica_groups=[list(range(N_CORES))],
                    ins=[bn2_in.opt()], outs=[bn2_out.opt()])
                red2 = small.tile([128, 2], F32)
                nc.gpsimd.dma_start(out=red2, in_=bn2_out)
                s2t2 = small.tile([128, 2], F32)
                _bn_scale_bias(nc, s2t2, red2, w32, BN2_OFF, 32768.0,
                               eps_t, small, rows=128)

            # ---------------- main loop ----------------
            with (
                tc.tile_pool(name="xp", bufs=XP_BUFS) as xp,
                tc.tile_pool(name="dp", bufs=3) as dp,
                tc.tile_pool(name="pp", bufs=3) as pp,
                tc.tile_pool(name="stp", bufs=4) as stp,
            ):
                def new_x_tile(name):
                    # pad columns (0, 33) of every xp slot were zeroed once
                    # below; applies only write the interior, so they persist.
                    return xp.tile([128, 32 * 34], F16, tag="X", name=name)

                # one-time zeroing of the pad columns of all X slots: the
                # dummies are simultaneously live (kept alive by the reads
                # below), so by pigeonhole they cover all slots.
                _dummies = []
                for i in range(XP_BUFS):
                    zt = xp.tile([128, 32 * 34], F16, tag="X", name=f"xz{i}")
                    for xc in (0, 33):
                        pads = bass.AP(tensor=zt.tensor, offset=zt.offset + xc,
                                       ap=[zt.ap[0], [34, 32]])
                        nc.vector.tensor_copy(out=pads, in_=z16)
                    _dummies.append(zt)
                _pad_scratch = small.tile([128, 1], F16)
                for zt in _dummies:
                    nc.scalar.copy(out=_pad_scratch, in_=zt[:, 0:1])

                Xcur = {}
                for s in range(BPC):
                    xt = new_x_tile(f"X1_0_{s}")
                    xtr = xt.rearrange("p (y x) -> p y x", y=32, x=34)
                    h3r = h3.rearrange("p (s y x) -> p s y x", s=4, y=32, x=32)
                    nc.scalar.activation(out=xtr[:, :, 1:33],
                                         in_=h3r[:, s, :, :],
                                         func=AF.Relu,
                                         scale=s2t2[:, 0:1], bias=s2t2[:, 1:2])
                    Xcur[(0, s)] = xt

                pooled_in = small.tile([128, 2, 4, 4], F32)

                for L in range(1, LAYERS + 1):
                    gs_in = sorted({g for (g, _s) in Xcur})
                    mgs = [2, 3] if L == LAYERS else [0, 1, 2, 3]
                    Xnext = {}
                    for s in range(BPC):
                        # D tile for this (layer, sample): 4 groups x 1024 fp16
                        d16 = dp.tile([128, 4096], F16, tag="D",
                                      name=f"D{L}_{s}")
                        d16r = d16.rearrange("p (g y x) -> p g y x",
                                             g=4, y=32, x=32)
                        for g in gs_in:
                            Xr = Xcur[(g, s)].rearrange("p (y x) -> p y x",
                                                        y=32, x=34)
                            plan = DW_PLAN[g] if len(gs_in) > 1 else 'P'
                            if plan != 'P':
                                # SBUF-side depthwise accumulated in d16[g]
                                dy0, dx0 = TAPS[0]
                                nc.vector.tensor_scalar_mul(
                                    out=d16r[:, g],
                                    in0=Xr[:, :, dx0:dx0 + 32],
                                    scalar1=kvec(g, 0))
                                for ti, (dy, dx) in enumerate(TAPS[1:], 1):
                                    oy0, oy1 = max(0, 1 - dy), min(32, 33 - dy)
                                    xin = Xr[:, oy0 + dy - 1:oy1 + dy - 1,
                                             dx:dx + 32]
                                    dslice = d16r[:, g, oy0:oy1, :]
                                    eng = plan[ti]
                                    prod = pp.tile([128, 1024], F16, tag="P",
                                                   name=f"P{L}_{s}_{g}_{ti}")
                                    pr = prod.rearrange("p (y x) -> p y x",
                                                        y=32, x=32)
                                    if eng == 'A':
                                        nc.scalar.activation(
                                            out=pr[:, oy0:oy1, :], in_=xin,
                                            func=AF.Identity,
                                            scale=kvec(g, ti))
                                        nc.vector.tensor_add(
                                            out=dslice, in0=dslice,
                                            in1=pr[:, oy0:oy1, :])
                                    elif eng == 'G':
                                        nc.gpsimd.tensor_scalar_mul(
                                            out=pr[:, oy0:oy1, :], in0=xin,
                                            scalar1=kvec(g, ti))
                                        nc.gpsimd.tensor_add(
                                            out=dslice, in0=dslice,
                                            in1=pr[:, oy0:oy1, :])
                                    else:
                                        nc.vector.tensor_scalar_mul(
                                            out=pr[:, oy0:oy1, :], in0=xin,
                                            scalar1=kvec(g, ti))
                                        nc.vector.tensor_add(
                                            out=dslice, in0=dslice,
                                            in1=pr[:, oy0:oy1, :])
                                continue
                            # PE depthwise: 2 psum halves x 9 clipped taps
                            for h in range(2):
                                pdw = psA.tile([128, 512], F32, tag="a",
                                               name=f"pdw{L}_{s}_{g}_{h}")
                                pdwr = pdw.rearrange("p (y x) -> p y x",
                                                     y=16, x=32)
                                for ti, (dy, dx) in enumerate(TAPS):
                                    oy0, oy1 = _clip(h * 16, h * 16 + 16,
                                                     max(0, 1 - dy), 33 - dy)
                                    if oy0 >= oy1:
                                        continue
                                    rhs = Xr[:, oy0 + dy - 1:oy1 + dy - 1,
                                             dx:dx + 32]
                                    outp = pdwr[:, oy0 - h * 16:oy1 - h * 16, :]
                                    t = TAPS.index((dy, dx))
                                    nc.tensor.matmul(
                                        outp,
                                        w16cols(DW16_OFF + (g * 9 + t) * 128,
                                                128),
                                        rhs,
                                        start=(ti == 0),
                                        stop=(ti == len(TAPS) - 1))
                                # cast+copy to fp16 D (GPSIMD can't read PSUM)
                                nc.scalar.copy(
                                    out=d16[:, g * 1024 + h * 512:
                                            g * 1024 + h * 512 + 512],
                                    in_=pdw)
                        # channel mix + instnorm per output group
                        for mg in mgs:
                            pm = psB.tile([128, 1024], F32, tag="b",
                                          name=f"pm{L}_{s}_{mg}")
                            for h in range(2):
                                for ki, kg in enumerate(gs_in):
                                    nc.tensor.matmul(
                                        pm[:, h * 512:h * 512 + 512],
                                        w16cols(MIX16_OFF + kg * 512
                                                + mg * 128, 128),
                                        d16[:, kg * 1024 + h * 512:
                                            kg * 1024 + h * 512 + 512],
                                        start=(ki == 0),
                                        stop=(ki == len(gs_in) - 1))
                            st = stp.tile([128, 2, 6], F32, tag="st")
                            nc.vector.bn_stats(out=st[:, 0, :],
                                               in_=pm[:, 0:512])
                            nc.vector.bn_stats(out=st[:, 1, :],
                                               in_=pm[:, 512:1024])
                            mv = stp.tile([128, 2], F32, tag="mv")
                            nc.vector.bn_aggr(out=mv, in_=st)
                            sc = stp.tile([128, 1], F32, tag="sc")
                            tt = stp.tile([128, 1], F32, tag="tt")
                            nc.scalar.activation(out=sc, in_=mv[:, 1:2],
                                                 func=AF.Sqrt, bias=eps_t)
                            nc.vector.reciprocal(out=sc, in_=sc)
                            nc.vector.tensor_scalar_mul(
                                out=sc, in0=sc,
                                scalar1=w32[:, NGB_OFF + 2 * mg:
                                            NGB_OFF + 2 * mg + 1])
                            nc.vector.tensor_mul(out=tt, in0=mv[:, 0:1], in1=sc)
                            nc.vector.tensor_scalar(
                                out=tt, in0=tt, scalar1=-1.0,
                                scalar2=w32[:, NGB_OFF + 2 * mg + 1:
                                            NGB_OFF + 2 * mg + 2],
                                op0=mybir.AluOpType.mult,
                                op1=mybir.AluOpType.add)
                            if L < LAYERS:
                                xt = new_x_tile(f"X{L + 1}_{mg}_{s}")
                                Xnext[(mg, s)] = xt
                                xtr = xt.rearrange("p (y x) -> p y x",
                                                   y=32, x=34)
                                pmr2 = pm.rearrange("p (y x) -> p y x",
                                                    y=32, x=32)
                                nc.scalar.activation(
                                    out=xtr[:, :, 1:33], in_=pmr2,
                                    func=AF.Relu, scale=sc, bias=tt)
                            else:
                                pmr = pm.rearrange("p (y x) -> p y x",
                                                   y=32, x=32)
                                nc.scalar.activation(
                                    out=pooled_in[:, mg - 2, s, :],
                                    in_=pmr[:, HALF - 1:HALF + 1,
                                            HALF - 1:HALF + 1],
                                    func=AF.Identity, scale=sc, bias=tt)
                    Xcur = Xnext

                # ---------------- readout ----------------
                tadd = small.tile([128, 2, 4], F32)
                tadd2 = small.tile([128, 2, 4], F32)
                pooled = small.tile([128, 2, 4], F32R)
                nc.vector.tensor_add(out=tadd, in0=pooled_in[:, :, :, 0],
                                     in1=pooled_in[:, :, :, 1])
                nc.vector.tensor_add(out=tadd2, in0=pooled_in[:, :, :, 2],
                                     in1=pooled_in[:, :, :, 3])
                nc.vector.tensor_add(out=pooled, in0=tadd, in1=tadd2)
                y_sb = small.tile([128, 4, 8], F32)
                for mo in range(8):
                    mlen = 128 if mo < 7 else OUT - 7 * 128
                    pf = psA.tile([128, 512], F32, tag="a", name=f"pf{mo}")
                    for kgi in range(2):
                        nc.tensor.matmul(
                            pf[0:mlen, 0:4],
                            w_sb[:, FCW_OFF + kgi * 1000 + mo * 128:
                                 FCW_OFF + kgi * 1000 + mo * 128 + mlen],
                            pooled[:, kgi, :],
                            start=(kgi == 0), stop=(kgi == 1))
                    nc.scalar.activation(
                        out=y_sb[0:mlen, :, mo], in_=pf[0:mlen, 0:4],
                        func=AF.Identity,
                        bias=w32[0:mlen, FCB_OFF + mo:FCB_OFF + mo + 1],
                        scale=1.0)
                for s in range(BPC):
                    dst1 = bass.AP(tensor=y4.tensor, offset=OUT * s,
                                   ap=[[1, 128], [128, 7]])
                    nc.sync.dma_start(out=dst1, in_=y_sb[:, s, 0:7])
                    dst2 = bass.AP(tensor=y4.tensor, offset=OUT * s + 896,
                                   ap=[[1, 104]])
                    nc.sync.dma_start(out=dst2, in_=y_sb[0:104, s, 7])

    nc.finalize()
    return nc


def di_col(dx):
    # column index of conv1 tap dx within w1x block (emission order 1,0,2)
    return {1: 0, 0: 1, 2: 2}[dx]


def _bn_scale_bias(nc, out_st, sums, w32, gb_off, n_tot, eps_t, pool, rows):
    """out_st[:rows, 0] = gamma*rsqrt(var+eps); out_st[:rows, 1] = beta - mu*scale."""
    r = slice(0, rows)
    mu = pool.tile([128, 1], F32, name=f"mu{gb_off}")
    ex2 = pool.tile([128, 1], F32, name=f"ex2{gb_off}")
    var = pool.tile([128, 1], F32, name=f"var{gb_off}")
    nc.vector.tensor_scalar_mul(out=mu[r], in0=sums[r, 0:1], scalar1=1.0 / n_tot)
    nc.vector.tensor_scalar_mul(out=ex2[r], in0=sums[r, 1:2], scalar1=1.0 / n_tot)
    nc.vector.tensor_mul(out=var[r], in0=mu[r], in1=mu[r])
    nc.vector.tensor_sub(out=var[r], in0=ex2[r], in1=var[r])
    nc.scalar.activation(out=var[r], in_=var[r], func=AF.Sqrt, bias=eps_t[r])
    nc.vector.reciprocal(out=var[r], in_=var[r])
    nc.vector.tensor_scalar_mul(out=out_st[r, 0:1], in0=var[r],
                                scalar1=w32[r, gb_off:gb_off + 1])
    nc.vector.tensor_mul(out=mu[r], in0=mu[r], in1=out_st[r, 0:1])
    nc.vector.tensor_scalar(out=out_st[r, 1:2], in0=mu[r], scalar1=-1.0,
                            scalar2=w32[r, gb_off + 1:gb_off + 2],
                            op0=mybir.AluOpType.mult,
                            op1=mybir.AluOpType.add)


def _pack_weights(ds_w1, ds_w2, ds_w3, conv_w, graph_w, fc_w, fc_b,
                  bn1_g, bn1_b, bn2_g, bn2_b, norm_g, norm_b):
    wts = np.zeros((128, WCOLS), np.float32)
    w16 = np.zeros((128, W16COLS), np.float16)
    # pruned graph weight
    k = int((1.0 - PRUNE) * DIM * DIM)
    a = np.abs(graph_w).ravel()
    thresh = np.partition(a, -k)[-k]
    w_eff = np.where(np.abs(graph_w) >= thresh, graph_w, 0.0).astype(np.float32)
    # conv1 taps, paired block-diag:
    # rows 64*q + 27*a + 3*dy + c, cols 64*a + o = w1[o, c, dy, dx]
    for dx in range(3):
        dc = di_col(dx)
        blk = np.zeros((128, 128), np.float32)
        for qq in range(2):
            for aa in range(2):
                for dy in range(3):
                    for c in range(3):
                        blk[64 * qq + 27 * aa + 3 * dy + c,
                            64 * aa:64 * aa + 64] = ds_w1[:, c, dy, dx]
        wts[:, W1X_OFF + dc * 128:W1X_OFF + (dc + 1) * 128] = blk
    # conv2 diag-dup taps
    for t, (dy, dx) in enumerate(TAPS):
        blk = np.zeros((128, 128), np.float32)
        d = ds_w2[:, 0, dy, dx]
        for aa in range(2):
            idx = np.arange(64)
            blk[64 * aa + idx, 64 * aa + idx] = d
        wts[:, W2D_OFF + t * 128:W2D_OFF + (t + 1) * 128] = blk
    # conv3: [64a + c, o] = w3[o, c]
    w3 = ds_w3[:, :, 0, 0]  # [128, 64]
    wts[0:64, W3_OFF:W3_OFF + 128] = w3.T
    wts[64:128, W3_OFF:W3_OFF + 128] = w3.T
    # main dw diag taps (fp16)
    for g in range(4):
        for t, (dy, dx) in enumerate(TAPS):
            blk = np.zeros((128, 128), np.float16)
            idx = np.arange(128)
            blk[idx, idx] = conv_w[g * 128:(g + 1) * 128, 0, dy, dx]
            off = DW16_OFF + (g * 9 + t) * 128
            w16[:, off:off + 128] = blk
    # dw k vectors for the DVE path (f32)
    for g in range(4):
        for t, (dy, dx) in enumerate(TAPS):
            wts[:, KV_OFF + g * 9 + t] = conv_w[g * 128:(g + 1) * 128, 0, dy, dx]
    # mix (fp16): [p, kg*512 + mg*128 + j] = w_eff[mg*128 + j, kg*128 + p]
    weT = w_eff.T  # [in, out]
    for kg in range(4):
        w16[:, MIX16_OFF + kg * 512:MIX16_OFF + (kg + 1) * 512] = \
            weT[kg * 128:(kg + 1) * 128, :].astype(np.float16)
    # fc: [p, kg*1000 + m] = 0.25 * fc_w[m, kg*128 + p]
    for kg in range(2):
        wts[:, FCW_OFF + kg * 1000:FCW_OFF + (kg + 1) * 1000] = \
            0.25 * fc_w[:, kg * 128:(kg + 1) * 128].T
    # fc bias [p, mo]
    fcb = np.zeros((128, 8), np.float32)
    fb = np.zeros(1024, np.float32)
    fb[:OUT] = fc_b
    fcb[:, :] = fb.reshape(8, 128).T
    wts[:, FCB_OFF:FCB_OFF + 8] = fcb
    # bn gammas/betas
    wts[0:64, BN1_OFF] = bn1_g
    wts[64:128, BN1_OFF] = bn1_g
    wts[0:64, BN1_OFF + 1] = bn1_b
    wts[64:128, BN1_OFF + 1] = bn1_b
    wts[:, BN2_OFF] = bn2_g
    wts[:, BN2_OFF + 1] = bn2_b
    for g in range(4):
        wts[:, NGB_OFF + 2 * g] = norm_g[g * 128:(g + 1) * 128]
        wts[:, NGB_OFF + 2 * g + 1] = norm_b[g * 128:(g + 1) * 128]
    return wts, w16


_nc_cache = None
last_results = None


def kernel(**inputs):
    global _nc_cache, last_results
    inputs = {k: np.asarray(v, np.float32) for k, v in inputs.items()}
    wts, w16 = _pack_weights(
        inputs["ds_w1"], inputs["ds_w2"], inputs["ds_w3"], inputs["conv_w"],
        inputs["graph_w"], inputs["fc_w"], inputs["fc_b"],
        inputs["bn1_g"], inputs["bn1_b"], inputs["bn2_g"], inputs["bn2_b"],
        inputs["norm_g"], inputs["norm_b"])
    x = inputs["x"]
    if _nc_cache is None:
        _nc_cache = build_nc()
    nc = _nc_cache
    in_maps = [{"x4": np.ascontiguousarray(x[c * BPC:(c + 1) * BPC]),
                "wts": wts, "wts16": w16} for c in range(N_CORES)]
    res = run_bass_kernel_spmd(nc, in_maps, core_ids=list(range(N_CORES)))
    last_results = res
    return np.concatenate([res.results[c]["y4"] for c in range(N_CORES)], axis=0)


# revision 16
# speedup vs baseline: 1.2055x; 1.2055x over previous
"""Trainium2 Bass kernel for nn_DiscreteTimeNeuralGraph.

Strategy (8 NeuronCores, batch-parallel, engine-balanced):
  - Shard the batch of 32 across 8 cores (4 samples each); weights replicated.
  - Downsample path on-device; BatchNorm batch stats via per-core partial
    sums + one tiny AllReduce each.
  - Main loop in fp16 storage (X, D, weights; fp32 PSUM accumulation):
    depthwise 3x3 conv groups 0-2 as rect-clipped diagonal matmuls on PE;
    group 3 computed on the Vector engine as tensor_scalar(mul, 4x mode) +
    tensor_tensor(add, 2x mode) chains writing fp16 SBUF directly.
    PSUM->SBUF depthwise results copied (and cast to fp16) on the Pool
    engine, freeing ACT for the instnorm applies.
    Channel mix as fp16 blocked matmuls; instance-norm stats on VectorE;
    instnorm+ReLU fused into one ScalarE activation producing fp16 X.
  - Pad-column zeroing via engine memsets (not DMA).
  - Readout: center 2x2 mean (folded into fc weights) + fc matmul (f32r).

Top-k threshold for the pruned graph weight is computed on host
(np.partition) -- it is weight preprocessing of a replicated input.
"""
import numpy as np

import concourse.bass as bass
import concourse.tile as tile
from concourse import bacc, mybir
from concourse.bass_utils import run_bass_kernel_spmd

F32 = mybir.dt.float32
F32R = mybir.dt.float32r
F16 = mybir.dt.float16
AF = mybir.ActivationFunctionType
ALU = mybir.AluOpType

N_CORES = 8
B = 32
BPC = B // N_CORES          # 4 samples per core
DIM = 512
DS = 128
FEAT = 256
LAYERS = 8
IMG = 128
OUT = 1000
EPS = 1e-5
HALF = IMG // 4 // 2 - 1    # 15
PRUNE = 0.9

# f32 mega-weight column layout ([128, WCOLS])
W1X_OFF = 0                  # 3 dx-taps x [128,128] for conv1
W2D_OFF = W1X_OFF + 3 * 128  # 9 taps x [128,128] diag-dup for conv2
W3_OFF = W2D_OFF + 9 * 128   # [128,128] conv3 (w3 stacked twice on K)
FCW_OFF = W3_OFF + 128       # 2 kg x [128, 1000] fc lhsT (x0.25 pooled)
FCB_OFF = FCW_OFF + 2 * 1000  # [128, 8] fc bias chunks
BN1_OFF = FCB_OFF + 8          # [128, 2] bn1 gamma/beta (dup across halves)
BN2_OFF = BN1_OFF + 2          # [128, 2]
NGB_OFF = BN2_OFF + 2          # [128, 8] instnorm gamma/beta per group
KV_OFF = NGB_OFF + 8           # [128, 36] dw k vectors (g*9+t)
WCOLS = KV_OFF + 36

# fp16 weight layout ([128, W16COLS])
DW16_OFF = 0                   # 36 taps x [128,128] diag (g*9+t)
MIX16_OFF = DW16_OFF + 36 * 128  # 4 kg x [128, 512] = w_eff.T blocks
W16COLS = MIX16_OFF + 4 * 512

XP_BUFS = 20

# engine split for the main-loop depthwise conv: per group, per tap-index
# 'P' = whole group on PE (psum); otherwise per-tap: 'V' = DVE mul+add pair,
# 'A' = ACT product + DVE add, 'G' = Pool fused scalar_tensor_tensor.
# Tap 0 (the full-coverage (1,1) tap) of a non-PE group always inits on DVE.
DW_PLAN = {
    0: 'P',
    1: 'P',
    2: ['V', 'A', 'A', 'A', 'A', 'A', 'A', 'A', 'V'],
    3: ['V', 'V', 'V', 'V', 'V', 'G', 'G', 'G', 'G'],
}

# tap order: full-coverage tap first (start=True zeroes the psum region)
TAPS = [(1, 1), (0, 0), (0, 1), (0, 2), (1, 0), (1, 2), (2, 0), (2, 1), (2, 2)]


def _clip(lo, hi, lo2, hi2):
    return max(lo, lo2), min(hi, hi2)


def build_nc():
    nc = bacc.Bacc(num_devices=N_CORES)
    x4 = nc.dram_tensor("x4", [BPC, 3, IMG, IMG], F32R, kind="ExternalInput").ap()
    wts = nc.dram_tensor("wts", [128, WCOLS], F32R, kind="ExternalInput").ap()
    wts16 = nc.dram_tensor("wts16", [128, W16COLS], F16,
                           kind="ExternalInput").ap()
    y4 = nc.dram_tensor("y4", [BPC, OUT], F32, kind="ExternalOutput").ap()

    with tile.TileContext(nc) as tc:
        with (
            tc.tile_pool(name="wp", bufs=1) as wp,
            tc.tile_pool(name="wp16", bufs=1) as wp16,
            tc.tile_pool(name="small", bufs=1) as small,
            tc.tile_pool(name="psA", bufs=2, space="PSUM") as psA,
            tc.tile_pool(name="psB", bufs=3, space="PSUM") as psB,
            tc.tile_pool(name="dram", bufs=1, space="DRAM") as dram,
        ):
            w_sb = wp.tile([128, WCOLS], F32R)
            nc.sync.dma_start(out=w_sb, in_=wts)
            w32 = w_sb.bitcast(F32)
            w16 = wp16.tile([128, W16COLS], F16)
            nc.sync.dma_start(out=w16, in_=wts16)

            def wcols(off, n):
                return w_sb[:, off:off + n]

            def w16cols(off, n):
                return w16[:, off:off + n]

            def kvec(g, t):
                return w32[:, KV_OFF + g * 9 + t:KV_OFF + g * 9 + t + 1]

            eps_t = small.tile([128, 1], F32)
            nc.vector.memset(eps_t, EPS)
            z32 = small.tile([128, 64], F32)
            nc.vector.memset(z32, 0.0)
            z16 = small.tile([128, 32], F16)
            nc.vector.memset(z16, 0.0)

            # ---------------- downsample ----------------
            with tc.tile_pool(name="ds1", bufs=1) as ds1:
                # im2col9: partition p = 32*s + 3*dy + c ; free = (oy 64, ix' 130)
                # ix' = ix + 1 (x padded by 1 on both sides)
                im9 = ds1.tile([128, 64 * 130], F32R)
                im9r = im9.rearrange("p (y x) -> p y x", y=64, x=130)
                # zero the x pads (cols 0 and 129)
                for xc in (0, 129):
                    im9_pads = bass.AP(tensor=im9.tensor,
                                       offset=im9.offset + xc,
                                       ap=[im9.ap[0], [130, 64]])
                    nc.vector.tensor_copy(out=im9_pads, in_=z32[:, 0:64])
                # row oy=0 is out of range for dy=0 taps: zero it everywhere
                # first (dy=1/2 loads overwrite their row 0 afterwards; cols
                # 0/129 are the x-pads zeroed above)
                nc.vector.tensor_copy(out=im9[:, 1:65], in_=z32[:, 0:64])
                nc.vector.tensor_copy(out=im9[:, 65:129], in_=z32[:, 0:64])
                # x rows: iy = 2*oy + dy - 1
                # partition base: sample s -> 64*(s%2) + 27*(s//2)
                x4r = x4.rearrange("s c (y2 two) x -> s c y2 two x", two=2)
                for s in range(BPC):
                    for dy in range(3):
                        p0 = 64 * (s % 2) + 27 * (s // 2) + 3 * dy
                        if dy == 0:
                            # oy in [1,64): iy = 2*(oy-1)+1
                            nc.sync.dma_start(
                                out=im9r[p0:p0 + 3, 1:64, 1:129],
                                in_=x4r[s, :, 0:63, 1, :])
                        elif dy == 1:
                            nc.sync.dma_start(
                                out=im9r[p0:p0 + 3, :, 1:129],
                                in_=x4r[s, :, :, 0, :])
                        else:
                            nc.sync.dma_start(
                                out=im9r[p0:p0 + 3, :, 1:129],
                                in_=x4r[s, :, :, 1, :])

                # conv1: out h1 [128 = 64*(s//2)+ch, (s%2, oy 64, ox 64)]
                h1 = ds1.tile([128, 8192], F32)
                h1r = h1.rearrange("p (sh y x) -> p sh y x", sh=2, y=64, x=64)
                # im2col x-read: ix' = 2*ox + dx (x2 = ox + dx//2, tx = dx%2)
                # paired matmul: K=54 block-diag covers samples (q, q+2):
                # out partitions 0-63 <- sample q, 64-127 <- sample q+2.
                im9x = im9.rearrange("p (y x2 two) -> p y x2 two", x2=65, two=2)
                for q in range(2):
                    for yb in range(4):           # 16-oy blocks
                        for h in range(2):
                            pc1 = psA.tile([128, 512], F32, tag="a",
                                           name="pc1")
                            pc1r = pc1.rearrange("p (y x) -> p y x", y=8, x=64)
                            oy0 = yb * 16 + h * 8
                            for di, dx in enumerate([1, 0, 2]):
                                rhs = im9x[64 * q:64 * q + 54, oy0:oy0 + 8,
                                           dx // 2:dx // 2 + 64, dx % 2]
                                lhsT = w_sb[64 * q:64 * q + 54,
                                            W1X_OFF + di_col(dx) * 128:
                                            W1X_OFF + di_col(dx) * 128 + 128]
                                nc.tensor.matmul(pc1r, lhsT, rhs,
                                                 start=(di == 0), stop=(di == 2),
                                                 tile_position=(64 * q, 0))
                            if (q + yb + h) % 2 == 0:
                                nc.scalar.copy(
                                    out=h1r[:, q, oy0:oy0 + 8, :], in_=pc1)
                            else:
                                nc.vector.tensor_copy(
                                    out=h1r[:, q, oy0:oy0 + 8, :], in_=pc1)

                # BN1 partial stats
                st1 = small.tile([128, 16, 6], F32)
                for i in range(16):
                    nc.vector.bn_stats(out=st1[:, i, :],
                                       in_=h1[:, i * 512:(i + 1) * 512])
                mv1 = small.tile([128, 2], F32)
                nc.vector.bn_aggr(out=mv1, in_=st1)
                sums1 = small.tile([128, 2], F32)
                tmp1 = small.tile([128, 1], F32)
                nc.vector.tensor_scalar_mul(out=sums1[:, 0:1], in0=mv1[:, 0:1],
                                            scalar1=8192.0)
                nc.vector.tensor_mul(out=tmp1, in0=mv1[:, 0:1], in1=mv1[:, 0:1])
                nc.vector.tensor_add(out=tmp1, in0=tmp1, in1=mv1[:, 1:2])
                nc.vector.tensor_scalar_mul(out=sums1[:, 1:2], in0=tmp1,
                                            scalar1=8192.0)
                bn1_in = dram.tile([128, 2], F32)
                bn1_out = dram.tile([128, 2], F32)
                nc.gpsimd.dma_start(out=bn1_in, in_=sums1)
                nc.gpsimd.collective_compute(
                    "AllReduce", mybir.AluOpType.add,
                    replica_groups=[list(range(N_CORES))],
                    ins=[bn1_in.opt()], outs=[bn1_out.opt()])
                red1 = small.tile([128, 2], F32)
                nc.gpsimd.dma_start(out=red1, in_=bn1_out)
                comb1 = small.tile([128, 2], F32)
                nc.gpsimd.dma_start(out=comb1[0:64, :], in_=red1[0:64, :])
                nc.gpsimd.dma_start(out=comb1[0:64, :], in_=red1[64:128, :],
                                    accum_op=mybir.AluOpType.add)
                # scale/bias on rows 0:64, then duplicate
                s1t1 = small.tile([128, 2], F32)
                _bn_scale_bias(nc, s1t1, comb1, w32, BN1_OFF, 131072.0,
                               eps_t, small, rows=64)
                nc.gpsimd.dma_start(out=s1t1[64:128, :], in_=s1t1[0:64, :])

                # apply BN1 + relu -> h1n (f32r), x padded to 66 (ix' = ix+1)
                h1n = ds1.tile([128, 2 * 64 * 66], F32R)
                h1nr3 = h1n.rearrange("p (sh y x) -> p sh y x",
                                      sh=2, y=64, x=66)
                for sh in range(2):
                    for xc in (0, 65):
                        h1n_pads = bass.AP(tensor=h1n.tensor,
                                           offset=h1n.offset + 4224 * sh + xc,
                                           ap=[h1n.ap[0], [66, 64]])
                        nc.vector.tensor_copy(out=h1n_pads, in_=z32[:, 0:64])
                h1r4 = h1.rearrange("p (sh y x) -> p sh y x", sh=2, y=64, x=64)
                for sh in range(2):
                    nc.scalar.activation(out=h1nr3[:, sh, :, 1:65],
                                         in_=h1r4[:, sh, :, :], func=AF.Relu,
                                         scale=s1t1[:, 0:1], bias=s1t1[:, 1:2])

                # conv2: depthwise 3x3 stride 2 -> d2 [128, (sh, 32, 32)]
                # row iy = 2*oy + dy - 1 (unpadded), col ix' = 2*ox + dx (padded)
                h1nr = h1n.rearrange(
                    "p (sh y2 ty x2 tx) -> p sh y2 ty x2 tx",
                    sh=2, y2=32, ty=2, x2=33, tx=2)
                d2 = ds1.tile([128, 2048], F32R)
                for sh in range(2):
                    pd2 = psB.tile([128, 1024], F32, tag="b", name="pd2")
                    pd2r = pd2.rearrange("p (h y x) -> p h y x", h=2, y=16, x=32)
                    for h in range(2):
                        for ti, (dy, dx) in enumerate(TAPS):
                            oy0, oy1 = _clip(h * 16, h * 16 + 16,
                                             1 if dy == 0 else 0, 32)
                            if dy == 1:
                                ys, par = oy0, 0
                            elif dy == 0:
                                ys, par = oy0 - 1, 1
                            else:
                                ys, par = oy0, 1
                            rhs = h1nr[:, sh, ys:ys + (oy1 - oy0), par,
                                       dx // 2:dx // 2 + 32, dx % 2]
                            outp = pd2r[:, h, oy0 - h * 16:oy1 - h * 16, :]
                            t = TAPS.index((dy, dx))
                            nc.tensor.matmul(
                                outp, wcols(W2D_OFF + t * 128, 128), rhs,
                                start=(ti == 0), stop=(ti == len(TAPS) - 1))
                    nc.scalar.copy(out=d2[:, sh * 1024:(sh + 1) * 1024], in_=pd2)

                # conv3: 1x1, 64 -> 128 ; h3 [128=outc, (s, 1024px)]
                h3 = small.tile([128, 4096], F32)
                for a in range(2):
                    for nb in range(4):
                        pc3 = psA.tile([128, 512], F32, tag="a",
                                       name=f"pc3_{a}_{nb}")
                        nc.tensor.matmul(
                            pc3,
                            w_sb[64 * a:64 * a + 64, W3_OFF:W3_OFF + 128],
                            d2[64 * a:64 * a + 64, nb * 512:(nb + 1) * 512],
                            start=True, stop=True)
                        s_full = 2 * a + nb // 2
                        dst = h3[:, s_full * 1024 + (nb % 2) * 512:
                                 s_full * 1024 + (nb % 2) * 512 + 512]
                        if nb % 2 == 0:
                            nc.scalar.copy(out=dst, in_=pc3)
                        else:
                            nc.vector.tensor_copy(out=dst, in_=pc3)

                # BN2 stats + allreduce
                st2 = small.tile([128, 8, 6], F32)
                for i in range(8):
                    nc.vector.bn_stats(out=st2[:, i, :],
                                       in_=h3[:, i * 512:(i + 1) * 512])
                mv2 = small.tile([128, 2], F32)
                nc.vector.bn_aggr(out=mv2, in_=st2)
                sums2 = small.tile([128, 2], F32)
                tmp2 = small.tile([128, 1], F32)
                nc.vector.tensor_scalar_mul(out=sums2[:, 0:1], in0=mv2[:, 0:1],
                                            scalar1=4096.0)
                nc.vector.tensor_mul(out=tmp2, in0=mv2[:, 0:1], in1=mv2[:, 0:1])
                nc.vector.tensor_add(out=tmp2, in0=tmp2, in1=mv2[:, 1:2])
                nc.vector.tensor_scalar_mul(out=sums2[:, 1:2], in0=tmp2,
                                            scalar1=4096.0)
                bn2_in = dram.tile([128, 2], F32)
                bn2_out = dram.tile([128, 2], F32)
                nc.gpsimd.dma_start(out=bn2_in, in_=sums2)
                nc.gpsimd.collective_compute(
                    "AllReduce", mybir.AluOpType.add,
                    replica_groups=[list(range(N_CORES))],
                    ins=[bn2_in.opt()], outs=[bn2_out.opt()])
                red2 = small.tile([128, 2], F32)
                nc.gpsimd.dma_start(out=red2, in_=bn2_out)
                s2t2 = small.tile([128, 2], F32)
                _bn_scale_bias(nc, s2t2, red2, w32, BN2_OFF, 32768.0,
                               eps_t, small, rows=128)

            # ---------------- main loop ----------------
            with (
                tc.tile_pool(name="xp", bufs=XP_BUFS) as xp,
                tc.tile_pool(name="dp", bufs=3) as dp,
                tc.tile_pool(name="pp", bufs=3) as pp,
                tc.tile_pool(name="stp", bufs=4) as stp,
            ):
                def new_x_tile(name):
                    # pad columns (0, 33) of every xp slot were zeroed once
                    # below; applies only write the interior, so they persist.
                    return xp.tile([128, 32 * 34], F16, tag="X", name=name)

                # one-time zeroing of the pad columns of all X slots: the
                # dummies are simultaneously live (kept alive by the reads
                # below), so by pigeonhole they cover all slots.
                _dummies = []
                for i in range(XP_BUFS):
                    zt = xp.tile([128, 32 * 34], F16, tag="X", name=f"xz{i}")
                    for xc in (0, 33):
                        pads = bass.AP(tensor=zt.tensor, offset=zt.offset + xc,
                                       ap=[zt.ap[0], [34, 32]])
                        nc.vector.tensor_copy(out=pads, in_=z16)
                    _dummies.append(zt)
                _pad_scratch = small.tile([128, 1], F16)
                for zt in _dummies:
                    nc.scalar.copy(out=_pad_scratch, in_=zt[:, 0:1])

                Xcur = {}
                for s in range(BPC):
                    xt = new_x_tile(f"X1_0_{s}")
                    xtr = xt.rearrange("p (y x) -> p y x", y=32, x=34)
                    h3r = h3.rearrange("p (s y x) -> p s y x", s=4, y=32, x=32)
                    nc.scalar.activation(out=xtr[:, :, 1:33],
                                         in_=h3r[:, s, :, :],
                                         func=AF.Relu,
                                         scale=s2t2[:, 0:1], bias=s2t2[:, 1:2])
                    Xcur[(0, s)] = xt

                pooled_in = small.tile([128, 2, 4, 4], F32)

                for L in range(1, LAYERS + 1):
                    gs_in = sorted({g for (g, _s) in Xcur})
                    mgs = [2, 3] if L == LAYERS else [0, 1, 2, 3]
                    pe_gs = [g for g in gs_in
                             if DW_PLAN[g] == 'P' or len(gs_in) == 1]
                    sb_gs = [g for g in gs_in if g not in pe_gs]
                    Xnext = {}
                    d16s = {}
                    for s in range(BPC):
                        d16s[s] = dp.tile([128, 4096], F16, tag="D",
                                          name=f"D{L}_{s}")
                    for sp in ((0, 1), (2, 3)):
                        # phase A: PE depthwise for this sample pair
                        for s in sp:
                            Xg = {g: Xcur[(g, s)].rearrange(
                                "p (y x) -> p y x", y=32, x=34)
                                for g in gs_in}
                            for g in pe_gs:
                                for h in range(2):
                                    pdw = psA.tile([128, 512], F32, tag="a",
                                                   name=f"pdw{L}_{s}_{g}_{h}")
                                    pdwr = pdw.rearrange("p (y x) -> p y x",
                                                         y=16, x=32)
                                    for ti, (dy, dx) in enumerate(TAPS):
                                        oy0, oy1 = _clip(h * 16, h * 16 + 16,
                                                         max(0, 1 - dy),
                                                         33 - dy)
                                        if oy0 >= oy1:
                                            continue
                                        rhs = Xg[g][:, oy0 + dy - 1:
                                                    oy1 + dy - 1, dx:dx + 32]
                                        outp = pdwr[:, oy0 - h * 16:
                                                    oy1 - h * 16, :]
                                        t = TAPS.index((dy, dx))
                                        nc.tensor.matmul(
                                            outp,
                                            w16cols(DW16_OFF
                                                    + (g * 9 + t) * 128, 128),
                                            rhs,
                                            start=(ti == 0),
                                            stop=(ti == len(TAPS) - 1))
                                    nc.scalar.copy(
                                        out=d16s[s][:, g * 1024 + h * 512:
                                                    g * 1024 + h * 512 + 512],
                                        in_=pdw)
                        # phase B: SBUF depthwise chains, tap-interleaved
                        # across the sample pair and groups
                        if sb_gs:
                            for s in sp:
                                d16r = d16s[s].rearrange(
                                    "p (g y x) -> p g y x", g=4, y=32, x=32)
                                Xr = {g: Xcur[(g, s)].rearrange(
                                    "p (y x) -> p y x", y=32, x=34)
                                    for g in sb_gs}
                                for g in sb_gs:
                                    dy0, dx0 = TAPS[0]
                                    nc.vector.tensor_scalar_mul(
                                        out=d16r[:, g],
                                        in0=Xr[g][:, :, dx0:dx0 + 32],
                                        scalar1=kvec(g, 0))
                            for ti, (dy, dx) in enumerate(TAPS[1:], 1):
                                for s in sp:
                                    d16r = d16s[s].rearrange(
                                        "p (g y x) -> p g y x",
                                        g=4, y=32, x=32)
                                    for g in sb_gs:
                                        Xr = Xcur[(g, s)].rearrange(
                                            "p (y x) -> p y x", y=32, x=34)
                                        oy0 = max(0, 1 - dy)
                                        oy1 = min(32, 33 - dy)
                                        xin = Xr[:, oy0 + dy - 1:oy1 + dy - 1,
                                                 dx:dx + 32]
                                        dslice = d16r[:, g, oy0:oy1, :]
                                        eng = DW_PLAN[g][ti]
                                        prod = pp.tile(
                                            [128, 1024], F16, tag="P",
                                            name=f"P{L}_{s}_{g}_{ti}")
                                        pr = prod.rearrange(
                                            "p (y x) -> p y x", y=32, x=32)
                                        if eng == 'A':
                                            nc.scalar.activation(
                                                out=pr[:, oy0:oy1, :],
                                                in_=xin, func=AF.Identity,
                                                scale=kvec(g, ti))
                                            nc.vector.tensor_add(
                                                out=dslice, in0=dslice,
                                                in1=pr[:, oy0:oy1, :])
                                        elif eng == 'G':
                                            nc.gpsimd.tensor_scalar_mul(
                                                out=pr[:, oy0:oy1, :],
                                                in0=xin, scalar1=kvec(g, ti))
                                            nc.gpsimd.tensor_add(
                                                out=dslice, in0=dslice,
                                                in1=pr[:, oy0:oy1, :])
                                        else:
                                            nc.vector.tensor_scalar_mul(
                                                out=pr[:, oy0:oy1, :],
                                                in0=xin, scalar1=kvec(g, ti))
                                            nc.vector.tensor_add(
                                                out=dslice, in0=dslice,
                                                in1=pr[:, oy0:oy1, :])
                        # phase C: mix + instnorm + apply for the pair
                        for s in sp:
                            d16 = d16s[s]
                            for mg in mgs:
                                pm = psB.tile([128, 1024], F32, tag="b",
                                              name=f"pm{L}_{s}_{mg}")
                                for h in range(2):
                                    for ki, kg in enumerate(gs_in):
                                        nc.tensor.matmul(
                                            pm[:, h * 512:h * 512 + 512],
                                            w16cols(MIX16_OFF + kg * 512
                                                    + mg * 128, 128),
                                            d16[:, kg * 1024 + h * 512:
                                                kg * 1024 + h * 512 + 512],
                                            start=(ki == 0),
                                            stop=(ki == len(gs_in) - 1))
                                st = stp.tile([128, 2, 6], F32, tag="st")
                                nc.vector.bn_stats(out=st[:, 0, :],
                                                   in_=pm[:, 0:512])
                                nc.vector.bn_stats(out=st[:, 1, :],
                                                   in_=pm[:, 512:1024])
                                mv = stp.tile([128, 2], F32, tag="mv")
                                nc.vector.bn_aggr(out=mv, in_=st)
                                sc = stp.tile([128, 1], F32, tag="sc")
                                tt = stp.tile([128, 1], F32, tag="tt")
                                nc.scalar.activation(out=sc, in_=mv[:, 1:2],
                                                     func=AF.Sqrt, bias=eps_t)
                                nc.vector.reciprocal(out=sc, in_=sc)
                                nc.vector.tensor_scalar_mul(
                                    out=sc, in0=sc,
                                    scalar1=w32[:, NGB_OFF + 2 * mg:
                                                NGB_OFF + 2 * mg + 1])
                                nc.vector.tensor_mul(out=tt, in0=mv[:, 0:1],
                                                     in1=sc)
                                nc.vector.tensor_scalar(
                                    out=tt, in0=tt, scalar1=-1.0,
                                    scalar2=w32[:, NGB_OFF + 2 * mg + 1:
                                                NGB_OFF + 2 * mg + 2],
                                    op0=mybir.AluOpType.mult,
                                    op1=mybir.AluOpType.add)
                                if L < LAYERS:
                                    xt = new_x_tile(f"X{L + 1}_{mg}_{s}")
                                    Xnext[(mg, s)] = xt
                                    xtr = xt.rearrange("p (y x) -> p y x",
                                                       y=32, x=34)
                                    pmr2 = pm.rearrange("p (y x) -> p y x",
                                                        y=32, x=32)
                                    nc.scalar.activation(
                                        out=xtr[:, :, 1:33], in_=pmr2,
                                        func=AF.Relu, scale=sc, bias=tt)
                                else:
                                    pmr = pm.rearrange("p (y x) -> p y x",
                                                       y=32, x=32)
                                    nc.scalar.activation(
                                        out=pooled_in[:, mg - 2, s, :],
                                        in_=pmr[:, HALF - 1:HALF + 1,
                                                HALF - 1:HALF + 1],
                                        func=AF.Identity, scale=sc, bias=tt)
                    Xcur = Xnext

                # ---------------- readout ----------------
                tadd = small.tile([128, 2, 4], F32)
                tadd2 = small.tile([128, 2, 4], F32)
                pooled = small.tile([128, 2, 4], F32R)
                nc.vector.tensor_add(out=tadd, in0=pooled_in[:, :, :, 0],
                                     in1=pooled_in[:, :, :, 1])
                nc.vector.tensor_add(out=tadd2, in0=pooled_in[:, :, :, 2],
                                     in1=pooled_in[:, :, :, 3])
                nc.vector.tensor_add(out=pooled, in0=tadd, in1=tadd2)
                y_sb = small.tile([128, 4, 8], F32)
                for mo in range(8):
                    mlen = 128 if mo < 7 else OUT - 7 * 128
                    pf = psA.tile([128, 512], F32, tag="a", name=f"pf{mo}")
                    for kgi in range(2):
                        nc.tensor.matmul(
                            pf[0:mlen, 0:4],
                            w_sb[:, FCW_OFF + kgi * 1000 + mo * 128:
                                 FCW_OFF + kgi * 1000 + mo * 128 + mlen],
                            pooled[:, kgi, :],
                            start=(kgi == 0), stop=(kgi == 1))
                    nc.scalar.activation(
                        out=y_sb[0:mlen, :, mo], in_=pf[0:mlen, 0:4],
                        func=AF.Identity,
                        bias=w32[0:mlen, FCB_OFF + mo:FCB_OFF + mo + 1],
                        scale=1.0)
                for s in range(BPC):
                    dst1 = bass.AP(tensor=y4.tensor, offset=OUT * s,
                                   ap=[[1, 128], [128, 7]])
                    nc.sync.dma_start(out=dst1, in_=y_sb[:, s, 0:7])
                    dst2 = bass.AP(tensor=y4.tensor, offset=OUT * s + 896,
                                   ap=[[1, 104]])
                    nc.sync.dma_start(out=dst2, in_=y_sb[0:104, s, 7])

    nc.finalize()
    return nc


def di_col(dx):
    # column index of conv1 tap dx within w1x block (emission order 1,0,2)
    return {1: 0, 0: 1, 2: 2}[dx]


def _bn_scale_bias(nc, out_st, sums, w32, gb_off, n_tot, eps_t, pool, rows):
    """out_st[:rows, 0] = gamma*rsqrt(var+eps); out_st[:rows, 1] = beta - mu*scale."""
    r = slice(0, rows)
    mu = pool.tile([128, 1], F32, name=f"mu{gb_off}")
    ex2 = pool.tile([128, 1], F32, name=f"ex2{gb_off}")
    var = pool.tile([128, 1], F32, name=f"var{gb_off}")
    nc.vector.tensor_scalar_mul(out=mu[r], in0=sums[r, 0:1], scalar1=1.0 / n_tot)
    nc.vector.tensor_scalar_mul(out=ex2[r], in0=sums[r, 1:2], scalar1=1.0 / n_tot)
    nc.vector.tensor_mul(out=var[r], in0=mu[r], in1=mu[r])
    nc.vector.tensor_sub(out=var[r], in0=ex2[r], in1=var[r])
    nc.scalar.activation(out=var[r], in_=var[r], func=AF.Sqrt, bias=eps_t[r])
    nc.vector.reciprocal(out=var[r], in_=var[r])
    nc.vector.tensor_scalar_mul(out=out_st[r, 0:1], in0=var[r],
                                scalar1=w32[r, gb_off:gb_off + 1])
    nc.vector.tensor_mul(out=mu[r], in0=mu[r], in1=out_st[r, 0:1])
    nc.vector.tensor_scalar(out=out_st[r, 1:2], in0=mu[r], scalar1=-1.0,
                            scalar2=w32[r, gb_off + 1:gb_off + 2],
                            op0=mybir.AluOpType.mult,
                            op1=mybir.AluOpType.add)


def _pack_weights(ds_w1, ds_w2, ds_w3, conv_w, graph_w, fc_w, fc_b,
                  bn1_g, bn1_b, bn2_g, bn2_b, norm_g, norm_b):
    wts = np.zeros((128, WCOLS), np.float32)
    w16 = np.zeros((128, W16COLS), np.float16)
    # pruned graph weight
    k = int((1.0 - PRUNE) * DIM * DIM)
    a = np.abs(graph_w).ravel()
    thresh = np.partition(a, -k)[-k]
    w_eff = np.where(np.abs(graph_w) >= thresh, graph_w, 0.0).astype(np.float32)
    # conv1 taps, paired block-diag:
    # rows 64*q + 27*a + 3*dy + c, cols 64*a + o = w1[o, c, dy, dx]
    for dx in range(3):
        dc = di_col(dx)
        blk = np.zeros((128, 128), np.float32)
        for qq in range(2):
            for aa in range(2):
                for dy in range(3):
                    for c in range(3):
                        blk[64 * qq + 27 * aa + 3 * dy + c,
                            64 * aa:64 * aa + 64] = ds_w1[:, c, dy, dx]
        wts[:, W1X_OFF + dc * 128:W1X_OFF + (dc + 1) * 128] = blk
    # conv2 diag-dup taps
    for t, (dy, dx) in enumerate(TAPS):
        blk = np.zeros((128, 128), np.float32)
        d = ds_w2[:, 0, dy, dx]
        for aa in range(2):
            idx = np.arange(64)
            blk[64 * aa + idx, 64 * aa + idx] = d
        wts[:, W2D_OFF + t * 128:W2D_OFF + (t + 1) * 128] = blk
    # conv3: [64a + c, o] = w3[o, c]
    w3 = ds_w3[:, :, 0, 0]  # [128, 64]
    wts[0:64, W3_OFF:W3_OFF + 128] = w3.T
    wts[64:128, W3_OFF:W3_OFF + 128] = w3.T
    # main dw diag taps (fp16)
    for g in range(4):
        for t, (dy, dx) in enumerate(TAPS):
            blk = np.zeros((128, 128), np.float16)
            idx = np.arange(128)
            blk[idx, idx] = conv_w[g * 128:(g + 1) * 128, 0, dy, dx]
            off = DW16_OFF + (g * 9 + t) * 128
            w16[:, off:off + 128] = blk
    # dw k vectors for the DVE path (f32)
    for g in range(4):
        for t, (dy, dx) in enumerate(TAPS):
            wts[:, KV_OFF + g * 9 + t] = conv_w[g * 128:(g + 1) * 128, 0, dy, dx]
    # mix (fp16): [p, kg*512 + mg*128 + j] = w_eff[mg*128 + j, kg*128 + p]
    weT = w_eff.T  # [in, out]
    for kg in range(4):
        w16[:, MIX16_OFF + kg * 512:MIX16_OFF + (kg + 1) * 512] = \
            weT[kg * 128:(kg + 1) * 128, :].astype(np.float16)
    # fc: [p, kg*1000 + m] = 0.25 * fc_w[m, kg*128 + p]
    for kg in range(2):
        wts[:, FCW_OFF + kg * 1000:FCW_OFF + (kg + 1) * 1000] = \
            0.25 * fc_w[:, kg * 128:(kg + 1) * 128].T
    # fc bias [p, mo]
    fcb = np.zeros((128, 8), np.float32)
    fb = np.zeros(1024, np.float32)
    fb[:OUT] = fc_b
    fcb[:, :] = fb.reshape(8, 128).T
    wts[:, FCB_OFF:FCB_OFF + 8] = fcb
    # bn gammas/betas
    wts[0:64, BN1_OFF] = bn1_g
    wts[64:128, BN1_OFF] = bn1_g
    wts[0:64, BN1_OFF + 1] = bn1_b
    wts[64:128, BN1_OFF + 1] = bn1_b
    wts[:, BN2_OFF] = bn2_g
    wts[:, BN2_OFF + 1] = bn2_b
    for g in range(4):
        wts[:, NGB_OFF + 2 * g] = norm_g[g * 128:(g + 1) * 128]
        wts[:, NGB_OFF + 2 * g + 1] = norm_b[g * 128:(g + 1) * 128]
    return wts, w16


_nc_cache = None
last_results = None


def kernel(**inputs):
    global _nc_cache, last_results
    inputs = {k: np.asarray(v, np.float32) for k, v in inputs.items()}
    wts, w16 = _pack_weights(
        inputs["ds_w1"], inputs["ds_w2"], inputs["ds_w3"], inputs["conv_w"],
        inputs["graph_w"], inputs["fc_w"], inputs["fc_b"],
        inputs["bn1_g"], inputs["bn1_b"], inputs["bn2_g"], inputs["bn2_b"],
        inputs["norm_g"], inputs["norm_b"])
    x = inputs["x"]
    if _nc_cache is None:
        _nc_cache = build_nc()
    nc = _nc_cache
    in_maps = [{"x4": np.ascontiguousarray(x[c * BPC:(c + 1) * BPC]),
                "wts": wts, "wts16": w16} for c in range(N_CORES)]
    res = run_bass_kernel_spmd(nc, in_maps, core_ids=list(range(N_CORES)))
    last_results = res
    return np.concatenate([res.results[c]["y4"] for c in range(N_CORES)], axis=0)


# revision 18
# speedup vs baseline: 1.5014x; 1.2454x over previous
"""Trainium2 Bass kernel for nn_DiscreteTimeNeuralGraph.

Strategy (8 NeuronCores, batch-parallel, engine-balanced):
  - Shard the batch of 32 across 8 cores (4 samples each); weights replicated.
  - Downsample path on-device; BatchNorm batch stats via per-core partial
    sums + one tiny AllReduce each.
  - Main loop in fp16 storage (X, D, weights; fp32 PSUM accumulation):
    depthwise 3x3 conv groups 0-2 as rect-clipped diagonal matmuls on PE;
    group 3 computed on the Vector engine as tensor_scalar(mul, 4x mode) +
    tensor_tensor(add, 2x mode) chains writing fp16 SBUF directly.
    PSUM->SBUF depthwise results copied (and cast to fp16) on the Pool
    engine, freeing ACT for the instnorm applies.
    Channel mix as fp16 blocked matmuls; instance-norm stats on VectorE;
    instnorm+ReLU fused into one ScalarE activation producing fp16 X.
  - Pad-column zeroing via engine memsets (not DMA).
  - Readout: center 2x2 mean (folded into fc weights) + fc matmul (f32r).

Top-k threshold for the pruned graph weight is computed on host
(np.partition) -- it is weight preprocessing of a replicated input.
"""
import numpy as np

import concourse.bass as bass
import concourse.tile as tile
from concourse import bacc, mybir
from concourse.bass_utils import run_bass_kernel_spmd

F32 = mybir.dt.float32
F32R = mybir.dt.float32r
F16 = mybir.dt.float16
AF = mybir.ActivationFunctionType
ALU = mybir.AluOpType

N_CORES = 8
B = 32
BPC = B // N_CORES          # 4 samples per core
DIM = 512
DS = 128
FEAT = 256
LAYERS = 8
IMG = 128
OUT = 1000
EPS = 1e-5
HALF = IMG // 4 // 2 - 1    # 15
PRUNE = 0.9

# f32 mega-weight column layout ([128, WCOLS])
W1X_OFF = 0                  # 3 dx-taps x [128,128] for conv1
W2D_OFF = W1X_OFF + 3 * 128  # 9 taps x [128,128] diag-dup for conv2
W3_OFF = W2D_OFF + 9 * 128   # [128,128] conv3 (w3 stacked twice on K)
FCW_OFF = W3_OFF + 128       # 2 kg x [128, 1000] fc lhsT (x0.25 pooled)
FCB_OFF = FCW_OFF + 2 * 1000  # [128, 8] fc bias chunks
BN1_OFF = FCB_OFF + 8          # [128, 2] bn1 gamma/beta (dup across halves)
BN2_OFF = BN1_OFF + 2          # [128, 2]
NGB_OFF = BN2_OFF + 2          # [128, 8] instnorm gamma/beta per group
KV_OFF = NGB_OFF + 8           # [128, 36] dw k vectors (g*9+t)
WCOLS = KV_OFF + 36

# fp16 weight layout ([128, W16COLS])
DW16_OFF = 0                   # 36 taps x [128,128] diag (g*9+t)
MIX16_OFF = DW16_OFF + 36 * 128  # 4 kg x [128, 512] = w_eff.T blocks
W16COLS = MIX16_OFF + 4 * 512

XP_BUFS = 20

# engine split for the main-loop depthwise conv: per group, per tap-index
# 'P' = whole group on PE (psum); otherwise per-tap: 'V' = DVE mul+add pair,
# 'A' = ACT product + DVE add, 'G' = Pool fused scalar_tensor_tensor.
# Tap 0 (the full-coverage (1,1) tap) of a non-PE group always inits on DVE.
# PE groups do the depthwise as diagonal matmuls into PSUM; SBUF groups
# compute 9 full-row tap products (engines per PROD table) and combine them
# with a pairwise add tree (engines per ADD table, ops in fixed order:
# P0+=P1, P2+=P3, P4+=P5, P6+=P7, P0+=P2, P4+=P6, P0+=P4, d16=P0+P8).
PE_GROUPS = (0, 1)
DW_PROD = {
    2: ['V', 'A', 'A', 'A', 'A', 'A', 'A', 'A', 'V'],
    3: ['V', 'V', 'G', 'G', 'G', 'G', 'G', 'V', 'V'],
}
DW_ADD = {
    2: ['V', 'V', 'V', 'G', 'V', 'V', 'V', 'V'],
    3: ['V', 'V', 'V', 'G', 'V', 'V', 'G', 'V'],
}
ADD_TREE = [(0, 1), (2, 3), (4, 5), (6, 7), (0, 2), (4, 6), (0, 4)]

# tap order: full-coverage tap first (start=True zeroes the psum region)
TAPS = [(1, 1), (0, 0), (0, 1), (0, 2), (1, 0), (1, 2), (2, 0), (2, 1), (2, 2)]


def _clip(lo, hi, lo2, hi2):
    return max(lo, lo2), min(hi, hi2)


def build_nc():
    nc = bacc.Bacc(num_devices=N_CORES)
    x4 = nc.dram_tensor("x4", [BPC, 3, IMG, IMG], F32R, kind="ExternalInput").ap()
    wts = nc.dram_tensor("wts", [128, WCOLS], F32R, kind="ExternalInput").ap()
    wts16 = nc.dram_tensor("wts16", [128, W16COLS], F16,
                           kind="ExternalInput").ap()
    y4 = nc.dram_tensor("y4", [BPC, OUT], F32, kind="ExternalOutput").ap()

    with tile.TileContext(nc) as tc:
        with (
            tc.tile_pool(name="wp", bufs=1) as wp,
            tc.tile_pool(name="wp16", bufs=1) as wp16,
            tc.tile_pool(name="small", bufs=1) as small,
            tc.tile_pool(name="psA", bufs=3, space="PSUM") as psA,
            tc.tile_pool(name="psB", bufs=2, space="PSUM") as psB,
            tc.tile_pool(name="dram", bufs=1, space="DRAM") as dram,
        ):
            w_sb = wp.tile([128, WCOLS], F32R)
            nc.sync.dma_start(out=w_sb, in_=wts)
            w32 = w_sb.bitcast(F32)
            w16 = wp16.tile([128, W16COLS], F16)
            nc.sync.dma_start(out=w16, in_=wts16)

            def wcols(off, n):
                return w_sb[:, off:off + n]

            def w16cols(off, n):
                return w16[:, off:off + n]

            def kvec(g, t):
                return w32[:, KV_OFF + g * 9 + t:KV_OFF + g * 9 + t + 1]

            eps_t = small.tile([128, 1], F32)
            nc.vector.memset(eps_t, EPS)
            z32 = small.tile([128, 64], F32)
            nc.vector.memset(z32, 0.0)
            z16 = small.tile([128, 64], F16)
            nc.vector.memset(z16, 0.0)

            # ---------------- downsample ----------------
            with tc.tile_pool(name="ds1", bufs=1) as ds1:
                # im2col9: partition p = 32*s + 3*dy + c ; free = (oy 64, ix' 130)
                # ix' = ix + 1 (x padded by 1 on both sides)
                im9 = ds1.tile([128, 64 * 130], F32R)
                im9r = im9.rearrange("p (y x) -> p y x", y=64, x=130)
                # zero the x pads (cols 0 and 129)
                for xc in (0, 129):
                    im9_pads = bass.AP(tensor=im9.tensor,
                                       offset=im9.offset + xc,
                                       ap=[im9.ap[0], [130, 64]])
                    nc.vector.tensor_copy(out=im9_pads, in_=z32[:, 0:64])
                # row oy=0 is out of range for dy=0 taps: zero it everywhere
                # first (dy=1/2 loads overwrite their row 0 afterwards; cols
                # 0/129 are the x-pads zeroed above)
                nc.vector.tensor_copy(out=im9[:, 1:65], in_=z32[:, 0:64])
                nc.vector.tensor_copy(out=im9[:, 65:129], in_=z32[:, 0:64])
                # x rows: iy = 2*oy + dy - 1
                # partition base: sample s -> 64*(s%2) + 27*(s//2)
                x4r = x4.rearrange("s c (y2 two) x -> s c y2 two x", two=2)
                for s in range(BPC):
                    for dy in range(3):
                        p0 = 64 * (s % 2) + 27 * (s // 2) + 3 * dy
                        if dy == 0:
                            # oy in [1,64): iy = 2*(oy-1)+1
                            nc.sync.dma_start(
                                out=im9r[p0:p0 + 3, 1:64, 1:129],
                                in_=x4r[s, :, 0:63, 1, :])
                        elif dy == 1:
                            nc.sync.dma_start(
                                out=im9r[p0:p0 + 3, :, 1:129],
                                in_=x4r[s, :, :, 0, :])
                        else:
                            nc.sync.dma_start(
                                out=im9r[p0:p0 + 3, :, 1:129],
                                in_=x4r[s, :, :, 1, :])

                # conv1: out h1 [128 = 64*(s//2)+ch, (s%2, oy 64, ox 64)]
                h1 = ds1.tile([128, 8192], F32)
                h1r = h1.rearrange("p (sh y x) -> p sh y x", sh=2, y=64, x=64)
                # im2col x-read: ix' = 2*ox + dx (x2 = ox + dx//2, tx = dx%2)
                # paired matmul: K=54 block-diag covers samples (q, q+2):
                # out partitions 0-63 <- sample q, 64-127 <- sample q+2.
                im9x = im9.rearrange("p (y x2 two) -> p y x2 two", x2=65, two=2)
                for q in range(2):
                    for yb in range(4):           # 16-oy blocks
                        for h in range(2):
                            pc1 = psA.tile([128, 512], F32, tag="a",
                                           name="pc1")
                            pc1r = pc1.rearrange("p (y x) -> p y x", y=8, x=64)
                            oy0 = yb * 16 + h * 8
                            for di, dx in enumerate([1, 0, 2]):
                                rhs = im9x[64 * q:64 * q + 54, oy0:oy0 + 8,
                                           dx // 2:dx // 2 + 64, dx % 2]
                                lhsT = w_sb[64 * q:64 * q + 54,
                                            W1X_OFF + di_col(dx) * 128:
                                            W1X_OFF + di_col(dx) * 128 + 128]
                                nc.tensor.matmul(pc1r, lhsT, rhs,
                                                 start=(di == 0), stop=(di == 2),
                                                 tile_position=(64 * q, 0))
                            if (q + yb + h) % 2 == 0:
                                nc.scalar.copy(
                                    out=h1r[:, q, oy0:oy0 + 8, :], in_=pc1)
                            else:
                                nc.vector.tensor_copy(
                                    out=h1r[:, q, oy0:oy0 + 8, :], in_=pc1)

                # BN1 partial stats
                st1 = small.tile([128, 16, 6], F32)
                for i in range(16):
                    nc.vector.bn_stats(out=st1[:, i, :],
                                       in_=h1[:, i * 512:(i + 1) * 512])
                mv1 = small.tile([128, 2], F32)
                nc.vector.bn_aggr(out=mv1, in_=st1)
                sums1 = small.tile([128, 2], F32)
                tmp1 = small.tile([128, 1], F32)
                nc.vector.tensor_scalar_mul(out=sums1[:, 0:1], in0=mv1[:, 0:1],
                                            scalar1=8192.0)
                nc.vector.tensor_mul(out=tmp1, in0=mv1[:, 0:1], in1=mv1[:, 0:1])
                nc.vector.tensor_add(out=tmp1, in0=tmp1, in1=mv1[:, 1:2])
                nc.vector.tensor_scalar_mul(out=sums1[:, 1:2], in0=tmp1,
                                            scalar1=8192.0)
                bn1_in = dram.tile([128, 2], F32)
                bn1_out = dram.tile([128, 2], F32)
                nc.gpsimd.dma_start(out=bn1_in, in_=sums1)
                nc.gpsimd.collective_compute(
                    "AllReduce", mybir.AluOpType.add,
                    replica_groups=[list(range(N_CORES))],
                    ins=[bn1_in.opt()], outs=[bn1_out.opt()])
                red1 = small.tile([128, 2], F32)
                nc.gpsimd.dma_start(out=red1, in_=bn1_out)
                comb1 = small.tile([128, 2], F32)
                nc.gpsimd.dma_start(out=comb1[0:64, :], in_=red1[0:64, :])
                nc.gpsimd.dma_start(out=comb1[0:64, :], in_=red1[64:128, :],
                                    accum_op=mybir.AluOpType.add)
                # scale/bias on rows 0:64, then duplicate
                s1t1 = small.tile([128, 2], F32)
                _bn_scale_bias(nc, s1t1, comb1, w32, BN1_OFF, 131072.0,
                               eps_t, small, rows=64)
                nc.gpsimd.dma_start(out=s1t1[64:128, :], in_=s1t1[0:64, :])

                # apply BN1 + relu -> h1n (f32r), x padded to 66 (ix' = ix+1)
                h1n = ds1.tile([128, 2 * 64 * 66], F32R)
                h1nr3 = h1n.rearrange("p (sh y x) -> p sh y x",
                                      sh=2, y=64, x=66)
                for sh in range(2):
                    for xc in (0, 65):
                        h1n_pads = bass.AP(tensor=h1n.tensor,
                                           offset=h1n.offset + 4224 * sh + xc,
                                           ap=[h1n.ap[0], [66, 64]])
                        nc.vector.tensor_copy(out=h1n_pads, in_=z32[:, 0:64])
                h1r4 = h1.rearrange("p (sh y x) -> p sh y x", sh=2, y=64, x=64)
                for sh in range(2):
                    nc.scalar.activation(out=h1nr3[:, sh, :, 1:65],
                                         in_=h1r4[:, sh, :, :], func=AF.Relu,
                                         scale=s1t1[:, 0:1], bias=s1t1[:, 1:2])

                # conv2: depthwise 3x3 stride 2 -> d2 [128, (sh, 32, 32)]
                # row iy = 2*oy + dy - 1 (unpadded), col ix' = 2*ox + dx (padded)
                h1nr = h1n.rearrange(
                    "p (sh y2 ty x2 tx) -> p sh y2 ty x2 tx",
                    sh=2, y2=32, ty=2, x2=33, tx=2)
                d2 = ds1.tile([128, 2048], F32R)
                for sh in range(2):
                    pd2 = psB.tile([128, 1024], F32, tag="b", name="pd2")
                    pd2r = pd2.rearrange("p (h y x) -> p h y x", h=2, y=16, x=32)
                    for h in range(2):
                        for ti, (dy, dx) in enumerate(TAPS):
                            oy0, oy1 = _clip(h * 16, h * 16 + 16,
                                             1 if dy == 0 else 0, 32)
                            if dy == 1:
                                ys, par = oy0, 0
                            elif dy == 0:
                                ys, par = oy0 - 1, 1
                            else:
                                ys, par = oy0, 1
                            rhs = h1nr[:, sh, ys:ys + (oy1 - oy0), par,
                                       dx // 2:dx // 2 + 32, dx % 2]
                            outp = pd2r[:, h, oy0 - h * 16:oy1 - h * 16, :]
                            t = TAPS.index((dy, dx))
                            nc.tensor.matmul(
                                outp, wcols(W2D_OFF + t * 128, 128), rhs,
                                start=(ti == 0), stop=(ti == len(TAPS) - 1))
                    nc.scalar.copy(out=d2[:, sh * 1024:(sh + 1) * 1024], in_=pd2)

                # conv3: 1x1, 64 -> 128 ; h3 [128=outc, (s, 1024px)]
                h3 = small.tile([128, 4096], F32)
                for a in range(2):
                    for nb in range(4):
                        pc3 = psA.tile([128, 512], F32, tag="a",
                                       name=f"pc3_{a}_{nb}")
                        nc.tensor.matmul(
                            pc3,
                            w_sb[64 * a:64 * a + 64, W3_OFF:W3_OFF + 128],
                            d2[64 * a:64 * a + 64, nb * 512:(nb + 1) * 512],
                            start=True, stop=True)
                        s_full = 2 * a + nb // 2
                        dst = h3[:, s_full * 1024 + (nb % 2) * 512:
                                 s_full * 1024 + (nb % 2) * 512 + 512]
                        if nb % 2 == 0:
                            nc.scalar.copy(out=dst, in_=pc3)
                        else:
                            nc.vector.tensor_copy(out=dst, in_=pc3)

                # BN2 stats + allreduce
                st2 = small.tile([128, 8, 6], F32)
                for i in range(8):
                    nc.vector.bn_stats(out=st2[:, i, :],
                                       in_=h3[:, i * 512:(i + 1) * 512])
                mv2 = small.tile([128, 2], F32)
                nc.vector.bn_aggr(out=mv2, in_=st2)
                sums2 = small.tile([128, 2], F32)
                tmp2 = small.tile([128, 1], F32)
                nc.vector.tensor_scalar_mul(out=sums2[:, 0:1], in0=mv2[:, 0:1],
                                            scalar1=4096.0)
                nc.vector.tensor_mul(out=tmp2, in0=mv2[:, 0:1], in1=mv2[:, 0:1])
                nc.vector.tensor_add(out=tmp2, in0=tmp2, in1=mv2[:, 1:2])
                nc.vector.tensor_scalar_mul(out=sums2[:, 1:2], in0=tmp2,
                                            scalar1=4096.0)
                bn2_in = dram.tile([128, 2], F32)
                bn2_out = dram.tile([128, 2], F32)
                nc.gpsimd.dma_start(out=bn2_in, in_=sums2)
                nc.gpsimd.collective_compute(
                    "AllReduce", mybir.AluOpType.add,
                    replica_groups=[list(range(N_CORES))],
                    ins=[bn2_in.opt()], outs=[bn2_out.opt()])
                red2 = small.tile([128, 2], F32)
                nc.gpsimd.dma_start(out=red2, in_=bn2_out)
                s2t2 = small.tile([128, 2], F32)
                _bn_scale_bias(nc, s2t2, red2, w32, BN2_OFF, 32768.0,
                               eps_t, small, rows=128)

            # ---------------- main loop ----------------
            with (
                tc.tile_pool(name="xp", bufs=XP_BUFS) as xp,
                tc.tile_pool(name="dp", bufs=5) as dp,
                tc.tile_pool(name="pp", bufs=12) as pp,
                tc.tile_pool(name="stp", bufs=4) as stp,
            ):
                def new_x_tile(name):
                    # pad rows (-1, 32) and columns (0, 33) of every xp slot
                    # were zeroed once below; applies only write the interior
                    # (rows 1..32, cols 1..32 of the 34x34 grid).
                    return xp.tile([128, 34 * 34], F16, tag="X", name=name)

                # one-time zeroing of the pad columns of all X slots: the
                # dummies are simultaneously live (kept alive by the reads
                # below), so by pigeonhole they cover all slots.
                _dummies = []
                for i in range(XP_BUFS):
                    zt = xp.tile([128, 34 * 34], F16, tag="X", name=f"xz{i}")
                    # pad rows -1 and 32 (contiguous 34-elem spans)
                    nc.vector.tensor_copy(out=zt[:, 0:34], in_=z16[:, 0:34])
                    nc.vector.tensor_copy(out=zt[:, 1122:1156],
                                          in_=z16[:, 0:34])
                    for xc in (0, 33):
                        pads = bass.AP(tensor=zt.tensor, offset=zt.offset + xc,
                                       ap=[zt.ap[0], [34, 34]])
                        nc.vector.tensor_copy(out=pads, in_=z16[:, 0:34])
                    _dummies.append(zt)
                _pad_scratch = small.tile([128, 1], F16)
                for zt in _dummies:
                    nc.scalar.copy(out=_pad_scratch, in_=zt[:, 0:1])

                Xcur = {}
                for s in range(BPC):
                    xt = new_x_tile(f"X1_0_{s}")
                    xtr = xt.rearrange("p (y x) -> p y x", y=34, x=34)
                    h3r = h3.rearrange("p (s y x) -> p s y x", s=4, y=32, x=32)
                    nc.scalar.activation(out=xtr[:, 1:33, 1:33],
                                         in_=h3r[:, s, :, :],
                                         func=AF.Relu,
                                         scale=s2t2[:, 0:1], bias=s2t2[:, 1:2])
                    Xcur[(0, s)] = xt

                pooled_in = small.tile([128, 2, 4, 4], F32)

                for L in range(1, LAYERS + 1):
                    gs_in = sorted({g for (g, _s) in Xcur})
                    mgs = [2, 3] if L == LAYERS else [0, 1, 2, 3]
                    pe_gs = [g for g in gs_in
                             if g in PE_GROUPS or len(gs_in) == 1]
                    sb_gs = [g for g in gs_in if g not in pe_gs]
                    Xnext = {}
                    d16s = {}
                    for s in range(BPC):
                        d16s[s] = dp.tile([128, 4096], F16, tag="D",
                                          name=f"D{L}_{s}")

                    def xv(g, s, dy, dx):
                        Xr = Xcur[(g, s)].rearrange("p (y x) -> p y x",
                                                    y=34, x=34)
                        return Xr[:, dy:dy + 32, dx:dx + 32]

                    # phase A: PE depthwise (PSUM) + ACT copies to fp16 D
                    for s in range(BPC):
                        for g in pe_gs:
                            for h in range(2):
                                pdw = psA.tile([128, 512], F32, tag="a",
                                               name=f"pdw{L}_{s}_{g}_{h}")
                                pdwr = pdw.rearrange("p (y x) -> p y x",
                                                     y=16, x=32)
                                for ti, (dy, dx) in enumerate(TAPS):
                                    rhs = xv(g, s, dy, dx)[:, h * 16:
                                                           h * 16 + 16, :]
                                    t = TAPS.index((dy, dx))
                                    nc.tensor.matmul(
                                        pdwr,
                                        w16cols(DW16_OFF + (g * 9 + t) * 128,
                                                128),
                                        rhs,
                                        start=(ti == 0),
                                        stop=(ti == len(TAPS) - 1))
                                nc.scalar.copy(
                                    out=d16s[s][:, g * 1024 + h * 512:
                                                g * 1024 + h * 512 + 512],
                                    in_=pdw)

                    # phase B: SBUF depthwise via 9 full-row products and a
                    # pairwise add tree per (sample, group)
                    def emit_prod(eng, out, xin, g, ti):
                        if eng == 'A':
                            nc.scalar.activation(out=out, in_=xin,
                                                 func=AF.Identity,
                                                 scale=kvec(g, ti))
                        elif eng == 'G':
                            nc.gpsimd.tensor_scalar_mul(out=out, in0=xin,
                                                        scalar1=kvec(g, ti))
                        else:
                            nc.vector.tensor_scalar_mul(out=out, in0=xin,
                                                        scalar1=kvec(g, ti))

                    def emit_add(eng, out, in0, in1):
                        if eng == 'G':
                            nc.gpsimd.tensor_add(out=out, in0=in0, in1=in1)
                        else:
                            nc.vector.tensor_add(out=out, in0=in0, in1=in1)

                    for s in range(BPC):
                        prods = {}
                        for g in sb_gs:
                            for ti, (dy, dx) in enumerate(TAPS):
                                prod = pp.tile([128, 1024], F16, tag="P",
                                               name=f"P{L}_{s}_{g}_{ti}")
                                emit_prod(DW_PROD[g][ti], prod,
                                          xv(g, s, dy, dx), g, ti)
                                prods[(g, ti)] = prod
                        for g in sb_gs:
                            for ai, (d, e) in enumerate(ADD_TREE):
                                emit_add(DW_ADD[g][ai], prods[(g, d)],
                                         prods[(g, d)], prods[(g, e)])
                        for g in sb_gs:
                            emit_add(DW_ADD[g][7],
                                     d16s[s][:, g * 1024:g * 1024 + 1024],
                                     prods[(g, 0)], prods[(g, 8)])

                    # phase C: mix + instnorm + apply
                    for s in range(BPC):
                        d16 = d16s[s]
                        for mg in mgs:
                            pm = psB.tile([128, 1024], F32, tag="b",
                                          name=f"pm{L}_{s}_{mg}")
                            for h in range(2):
                                for ki, kg in enumerate(gs_in):
                                    nc.tensor.matmul(
                                        pm[:, h * 512:h * 512 + 512],
                                        w16cols(MIX16_OFF + kg * 512
                                                + mg * 128, 128),
                                        d16[:, kg * 1024 + h * 512:
                                            kg * 1024 + h * 512 + 512],
                                        start=(ki == 0),
                                        stop=(ki == len(gs_in) - 1))
                            st = stp.tile([128, 2, 6], F32, tag="st")
                            nc.vector.bn_stats(out=st[:, 0, :],
                                               in_=pm[:, 0:512])
                            nc.vector.bn_stats(out=st[:, 1, :],
                                               in_=pm[:, 512:1024])
                            mv = stp.tile([128, 2], F32, tag="mv")
                            nc.vector.bn_aggr(out=mv, in_=st)
                            sc = stp.tile([128, 1], F32, tag="sc")
                            tt = stp.tile([128, 1], F32, tag="tt")
                            nc.scalar.activation(out=sc, in_=mv[:, 1:2],
                                                 func=AF.Sqrt, bias=eps_t)
                            nc.vector.reciprocal(out=sc, in_=sc)
                            nc.vector.tensor_scalar_mul(
                                out=sc, in0=sc,
                                scalar1=w32[:, NGB_OFF + 2 * mg:
                                            NGB_OFF + 2 * mg + 1])
                            nc.vector.tensor_mul(out=tt, in0=mv[:, 0:1],
                                                 in1=sc)
                            nc.vector.tensor_scalar(
                                out=tt, in0=tt, scalar1=-1.0,
                                scalar2=w32[:, NGB_OFF + 2 * mg + 1:
                                            NGB_OFF + 2 * mg + 2],
                                op0=mybir.AluOpType.mult,
                                op1=mybir.AluOpType.add)
                            if L < LAYERS:
                                xt = new_x_tile(f"X{L + 1}_{mg}_{s}")
                                Xnext[(mg, s)] = xt
                                xtr = xt.rearrange("p (y x) -> p y x",
                                                   y=34, x=34)
                                pmr2 = pm.rearrange("p (y x) -> p y x",
                                                    y=32, x=32)
                                nc.scalar.activation(
                                    out=xtr[:, 1:33, 1:33], in_=pmr2,
                                    func=AF.Relu, scale=sc, bias=tt)
                            else:
                                pmr = pm.rearrange("p (y x) -> p y x",
                                                   y=32, x=32)
                                nc.scalar.activation(
                                    out=pooled_in[:, mg - 2, s, :],
                                    in_=pmr[:, HALF - 1:HALF + 1,
                                            HALF - 1:HALF + 1],
                                    func=AF.Identity, scale=sc, bias=tt)
                    Xcur = Xnext

                # ---------------- readout ----------------
                tadd = small.tile([128, 2, 4], F32)
                tadd2 = small.tile([128, 2, 4], F32)
                pooled = small.tile([128, 2, 4], F32R)
                nc.vector.tensor_add(out=tadd, in0=pooled_in[:, :, :, 0],
                                     in1=pooled_in[:, :, :, 1])
                nc.vector.tensor_add(out=tadd2, in0=pooled_in[:, :, :, 2],
                                     in1=pooled_in[:, :, :, 3])
                nc.vector.tensor_add(out=pooled, in0=tadd, in1=tadd2)
                y_sb = small.tile([128, 4, 8], F32)
                for mo in range(8):
                    mlen = 128 if mo < 7 else OUT - 7 * 128
                    pf = psA.tile([128, 512], F32, tag="a", name=f"pf{mo}")
                    for kgi in range(2):
                        nc.tensor.matmul(
                            pf[0:mlen, 0:4],
                            w_sb[:, FCW_OFF + kgi * 1000 + mo * 128:
                                 FCW_OFF + kgi * 1000 + mo * 128 + mlen],
                            pooled[:, kgi, :],
                            start=(kgi == 0), stop=(kgi == 1))
                    nc.scalar.activation(
                        out=y_sb[0:mlen, :, mo], in_=pf[0:mlen, 0:4],
                        func=AF.Identity,
                        bias=w32[0:mlen, FCB_OFF + mo:FCB_OFF + mo + 1],
                        scale=1.0)
                for s in range(BPC):
                    dst1 = bass.AP(tensor=y4.tensor, offset=OUT * s,
                                   ap=[[1, 128], [128, 7]])
                    nc.sync.dma_start(out=dst1, in_=y_sb[:, s, 0:7])
                    dst2 = bass.AP(tensor=y4.tensor, offset=OUT * s + 896,
                                   ap=[[1, 104]])
                    nc.sync.dma_start(out=dst2, in_=y_sb[0:104, s, 7])

    nc.finalize()
    return nc


def di_col(dx):
    # column index of conv1 tap dx within w1x block (emission order 1,0,2)
    return {1: 0, 0: 1, 2: 2}[dx]


def _bn_scale_bias(nc, out_st, sums, w32, gb_off, n_tot, eps_t, pool, rows):
    """out_st[:rows, 0] = gamma*rsqrt(var+eps); out_st[:rows, 1] = beta - mu*scale."""
    r = slice(0, rows)
    mu = pool.tile([128, 1], F32, name=f"mu{gb_off}")
    ex2 = pool.tile([128, 1], F32, name=f"ex2{gb_off}")
    var = pool.tile([128, 1], F32, name=f"var{gb_off}")
    nc.vector.tensor_scalar_mul(out=mu[r], in0=sums[r, 0:1], scalar1=1.0 / n_tot)
    nc.vector.tensor_scalar_mul(out=ex2[r], in0=sums[r, 1:2], scalar1=1.0 / n_tot)
    nc.vector.tensor_mul(out=var[r], in0=mu[r], in1=mu[r])
    nc.vector.tensor_sub(out=var[r], in0=ex2[r], in1=var[r])
    nc.scalar.activation(out=var[r], in_=var[r], func=AF.Sqrt, bias=eps_t[r])
    nc.vector.reciprocal(out=var[r], in_=var[r])
    nc.vector.tensor_scalar_mul(out=out_st[r, 0:1], in0=var[r],
                                scalar1=w32[r, gb_off:gb_off + 1])
    nc.vector.tensor_mul(out=mu[r], in0=mu[r], in1=out_st[r, 0:1])
    nc.vector.tensor_scalar(out=out_st[r, 1:2], in0=mu[r], scalar1=-1.0,
                            scalar2=w32[r, gb_off + 1:gb_off + 2],
                            op0=mybir.AluOpType.mult,
                            op1=mybir.AluOpType.add)


def _pack_weights(ds_w1, ds_w2, ds_w3, conv_w, graph_w, fc_w, fc_b,
                  bn1_g, bn1_b, bn2_g, bn2_b, norm_g, norm_b):
    wts = np.zeros((128, WCOLS), np.float32)
    w16 = np.zeros((128, W16COLS), np.float16)
    # pruned graph weight
    k = int((1.0 - PRUNE) * DIM * DIM)
    a = np.abs(graph_w).ravel()
    thresh = np.partition(a, -k)[-k]
    w_eff = np.where(np.abs(graph_w) >= thresh, graph_w, 0.0).astype(np.float32)
    # conv1 taps, paired block-diag:
    # rows 64*q + 27*a + 3*dy + c, cols 64*a + o = w1[o, c, dy, dx]
    for dx in range(3):
        dc = di_col(dx)
        blk = np.zeros((128, 128), np.float32)
        for qq in range(2):
            for aa in range(2):
                for dy in range(3):
                    for c in range(3):
                        blk[64 * qq + 27 * aa + 3 * dy + c,
                            64 * aa:64 * aa + 64] = ds_w1[:, c, dy, dx]
        wts[:, W1X_OFF + dc * 128:W1X_OFF + (dc + 1) * 128] = blk
    # conv2 diag-dup taps
    for t, (dy, dx) in enumerate(TAPS):
        blk = np.zeros((128, 128), np.float32)
        d = ds_w2[:, 0, dy, dx]
        for aa in range(2):
            idx = np.arange(64)
            blk[64 * aa + idx, 64 * aa + idx] = d
        wts[:, W2D_OFF + t * 128:W2D_OFF + (t + 1) * 128] = blk
    # conv3: [64a + c, o] = w3[o, c]
    w3 = ds_w3[:, :, 0, 0]  # [128, 64]
    wts[0:64, W3_OFF:W3_OFF + 128] = w3.T
    wts[64:128, W3_OFF:W3_OFF + 128] = w3.T
    # main dw diag taps (fp16)
    for g in range(4):
        for t, (dy, dx) in enumerate(TAPS):
            blk = np.zeros((128, 128), np.float16)
            idx = np.arange(128)
            blk[idx, idx] = conv_w[g * 128:(g + 1) * 128, 0, dy, dx]
            off = DW16_OFF + (g * 9 + t) * 128
            w16[:, off:off + 128] = blk
    # dw k vectors for the DVE path (f32)
    for g in range(4):
        for t, (dy, dx) in enumerate(TAPS):
            wts[:, KV_OFF + g * 9 + t] = conv_w[g * 128:(g + 1) * 128, 0, dy, dx]
    # mix (fp16): [p, kg*512 + mg*128 + j] = w_eff[mg*128 + j, kg*128 + p]
    weT = w_eff.T  # [in, out]
    for kg in range(4):
        w16[:, MIX16_OFF + kg * 512:MIX16_OFF + (kg + 1) * 512] = \
            weT[kg * 128:(kg + 1) * 128, :].astype(np.float16)
    # fc: [p, kg*1000 + m] = 0.25 * fc_w[m, kg*128 + p]
    for kg in range(2):
        wts[:, FCW_OFF + kg * 1000:FCW_OFF + (kg + 1) * 1000] = \
            0.25 * fc_w[:, kg * 128:(kg + 1) * 128].T
    # fc bias [p, mo]
    fcb = np.zeros((128, 8), np.float32)
    fb = np.zeros(1024, np.float32)
    fb[:OUT] = fc_b
    fcb[:, :] = fb.reshape(8, 128).T
    wts[:, FCB_OFF:FCB_OFF + 8] = fcb
    # bn gammas/betas
    wts[0:64, BN1_OFF] = bn1_g
    wts[64:128, BN1_OFF] = bn1_g
    wts[0:64, BN1_OFF + 1] = bn1_b
    wts[64:128, BN1_OFF + 1] = bn1_b
    wts[:, BN2_OFF] = bn2_g
    wts[:, BN2_OFF + 1] = bn2_b
    for g in range(4):
        wts[:, NGB_OFF + 2 * g] = norm_g[g * 128:(g + 1) * 128]
        wts[:, NGB_OFF + 2 * g + 1] = norm_b[g * 128:(g + 1) * 128]
    return wts, w16


_nc_cache = None
last_results = None


def kernel(**inputs):
    global _nc_cache, last_results
    inputs = {k: np.asarray(v, np.float32) for k, v in inputs.items()}
    wts, w16 = _pack_weights(
        inputs["ds_w1"], inputs["ds_w2"], inputs["ds_w3"], inputs["conv_w"],
        inputs["graph_w"], inputs["fc_w"], inputs["fc_b"],
        inputs["bn1_g"], inputs["bn1_b"], inputs["bn2_g"], inputs["bn2_b"],
        inputs["norm_g"], inputs["norm_b"])
    x = inputs["x"]
    if _nc_cache is None:
        _nc_cache = build_nc()
    nc = _nc_cache
    in_maps = [{"x4": np.ascontiguousarray(x[c * BPC:(c + 1) * BPC]),
                "wts": wts, "wts16": w16} for c in range(N_CORES)]
    res = run_bass_kernel_spmd(nc, in_maps, core_ids=list(range(N_CORES)))
    last_results = res
    return np.concatenate([res.results[c]["y4"] for c in range(N_CORES)], axis=0)


# revision 19
# speedup vs baseline: 1.5253x; 1.0159x over previous
"""Trainium2 Bass kernel for nn_DiscreteTimeNeuralGraph.

Strategy (8 NeuronCores, batch-parallel, engine-balanced):
  - Shard the batch of 32 across 8 cores (4 samples each); weights replicated.
  - Downsample path on-device; BatchNorm batch stats via per-core partial
    sums + one tiny AllReduce each.
  - Main loop in fp16 storage (X, D, weights; fp32 PSUM accumulation):
    depthwise 3x3 conv groups 0-2 as rect-clipped diagonal matmuls on PE;
    group 3 computed on the Vector engine as tensor_scalar(mul, 4x mode) +
    tensor_tensor(add, 2x mode) chains writing fp16 SBUF directly.
    PSUM->SBUF depthwise results copied (and cast to fp16) on the Pool
    engine, freeing ACT for the instnorm applies.
    Channel mix as fp16 blocked matmuls; instance-norm stats on VectorE;
    instnorm+ReLU fused into one ScalarE activation producing fp16 X.
  - Pad-column zeroing via engine memsets (not DMA).
  - Readout: center 2x2 mean (folded into fc weights) + fc matmul (f32r).

Top-k threshold for the pruned graph weight is computed on host
(np.partition) -- it is weight preprocessing of a replicated input.
"""
import numpy as np

import concourse.bass as bass
import concourse.tile as tile
from concourse import bacc, mybir
from concourse.bass_utils import run_bass_kernel_spmd

F32 = mybir.dt.float32
F32R = mybir.dt.float32r
F16 = mybir.dt.float16
AF = mybir.ActivationFunctionType
ALU = mybir.AluOpType

N_CORES = 8
B = 32
BPC = B // N_CORES          # 4 samples per core
DIM = 512
DS = 128
FEAT = 256
LAYERS = 8
IMG = 128
OUT = 1000
EPS = 1e-5
HALF = IMG // 4 // 2 - 1    # 15
PRUNE = 0.9

# f32 mega-weight column layout ([128, WCOLS])
W1X_OFF = 0                  # 3 dx-taps x [128,128] for conv1
W2D_OFF = W1X_OFF + 3 * 128  # 9 taps x [128,128] diag-dup for conv2
W3_OFF = W2D_OFF + 9 * 128   # [128,128] conv3 (w3 stacked twice on K)
FCW_OFF = W3_OFF + 128       # 2 kg x [128, 1000] fc lhsT (x0.25 pooled)
FCB_OFF = FCW_OFF + 2 * 1000  # [128, 8] fc bias chunks
BN1_OFF = FCB_OFF + 8          # [128, 2] bn1 gamma/beta (dup across halves)
BN2_OFF = BN1_OFF + 2          # [128, 2]
NGB_OFF = BN2_OFF + 2          # [128, 8] instnorm gamma/beta per group
KV_OFF = NGB_OFF + 8           # [128, 36] dw k vectors (g*9+t)
WCOLS = KV_OFF + 36

# fp16 weight layout ([128, W16COLS])
DW16_OFF = 0                   # 36 taps x [128,128] diag (g*9+t)
MIX16_OFF = DW16_OFF + 36 * 128  # 4 kg x [128, 512] = w_eff.T blocks
W16COLS = MIX16_OFF + 4 * 512

XP_BUFS = 20

# engine split for the main-loop depthwise conv: per group, per tap-index
# 'P' = whole group on PE (psum); otherwise per-tap: 'V' = DVE mul+add pair,
# 'A' = ACT product + DVE add, 'G' = Pool fused scalar_tensor_tensor.
# Tap 0 (the full-coverage (1,1) tap) of a non-PE group always inits on DVE.
# PE groups do the depthwise as diagonal matmuls into PSUM; SBUF groups
# compute 9 full-row tap products (engines per PROD table) and combine them
# with a pairwise add tree (engines per ADD table, ops in fixed order:
# P0+=P1, P2+=P3, P4+=P5, P6+=P7, P0+=P2, P4+=P6, P0+=P4, d16=P0+P8).
PE_GROUPS = (0, 1)
DW_PROD = {
    2: ['V', 'A', 'A', 'A', 'A', 'A', 'A', 'A', 'V'],
    3: ['V', 'V', 'G', 'G', 'G', 'G', 'G', 'V', 'V'],
}
DW_ADD = {
    2: ['V', 'V', 'V', 'G', 'V', 'V', 'V', 'V'],
    3: ['V', 'V', 'V', 'G', 'V', 'V', 'G', 'V'],
}
ADD_TREE = [(0, 1), (2, 3), (4, 5), (6, 7), (0, 2), (4, 6), (0, 4)]

# tap order: full-coverage tap first (start=True zeroes the psum region)
TAPS = [(1, 1), (0, 0), (0, 1), (0, 2), (1, 0), (1, 2), (2, 0), (2, 1), (2, 2)]


def _clip(lo, hi, lo2, hi2):
    return max(lo, lo2), min(hi, hi2)


def build_nc():
    nc = bacc.Bacc(num_devices=N_CORES)
    x4 = nc.dram_tensor("x4", [BPC, 3, IMG, IMG], F32R, kind="ExternalInput").ap()
    wts = nc.dram_tensor("wts", [128, WCOLS], F32R, kind="ExternalInput").ap()
    wts16 = nc.dram_tensor("wts16", [128, W16COLS], F16,
                           kind="ExternalInput").ap()
    y4 = nc.dram_tensor("y4", [BPC, OUT], F32, kind="ExternalOutput").ap()

    with tile.TileContext(nc) as tc:
        with (
            tc.tile_pool(name="wp", bufs=1) as wp,
            tc.tile_pool(name="wp16", bufs=1) as wp16,
            tc.tile_pool(name="small", bufs=1) as small,
            tc.tile_pool(name="psA", bufs=3, space="PSUM") as psA,
            tc.tile_pool(name="psB", bufs=2, space="PSUM") as psB,
            tc.tile_pool(name="dram", bufs=1, space="DRAM") as dram,
        ):
            w_sb = wp.tile([128, WCOLS], F32R)
            nc.sync.dma_start(out=w_sb, in_=wts)
            w32 = w_sb.bitcast(F32)
            w16 = wp16.tile([128, W16COLS], F16)
            nc.sync.dma_start(out=w16, in_=wts16)

            def wcols(off, n):
                return w_sb[:, off:off + n]

            def w16cols(off, n):
                return w16[:, off:off + n]

            def kvec(g, t):
                return w32[:, KV_OFF + g * 9 + t:KV_OFF + g * 9 + t + 1]

            eps_t = small.tile([128, 1], F32)
            nc.vector.memset(eps_t, EPS)
            z32 = small.tile([128, 64], F32)
            nc.vector.memset(z32, 0.0)
            z16 = small.tile([128, 64], F16)
            nc.vector.memset(z16, 0.0)

            # ---------------- downsample ----------------
            with tc.tile_pool(name="ds1", bufs=1) as ds1:
                # im2col9: partition p = 32*s + 3*dy + c ; free = (oy 64, ix' 130)
                # ix' = ix + 1 (x padded by 1 on both sides)
                im9 = ds1.tile([128, 64 * 130], F32R)
                im9r = im9.rearrange("p (y x) -> p y x", y=64, x=130)
                # zero the x pads (cols 0 and 129)
                for xc in (0, 129):
                    im9_pads = bass.AP(tensor=im9.tensor,
                                       offset=im9.offset + xc,
                                       ap=[im9.ap[0], [130, 64]])
                    nc.vector.tensor_copy(out=im9_pads, in_=z32[:, 0:64])
                # row oy=0 is out of range for dy=0 taps: zero it everywhere
                # first (dy=1/2 loads overwrite their row 0 afterwards; cols
                # 0/129 are the x-pads zeroed above)
                nc.vector.tensor_copy(out=im9[:, 1:65], in_=z32[:, 0:64])
                nc.vector.tensor_copy(out=im9[:, 65:129], in_=z32[:, 0:64])
                # x rows: iy = 2*oy + dy - 1
                # partition base: sample s -> 64*(s%2) + 27*(s//2)
                x4r = x4.rearrange("s c (y2 two) x -> s c y2 two x", two=2)
                for s in range(BPC):
                    for dy in range(3):
                        p0 = 64 * (s % 2) + 27 * (s // 2) + 3 * dy
                        if dy == 0:
                            # oy in [1,64): iy = 2*(oy-1)+1
                            nc.sync.dma_start(
                                out=im9r[p0:p0 + 3, 1:64, 1:129],
                                in_=x4r[s, :, 0:63, 1, :])
                        elif dy == 1:
                            nc.sync.dma_start(
                                out=im9r[p0:p0 + 3, :, 1:129],
                                in_=x4r[s, :, :, 0, :])
                        else:
                            nc.sync.dma_start(
                                out=im9r[p0:p0 + 3, :, 1:129],
                                in_=x4r[s, :, :, 1, :])

                # conv1: out h1 [128 = 64*(s//2)+ch, (s%2, oy 64, ox 64)]
                h1 = ds1.tile([128, 8192], F32)
                h1r = h1.rearrange("p (sh y x) -> p sh y x", sh=2, y=64, x=64)
                # im2col x-read: ix' = 2*ox + dx (x2 = ox + dx//2, tx = dx%2)
                # paired matmul: K=54 block-diag covers samples (q, q+2):
                # out partitions 0-63 <- sample q, 64-127 <- sample q+2.
                im9x = im9.rearrange("p (y x2 two) -> p y x2 two", x2=65, two=2)
                for q in range(2):
                    for yb in range(4):           # 16-oy blocks
                        for h in range(2):
                            pc1 = psA.tile([128, 512], F32, tag="a",
                                           name="pc1")
                            pc1r = pc1.rearrange("p (y x) -> p y x", y=8, x=64)
                            oy0 = yb * 16 + h * 8
                            for di, dx in enumerate([1, 0, 2]):
                                rhs = im9x[64 * q:64 * q + 54, oy0:oy0 + 8,
                                           dx // 2:dx // 2 + 64, dx % 2]
                                lhsT = w_sb[64 * q:64 * q + 54,
                                            W1X_OFF + di_col(dx) * 128:
                                            W1X_OFF + di_col(dx) * 128 + 128]
                                nc.tensor.matmul(pc1r, lhsT, rhs,
                                                 start=(di == 0), stop=(di == 2),
                                                 tile_position=(64 * q, 0))
                            if (q + yb + h) % 2 == 0:
                                nc.scalar.copy(
                                    out=h1r[:, q, oy0:oy0 + 8, :], in_=pc1)
                            else:
                                nc.vector.tensor_copy(
                                    out=h1r[:, q, oy0:oy0 + 8, :], in_=pc1)

                # BN1 partial stats
                st1 = small.tile([128, 16, 6], F32)
                for i in range(16):
                    nc.vector.bn_stats(out=st1[:, i, :],
                                       in_=h1[:, i * 512:(i + 1) * 512])
                mv1 = small.tile([128, 2], F32)
                nc.vector.bn_aggr(out=mv1, in_=st1)
                sums1 = small.tile([128, 2], F32)
                tmp1 = small.tile([128, 1], F32)
                nc.vector.tensor_scalar_mul(out=sums1[:, 0:1], in0=mv1[:, 0:1],
                                            scalar1=8192.0)
                nc.vector.tensor_mul(out=tmp1, in0=mv1[:, 0:1], in1=mv1[:, 0:1])
                nc.vector.tensor_add(out=tmp1, in0=tmp1, in1=mv1[:, 1:2])
                nc.vector.tensor_scalar_mul(out=sums1[:, 1:2], in0=tmp1,
                                            scalar1=8192.0)
                bn1_in = dram.tile([128, 2], F32)
                bn1_out = dram.tile([128, 2], F32)
                nc.gpsimd.dma_start(out=bn1_in, in_=sums1)
                nc.gpsimd.collective_compute(
                    "AllReduce", mybir.AluOpType.add,
                    replica_groups=[list(range(N_CORES))],
                    ins=[bn1_in.opt()], outs=[bn1_out.opt()])
                red1 = small.tile([128, 2], F32)
                nc.gpsimd.dma_start(out=red1, in_=bn1_out)
                comb1 = small.tile([128, 2], F32)
                nc.gpsimd.dma_start(out=comb1[0:64, :], in_=red1[0:64, :])
                nc.gpsimd.dma_start(out=comb1[0:64, :], in_=red1[64:128, :],
                                    accum_op=mybir.AluOpType.add)
                # scale/bias on rows 0:64, then duplicate
                s1t1 = small.tile([128, 2], F32)
                _bn_scale_bias(nc, s1t1, comb1, w32, BN1_OFF, 131072.0,
                               eps_t, small, rows=64)
                nc.gpsimd.dma_start(out=s1t1[64:128, :], in_=s1t1[0:64, :])

                # apply BN1 + relu -> h1n (f32r), x padded to 66 (ix' = ix+1)
                h1n = ds1.tile([128, 2 * 64 * 66], F32R)
                h1nr3 = h1n.rearrange("p (sh y x) -> p sh y x",
                                      sh=2, y=64, x=66)
                for sh in range(2):
                    for xc in (0, 65):
                        h1n_pads = bass.AP(tensor=h1n.tensor,
                                           offset=h1n.offset + 4224 * sh + xc,
                                           ap=[h1n.ap[0], [66, 64]])
                        nc.vector.tensor_copy(out=h1n_pads, in_=z32[:, 0:64])
                h1r4 = h1.rearrange("p (sh y x) -> p sh y x", sh=2, y=64, x=64)
                for sh in range(2):
                    nc.scalar.activation(out=h1nr3[:, sh, :, 1:65],
                                         in_=h1r4[:, sh, :, :], func=AF.Relu,
                                         scale=s1t1[:, 0:1], bias=s1t1[:, 1:2])

                # conv2: depthwise 3x3 stride 2 -> d2 [128, (sh, 32, 32)]
                # row iy = 2*oy + dy - 1 (unpadded), col ix' = 2*ox + dx (padded)
                h1nr = h1n.rearrange(
                    "p (sh y2 ty x2 tx) -> p sh y2 ty x2 tx",
                    sh=2, y2=32, ty=2, x2=33, tx=2)
                d2 = ds1.tile([128, 2048], F32R)
                for sh in range(2):
                    pd2 = psB.tile([128, 1024], F32, tag="b", name="pd2")
                    pd2r = pd2.rearrange("p (h y x) -> p h y x", h=2, y=16, x=32)
                    for h in range(2):
                        for ti, (dy, dx) in enumerate(TAPS):
                            oy0, oy1 = _clip(h * 16, h * 16 + 16,
                                             1 if dy == 0 else 0, 32)
                            if dy == 1:
                                ys, par = oy0, 0
                            elif dy == 0:
                                ys, par = oy0 - 1, 1
                            else:
                                ys, par = oy0, 1
                            rhs = h1nr[:, sh, ys:ys + (oy1 - oy0), par,
                                       dx // 2:dx // 2 + 32, dx % 2]
                            outp = pd2r[:, h, oy0 - h * 16:oy1 - h * 16, :]
                            t = TAPS.index((dy, dx))
                            nc.tensor.matmul(
                                outp, wcols(W2D_OFF + t * 128, 128), rhs,
                                start=(ti == 0), stop=(ti == len(TAPS) - 1))
                    nc.scalar.copy(out=d2[:, sh * 1024:(sh + 1) * 1024], in_=pd2)

                # conv3: 1x1, 64 -> 128 ; h3 [128=outc, (s, 1024px)]
                h3 = small.tile([128, 4096], F32)
                for a in range(2):
                    for nb in range(4):
                        pc3 = psA.tile([128, 512], F32, tag="a",
                                       name=f"pc3_{a}_{nb}")
                        nc.tensor.matmul(
                            pc3,
                            w_sb[64 * a:64 * a + 64, W3_OFF:W3_OFF + 128],
                            d2[64 * a:64 * a + 64, nb * 512:(nb + 1) * 512],
                            start=True, stop=True)
                        s_full = 2 * a + nb // 2
                        dst = h3[:, s_full * 1024 + (nb % 2) * 512:
                                 s_full * 1024 + (nb % 2) * 512 + 512]
                        if nb % 2 == 0:
                            nc.scalar.copy(out=dst, in_=pc3)
                        else:
                            nc.vector.tensor_copy(out=dst, in_=pc3)

                # BN2 stats + allreduce
                st2 = small.tile([128, 8, 6], F32)
                for i in range(8):
                    nc.vector.bn_stats(out=st2[:, i, :],
                                       in_=h3[:, i * 512:(i + 1) * 512])
                mv2 = small.tile([128, 2], F32)
                nc.vector.bn_aggr(out=mv2, in_=st2)
                sums2 = small.tile([128, 2], F32)
                tmp2 = small.tile([128, 1], F32)
                nc.vector.tensor_scalar_mul(out=sums2[:, 0:1], in0=mv2[:, 0:1],
                                            scalar1=4096.0)
                nc.vector.tensor_mul(out=tmp2, in0=mv2[:, 0:1], in1=mv2[:, 0:1])
                nc.vector.tensor_add(out=tmp2, in0=tmp2, in1=mv2[:, 1:2])
                nc.vector.tensor_scalar_mul(out=sums2[:, 1:2], in0=tmp2,
                                            scalar1=4096.0)
                bn2_in = dram.tile([128, 2], F32)
                bn2_out = dram.tile([128, 2], F32)
                nc.gpsimd.dma_start(out=bn2_in, in_=sums2)
                nc.gpsimd.collective_compute(
                    "AllReduce", mybir.AluOpType.add,
                    replica_groups=[list(range(N_CORES))],
                    ins=[bn2_in.opt()], outs=[bn2_out.opt()])
                red2 = small.tile([128, 2], F32)
                nc.gpsimd.dma_start(out=red2, in_=bn2_out)
                s2t2 = small.tile([128, 2], F32)
                _bn_scale_bias(nc, s2t2, red2, w32, BN2_OFF, 32768.0,
                               eps_t, small, rows=128)

            # ---------------- main loop ----------------
            with (
                tc.tile_pool(name="xp", bufs=XP_BUFS) as xp,
                tc.tile_pool(name="dp", bufs=5) as dp,
                tc.tile_pool(name="pp", bufs=12) as pp,
                tc.tile_pool(name="stp", bufs=4) as stp,
            ):
                def new_x_tile(name):
                    # pad rows (-1, 32) and columns (0, 33) of every xp slot
                    # were zeroed once below; applies only write the interior
                    # (rows 1..32, cols 1..32 of the 34x34 grid).
                    return xp.tile([128, 34 * 34], F16, tag="X", name=name)

                # one-time zeroing of the pad columns of all X slots: the
                # dummies are simultaneously live (kept alive by the reads
                # below), so by pigeonhole they cover all slots.
                _dummies = []
                for i in range(XP_BUFS):
                    zt = xp.tile([128, 34 * 34], F16, tag="X", name=f"xz{i}")
                    # pad rows -1 and 32 (contiguous 34-elem spans)
                    nc.vector.tensor_copy(out=zt[:, 0:34], in_=z16[:, 0:34])
                    nc.vector.tensor_copy(out=zt[:, 1122:1156],
                                          in_=z16[:, 0:34])
                    for xc in (0, 33):
                        pads = bass.AP(tensor=zt.tensor, offset=zt.offset + xc,
                                       ap=[zt.ap[0], [34, 34]])
                        nc.vector.tensor_copy(out=pads, in_=z16[:, 0:34])
                    _dummies.append(zt)
                _pad_scratch = small.tile([128, 1], F16)
                for zt in _dummies:
                    nc.scalar.copy(out=_pad_scratch, in_=zt[:, 0:1])

                Xcur = {}
                for s in range(BPC):
                    xt = new_x_tile(f"X1_0_{s}")
                    xtr = xt.rearrange("p (y x) -> p y x", y=34, x=34)
                    h3r = h3.rearrange("p (s y x) -> p s y x", s=4, y=32, x=32)
                    nc.scalar.activation(out=xtr[:, 1:33, 1:33],
                                         in_=h3r[:, s, :, :],
                                         func=AF.Relu,
                                         scale=s2t2[:, 0:1], bias=s2t2[:, 1:2])
                    Xcur[(0, s)] = xt

                pooled_in = small.tile([128, 2, 4, 4], F32)

                for L in range(1, LAYERS + 1):
                    gs_in = sorted({g for (g, _s) in Xcur})
                    mgs = [2, 3] if L == LAYERS else [0, 1, 2, 3]
                    pe_gs = [g for g in gs_in
                             if g in PE_GROUPS or len(gs_in) == 1]
                    sb_gs = [g for g in gs_in if g not in pe_gs]
                    Xnext = {}
                    d16s = {}
                    for s in range(BPC):
                        d16s[s] = dp.tile([128, 4096], F16, tag="D",
                                          name=f"D{L}_{s}")

                    def xv(g, s, dy, dx):
                        Xr = Xcur[(g, s)].rearrange("p (y x) -> p y x",
                                                    y=34, x=34)
                        return Xr[:, dy:dy + 32, dx:dx + 32]

                    # phase B: SBUF depthwise via 9 full-row products and a
                    # pairwise add tree per (sample, group)
                    def emit_prod(eng, out, xin, g, ti):
                        if eng == 'A':
                            nc.scalar.activation(out=out, in_=xin,
                                                 func=AF.Identity,
                                                 scale=kvec(g, ti))
                        elif eng == 'G':
                            nc.gpsimd.tensor_scalar_mul(out=out, in0=xin,
                                                        scalar1=kvec(g, ti))
                        else:
                            nc.vector.tensor_scalar_mul(out=out, in0=xin,
                                                        scalar1=kvec(g, ti))

                    def emit_add(eng, out, in0, in1):
                        if eng == 'G':
                            nc.gpsimd.tensor_add(out=out, in0=in0, in1=in1)
                        else:
                            nc.vector.tensor_add(out=out, in0=in0, in1=in1)

                    for s in range(BPC):
                        prods = {}
                        for g in sb_gs:
                            for ti, (dy, dx) in enumerate(TAPS):
                                prod = pp.tile([128, 1024], F16, tag="P",
                                               name=f"P{L}_{s}_{g}_{ti}")
                                emit_prod(DW_PROD[g][ti], prod,
                                          xv(g, s, dy, dx), g, ti)
                                prods[(g, ti)] = prod
                        for g in sb_gs:
                            for ai, (d, e) in enumerate(ADD_TREE):
                                emit_add(DW_ADD[g][ai], prods[(g, d)],
                                         prods[(g, d)], prods[(g, e)])
                        for g in sb_gs:
                            emit_add(DW_ADD[g][7],
                                     d16s[s][:, g * 1024:g * 1024 + 1024],
                                     prods[(g, 0)], prods[(g, 8)])

                    # phase A: PE depthwise (PSUM) + ACT copies to fp16 D
                    for s in range(BPC):
                        for g in pe_gs:
                            for h in range(2):
                                pdw = psA.tile([128, 512], F32, tag="a",
                                               name=f"pdw{L}_{s}_{g}_{h}")
                                pdwr = pdw.rearrange("p (y x) -> p y x",
                                                     y=16, x=32)
                                for ti, (dy, dx) in enumerate(TAPS):
                                    rhs = xv(g, s, dy, dx)[:, h * 16:
                                                           h * 16 + 16, :]
                                    t = TAPS.index((dy, dx))
                                    nc.tensor.matmul(
                                        pdwr,
                                        w16cols(DW16_OFF + (g * 9 + t) * 128,
                                                128),
                                        rhs,
                                        start=(ti == 0),
                                        stop=(ti == len(TAPS) - 1))
                                nc.scalar.copy(
                                    out=d16s[s][:, g * 1024 + h * 512:
                                                g * 1024 + h * 512 + 512],
                                    in_=pdw)

                    # phase C: mix + instnorm + apply
                    for s in range(BPC):
                        d16 = d16s[s]
                        for mg in mgs:
                            pm = psB.tile([128, 1024], F32, tag="b",
                                          name=f"pm{L}_{s}_{mg}")
                            for h in range(2):
                                for ki, kg in enumerate(gs_in):
                                    nc.tensor.matmul(
                                        pm[:, h * 512:h * 512 + 512],
                                        w16cols(MIX16_OFF + kg * 512
                                                + mg * 128, 128),
                                        d16[:, kg * 1024 + h * 512:
                                            kg * 1024 + h * 512 + 512],
                                        start=(ki == 0),
                                        stop=(ki == len(gs_in) - 1))
                            st = stp.tile([128, 2, 6], F32, tag="st")
                            nc.vector.bn_stats(out=st[:, 0, :],
                                               in_=pm[:, 0:512])
                            nc.vector.bn_stats(out=st[:, 1, :],
                                               in_=pm[:, 512:1024])
                            mv = stp.tile([128, 2], F32, tag="mv")
                            nc.vector.bn_aggr(out=mv, in_=st)
                            sc = stp.tile([128, 1], F32, tag="sc")
                            tt = stp.tile([128, 1], F32, tag="tt")
                            nc.scalar.activation(out=sc, in_=mv[:, 1:2],
                                                 func=AF.Sqrt, bias=eps_t)
                            nc.vector.reciprocal(out=sc, in_=sc)
                            nc.vector.tensor_scalar_mul(
                                out=sc, in0=sc,
                                scalar1=w32[:, NGB_OFF + 2 * mg:
                                            NGB_OFF + 2 * mg + 1])
                            nc.vector.tensor_mul(out=tt, in0=mv[:, 0:1],
                                                 in1=sc)
                            nc.vector.tensor_scalar(
                                out=tt, in0=tt, scalar1=-1.0,
                                scalar2=w32[:, NGB_OFF + 2 * mg + 1:
                                            NGB_OFF + 2 * mg + 2],
                                op0=mybir.AluOpType.mult,
                                op1=mybir.AluOpType.add)
                            if L < LAYERS:
                                xt = new_x_tile(f"X{L + 1}_{mg}_{s}")
                                Xnext[(mg, s)] = xt
                                xtr = xt.rearrange("p (y x) -> p y x",
                                                   y=34, x=34)
                                pmr2 = pm.rearrange("p (y x) -> p y x",
                                                    y=32, x=32)
                                nc.scalar.activation(
                                    out=xtr[:, 1:33, 1:33], in_=pmr2,
                                    func=AF.Relu, scale=sc, bias=tt)
                            else:
                                pmr = pm.rearrange("p (y x) -> p y x",
                                                   y=32, x=32)
                                nc.scalar.activation(
                                    out=pooled_in[:, mg - 2, s, :],
                                    in_=pmr[:, HALF - 1:HALF + 1,
                                            HALF - 1:HALF + 1],
                                    func=AF.Identity, scale=sc, bias=tt)
                    Xcur = Xnext

                # ---------------- readout ----------------
                tadd = small.tile([128, 2, 4], F32)
                tadd2 = small.tile([128, 2, 4], F32)
                pooled = small.tile([128, 2, 4], F32R)
                nc.vector.tensor_add(out=tadd, in0=pooled_in[:, :, :, 0],
                                     in1=pooled_in[:, :, :, 1])
                nc.vector.tensor_add(out=tadd2, in0=pooled_in[:, :, :, 2],
                                     in1=pooled_in[:, :, :, 3])
                nc.vector.tensor_add(out=pooled, in0=tadd, in1=tadd2)
                y_sb = small.tile([128, 4, 8], F32)
                for mo in range(8):
                    mlen = 128 if mo < 7 else OUT - 7 * 128
                    pf = psA.tile([128, 512], F32, tag="a", name=f"pf{mo}")
                    for kgi in range(2):
                        nc.tensor.matmul(
                            pf[0:mlen, 0:4],
                            w_sb[:, FCW_OFF + kgi * 1000 + mo * 128:
                                 FCW_OFF + kgi * 1000 + mo * 128 + mlen],
                            pooled[:, kgi, :],
                            start=(kgi == 0), stop=(kgi == 1))
                    nc.scalar.activation(
                        out=y_sb[0:mlen, :, mo], in_=pf[0:mlen, 0:4],
                        func=AF.Identity,
                        bias=w32[0:mlen, FCB_OFF + mo:FCB_OFF + mo + 1],
                        scale=1.0)
                for s in range(BPC):
                    dst1 = bass.AP(tensor=y4.tensor, offset=OUT * s,
                                   ap=[[1, 128], [128, 7]])
                    nc.sync.dma_start(out=dst1, in_=y_sb[:, s, 0:7])
                    dst2 = bass.AP(tensor=y4.tensor, offset=OUT * s + 896,
                                   ap=[[1, 104]])
                    nc.sync.dma_start(out=dst2, in_=y_sb[0:104, s, 7])

    nc.finalize()
    return nc


def di_col(dx):
    # column index of conv1 tap dx within w1x block (emission order 1,0,2)
    return {1: 0, 0: 1, 2: 2}[dx]


def _bn_scale_bias(nc, out_st, sums, w32, gb_off, n_tot, eps_t, pool, rows):
    """out_st[:rows, 0] = gamma*rsqrt(var+eps); out_st[:rows, 1] = beta - mu*scale."""
    r = slice(0, rows)
    mu = pool.tile([128, 1], F32, name=f"mu{gb_off}")
    ex2 = pool.tile([128, 1], F32, name=f"ex2{gb_off}")
    var = pool.tile([128, 1], F32, name=f"var{gb_off}")
    nc.vector.tensor_scalar_mul(out=mu[r], in0=sums[r, 0:1], scalar1=1.0 / n_tot)
    nc.vector.tensor_scalar_mul(out=ex2[r], in0=sums[r, 1:2], scalar1=1.0 / n_tot)
    nc.vector.tensor_mul(out=var[r], in0=mu[r], in1=mu[r])
    nc.vector.tensor_sub(out=var[r], in0=ex2[r], in1=var[r])
    nc.scalar.activation(out=var[r], in_=var[r], func=AF.Sqrt, bias=eps_t[r])
    nc.vector.reciprocal(out=var[r], in_=var[r])
    nc.vector.tensor_scalar_mul(out=out_st[r, 0:1], in0=var[r],
                                scalar1=w32[r, gb_off:gb_off + 1])
    nc.vector.tensor_mul(out=mu[r], in0=mu[r], in1=out_st[r, 0:1])
    nc.vector.tensor_scalar(out=out_st[r, 1:2], in0=mu[r], scalar1=-1.0,
                            scalar2=w32[r, gb_off + 1:gb_off + 2],
                            op0=mybir.AluOpType.mult,
                            op1=mybir.AluOpType.add)


def _pack_weights(ds_w1, ds_w2, ds_w3, conv_w, graph_w, fc_w, fc_b,
                  bn1_g, bn1_b, bn2_g, bn2_b, norm_g, norm_b):
    wts = np.zeros((128, WCOLS), np.float32)
    w16 = np.zeros((128, W16COLS), np.float16)
    # pruned graph weight
    k = int((1.0 - PRUNE) * DIM * DIM)
    a = np.abs(graph_w).ravel()
    thresh = np.partition(a, -k)[-k]
    w_eff = np.where(np.abs(graph_w) >= thresh, graph_w, 0.0).astype(np.float32)
    # conv1 taps, paired block-diag:
    # rows 64*q + 27*a + 3*dy + c, cols 64*a + o = w1[o, c, dy, dx]
    for dx in range(3):
        dc = di_col(dx)
        blk = np.zeros((128, 128), np.float32)
        for qq in range(2):
            for aa in range(2):
                for dy in range(3):
                    for c in range(3):
                        blk[64 * qq + 27 * aa + 3 * dy + c,
                            64 * aa:64 * aa + 64] = ds_w1[:, c, dy, dx]
        wts[:, W1X_OFF + dc * 128:W1X_OFF + (dc + 1) * 128] = blk
    # conv2 diag-dup taps
    for t, (dy, dx) in enumerate(TAPS):
        blk = np.zeros((128, 128), np.float32)
        d = ds_w2[:, 0, dy, dx]
        for aa in range(2):
            idx = np.arange(64)
            blk[64 * aa + idx, 64 * aa + idx] = d
        wts[:, W2D_OFF + t * 128:W2D_OFF + (t + 1) * 128] = blk
    # conv3: [64a + c, o] = w3[o, c]
    w3 = ds_w3[:, :, 0, 0]  # [128, 64]
    wts[0:64, W3_OFF:W3_OFF + 128] = w3.T
    wts[64:128, W3_OFF:W3_OFF + 128] = w3.T
    # main dw diag taps (fp16)
    for g in range(4):
        for t, (dy, dx) in enumerate(TAPS):
            blk = np.zeros((128, 128), np.float16)
            idx = np.arange(128)
            blk[idx, idx] = conv_w[g * 128:(g + 1) * 128, 0, dy, dx]
            off = DW16_OFF + (g * 9 + t) * 128
            w16[:, off:off + 128] = blk
    # dw k vectors for the DVE path (f32)
    for g in range(4):
        for t, (dy, dx) in enumerate(TAPS):
            wts[:, KV_OFF + g * 9 + t] = conv_w[g * 128:(g + 1) * 128, 0, dy, dx]
    # mix (fp16): [p, kg*512 + mg*128 + j] = w_eff[mg*128 + j, kg*128 + p]
    weT = w_eff.T  # [in, out]
    for kg in range(4):
        w16[:, MIX16_OFF + kg * 512:MIX16_OFF + (kg + 1) * 512] = \
            weT[kg * 128:(kg + 1) * 128, :].astype(np.float16)
    # fc: [p, kg*1000 + m] = 0.25 * fc_w[m, kg*128 + p]
    for kg in range(2):
        wts[:, FCW_OFF + kg * 1000:FCW_OFF + (kg + 1) * 1000] = \
            0.25 * fc_w[:, kg * 128:(kg + 1) * 128].T
    # fc bias [p, mo]
    fcb = np.zeros((128, 8), np.float32)
    fb = np.zeros(1024, np.float32)
    fb[:OUT] = fc_b
    fcb[:, :] = fb.reshape(8, 128).T
    wts[:, FCB_OFF:FCB_OFF + 8] = fcb
    # bn gammas/betas
    wts[0:64, BN1_OFF] = bn1_g
    wts[64:128, BN1_OFF] = bn1_g
    wts[0:64, BN1_OFF + 1] = bn1_b
    wts[64:128, BN1_OFF + 1] = bn1_b
    wts[:, BN2_OFF] = bn2_g
    wts[:, BN2_OFF + 1] = bn2_b
    for g in range(4):
        wts[:, NGB_OFF + 2 * g] = norm_g[g * 128:(g + 1) * 128]
        wts[:, NGB_OFF + 2 * g + 1] = norm_b[g * 128:(g + 1) * 128]
    return wts, w16


_nc_cache = None
last_results = None


def kernel(**inputs):
    global _nc_cache, last_results
    inputs = {k: np.asarray(v, np.float32) for k, v in inputs.items()}
    wts, w16 = _pack_weights(
        inputs["ds_w1"], inputs["ds_w2"], inputs["ds_w3"], inputs["conv_w"],
        inputs["graph_w"], inputs["fc_w"], inputs["fc_b"],
        inputs["bn1_g"], inputs["bn1_b"], inputs["bn2_g"], inputs["bn2_b"],
        inputs["norm_g"], inputs["norm_b"])
    x = inputs["x"]
    if _nc_cache is None:
        _nc_cache = build_nc()
    nc = _nc_cache
    in_maps = [{"x4": np.ascontiguousarray(x[c * BPC:(c + 1) * BPC]),
                "wts": wts, "wts16": w16} for c in range(N_CORES)]
    res = run_bass_kernel_spmd(nc, in_maps, core_ids=list(range(N_CORES)))
    last_results = res
    return np.concatenate([res.results[c]["y4"] for c in range(N_CORES)], axis=0)


# revision 21
# speedup vs baseline: 1.5486x; 1.0153x over previous
"""Trainium2 Bass kernel for nn_DiscreteTimeNeuralGraph.

Strategy (8 NeuronCores, batch-parallel, engine-balanced):
  - Shard the batch of 32 across 8 cores (4 samples each); weights replicated.
  - Downsample path on-device; BatchNorm batch stats via per-core partial
    sums + one tiny AllReduce each.
  - Main loop in fp16 storage (X, D, weights; fp32 PSUM accumulation):
    depthwise 3x3 conv groups 0-2 as rect-clipped diagonal matmuls on PE;
    group 3 computed on the Vector engine as tensor_scalar(mul, 4x mode) +
    tensor_tensor(add, 2x mode) chains writing fp16 SBUF directly.
    PSUM->SBUF depthwise results copied (and cast to fp16) on the Pool
    engine, freeing ACT for the instnorm applies.
    Channel mix as fp16 blocked matmuls; instance-norm stats on VectorE;
    instnorm+ReLU fused into one ScalarE activation producing fp16 X.
  - Pad-column zeroing via engine memsets (not DMA).
  - Readout: center 2x2 mean (folded into fc weights) + fc matmul (f32r).

Top-k threshold for the pruned graph weight is computed on host
(np.partition) -- it is weight preprocessing of a replicated input.
"""
import numpy as np

import concourse.bass as bass
import concourse.tile as tile
from concourse import bacc, mybir
from concourse.bass_utils import run_bass_kernel_spmd

F32 = mybir.dt.float32
F32R = mybir.dt.float32r
F16 = mybir.dt.float16
AF = mybir.ActivationFunctionType
ALU = mybir.AluOpType

N_CORES = 8
B = 32
BPC = B // N_CORES          # 4 samples per core
DIM = 512
DS = 128
FEAT = 256
LAYERS = 8
IMG = 128
OUT = 1000
EPS = 1e-5
HALF = IMG // 4 // 2 - 1    # 15
PRUNE = 0.9

# f32 mega-weight column layout ([128, WCOLS])
W1X_OFF = 0                  # 3 dx-taps x [128,128] for conv1
W2D_OFF = W1X_OFF + 3 * 128  # 9 taps x [128,128] diag-dup for conv2
W3_OFF = W2D_OFF + 9 * 128   # [128,128] conv3 (w3 stacked twice on K)
FCW_OFF = W3_OFF + 128       # 2 kg x [128, 1000] fc lhsT (x0.25 pooled)
FCB_OFF = FCW_OFF + 2 * 1000  # [128, 8] fc bias chunks
BN1_OFF = FCB_OFF + 8          # [128, 2] bn1 gamma/beta (dup across halves)
BN2_OFF = BN1_OFF + 2          # [128, 2]
NGB_OFF = BN2_OFF + 2          # [128, 8] instnorm gamma/beta per group
KV_OFF = NGB_OFF + 8           # [128, 36] dw k vectors (g*9+t)
WCOLS = KV_OFF + 36

# fp16 weight layout ([128, W16COLS])
DW16_OFF = 0                   # 36 taps x [128,128] diag (g*9+t)
MIX16_OFF = DW16_OFF + 36 * 128  # 4 kg x [128, 512] = w_eff.T blocks
W16COLS = MIX16_OFF + 4 * 512

XP_BUFS = 20

# engine split for the main-loop depthwise conv: per group, per tap-index
# 'P' = whole group on PE (psum); otherwise per-tap: 'V' = DVE mul+add pair,
# 'A' = ACT product + DVE add, 'G' = Pool fused scalar_tensor_tensor.
# Tap 0 (the full-coverage (1,1) tap) of a non-PE group always inits on DVE.
# PE groups do the depthwise as diagonal matmuls into PSUM; SBUF groups
# compute 9 full-row tap products (engines per PROD table) and combine them
# with a pairwise add tree (engines per ADD table, ops in fixed order:
# P0+=P1, P2+=P3, P4+=P5, P6+=P7, P0+=P2, P4+=P6, P0+=P4, d16=P0+P8).
PE_GROUPS = (0, 1)
DW_PROD = {
    2: ['V', 'A', 'A', 'A', 'A', 'A', 'A', 'A', 'V'],
    3: ['V', 'V', 'G', 'G', 'G', 'G', 'G', 'V', 'V'],
}
DW_ADD = {
    2: ['V', 'V', 'V', 'G', 'V', 'V', 'V', 'V'],
    3: ['V', 'V', 'V', 'G', 'V', 'V', 'G', 'V'],
}
ADD_TREE = [(0, 1), (2, 3), (4, 5), (6, 7), (0, 2), (4, 6), (0, 4)]

# tap order: full-coverage tap first (start=True zeroes the psum region)
TAPS = [(1, 1), (0, 0), (0, 1), (0, 2), (1, 0), (1, 2), (2, 0), (2, 1), (2, 2)]


def _clip(lo, hi, lo2, hi2):
    return max(lo, lo2), min(hi, hi2)


def build_nc():
    nc = bacc.Bacc(num_devices=N_CORES)
    x4 = nc.dram_tensor("x4", [BPC, 3, IMG, IMG], F32R, kind="ExternalInput").ap()
    wts = nc.dram_tensor("wts", [128, WCOLS], F32R, kind="ExternalInput").ap()
    wts16 = nc.dram_tensor("wts16", [128, W16COLS], F16,
                           kind="ExternalInput").ap()
    y4 = nc.dram_tensor("y4", [BPC, OUT], F32, kind="ExternalOutput").ap()

    with tile.TileContext(nc) as tc:
        with (
            tc.tile_pool(name="wp", bufs=1) as wp,
            tc.tile_pool(name="wp16", bufs=1) as wp16,
            tc.tile_pool(name="small", bufs=1) as small,
            tc.tile_pool(name="psA", bufs=3, space="PSUM") as psA,
            tc.tile_pool(name="psB", bufs=2, space="PSUM") as psB,
            tc.tile_pool(name="dram", bufs=1, space="DRAM") as dram,
        ):
            w_sb = wp.tile([128, WCOLS], F32R)
            nc.sync.dma_start(out=w_sb, in_=wts)
            w32 = w_sb.bitcast(F32)
            w16 = wp16.tile([128, W16COLS], F16)
            nc.sync.dma_start(out=w16, in_=wts16)

            def wcols(off, n):
                return w_sb[:, off:off + n]

            def w16cols(off, n):
                return w16[:, off:off + n]

            def kvec(g, t):
                return w32[:, KV_OFF + g * 9 + t:KV_OFF + g * 9 + t + 1]

            eps_t = small.tile([128, 1], F32)
            nc.vector.memset(eps_t, EPS)
            z32 = small.tile([128, 64], F32)
            nc.vector.memset(z32, 0.0)
            z16 = small.tile([128, 64], F16)
            nc.vector.memset(z16, 0.0)

            # ---------------- downsample ----------------
            with tc.tile_pool(name="ds1", bufs=1) as ds1:
                # im2col9: partition p = 32*s + 3*dy + c ; free = (oy 64, ix' 130)
                # ix' = ix + 1 (x padded by 1 on both sides)
                im9 = ds1.tile([128, 64 * 130], F32R)
                im9r = im9.rearrange("p (y x) -> p y x", y=64, x=130)
                # zero the x pads (cols 0 and 129)
                for xc in (0, 129):
                    im9_pads = bass.AP(tensor=im9.tensor,
                                       offset=im9.offset + xc,
                                       ap=[im9.ap[0], [130, 64]])
                    nc.vector.tensor_copy(out=im9_pads, in_=z32[:, 0:64])
                # row oy=0 is out of range for dy=0 taps: zero it everywhere
                # first (dy=1/2 loads overwrite their row 0 afterwards; cols
                # 0/129 are the x-pads zeroed above)
                nc.vector.tensor_copy(out=im9[:, 1:65], in_=z32[:, 0:64])
                nc.vector.tensor_copy(out=im9[:, 65:129], in_=z32[:, 0:64])
                # x rows: iy = 2*oy + dy - 1
                # partition base: sample s -> 64*(s%2) + 27*(s//2)
                x4r = x4.rearrange("s c (y2 two) x -> s c y2 two x", two=2)
                for s in range(BPC):
                    for dy in range(3):
                        p0 = 64 * (s % 2) + 27 * (s // 2) + 3 * dy
                        if dy == 0:
                            # oy in [1,64): iy = 2*(oy-1)+1
                            nc.sync.dma_start(
                                out=im9r[p0:p0 + 3, 1:64, 1:129],
                                in_=x4r[s, :, 0:63, 1, :])
                        elif dy == 1:
                            nc.sync.dma_start(
                                out=im9r[p0:p0 + 3, :, 1:129],
                                in_=x4r[s, :, :, 0, :])
                        else:
                            nc.sync.dma_start(
                                out=im9r[p0:p0 + 3, :, 1:129],
                                in_=x4r[s, :, :, 1, :])

                # conv1: out h1 [128 = 64*(s//2)+ch, (s%2, oy 64, ox 64)]
                h1 = ds1.tile([128, 8192], F32)
                h1r = h1.rearrange("p (sh y x) -> p sh y x", sh=2, y=64, x=64)
                # im2col x-read: ix' = 2*ox + dx (x2 = ox + dx//2, tx = dx%2)
                # paired matmul: K=54 block-diag covers samples (q, q+2):
                # out partitions 0-63 <- sample q, 64-127 <- sample q+2.
                im9x = im9.rearrange("p (y x2 two) -> p y x2 two", x2=65, two=2)
                for q in range(2):
                    for yb in range(4):           # 16-oy blocks
                        for h in range(2):
                            pc1 = psA.tile([128, 512], F32, tag="a",
                                           name="pc1")
                            pc1r = pc1.rearrange("p (y x) -> p y x", y=8, x=64)
                            oy0 = yb * 16 + h * 8
                            for di, dx in enumerate([1, 0, 2]):
                                rhs = im9x[64 * q:64 * q + 54, oy0:oy0 + 8,
                                           dx // 2:dx // 2 + 64, dx % 2]
                                lhsT = w_sb[64 * q:64 * q + 54,
                                            W1X_OFF + di_col(dx) * 128:
                                            W1X_OFF + di_col(dx) * 128 + 128]
                                nc.tensor.matmul(pc1r, lhsT, rhs,
                                                 start=(di == 0), stop=(di == 2),
                                                 tile_position=(64 * q, 0))
                            if (q + yb + h) % 2 == 0:
                                nc.scalar.copy(
                                    out=h1r[:, q, oy0:oy0 + 8, :], in_=pc1)
                            else:
                                nc.vector.tensor_copy(
                                    out=h1r[:, q, oy0:oy0 + 8, :], in_=pc1)

                # BN1 partial stats
                st1 = small.tile([128, 16, 6], F32)
                for i in range(16):
                    nc.vector.bn_stats(out=st1[:, i, :],
                                       in_=h1[:, i * 512:(i + 1) * 512])
                mv1 = small.tile([128, 2], F32)
                nc.vector.bn_aggr(out=mv1, in_=st1)
                sums1 = small.tile([128, 2], F32)
                tmp1 = small.tile([128, 1], F32)
                nc.vector.tensor_scalar_mul(out=sums1[:, 0:1], in0=mv1[:, 0:1],
                                            scalar1=8192.0)
                nc.vector.tensor_mul(out=tmp1, in0=mv1[:, 0:1], in1=mv1[:, 0:1])
                nc.vector.tensor_add(out=tmp1, in0=tmp1, in1=mv1[:, 1:2])
                nc.vector.tensor_scalar_mul(out=sums1[:, 1:2], in0=tmp1,
                                            scalar1=8192.0)
                bn1_in = dram.tile([128, 2], F32)
                bn1_out = dram.tile([8, 128, 2], F32)
                nc.gpsimd.dma_start(out=bn1_in, in_=sums1)
                nc.gpsimd.collective_compute(
                    "AllGather", mybir.AluOpType.bypass,
                    replica_groups=[list(range(N_CORES))],
                    ins=[bn1_in.opt()], outs=[bn1_out.opt()])
                ga1 = small.tile([128, 8, 2], F32)
                nc.gpsimd.dma_start(out=ga1, in_=bn1_out.rearrange(
                    "r p t -> p r t"))
                g4 = small.tile([128, 4, 2], F32)
                nc.vector.tensor_add(out=g4, in0=ga1[:, 0:4, :],
                                     in1=ga1[:, 4:8, :])
                g2t = small.tile([128, 2, 2], F32)
                nc.vector.tensor_add(out=g2t, in0=g4[:, 0:2, :],
                                     in1=g4[:, 2:4, :])
                red1 = small.tile([128, 2], F32)
                nc.vector.tensor_add(out=red1, in0=g2t[:, 0, :],
                                     in1=g2t[:, 1, :])
                comb1 = small.tile([128, 2], F32)
                nc.gpsimd.dma_start(out=comb1[0:64, :], in_=red1[0:64, :])
                nc.gpsimd.dma_start(out=comb1[0:64, :], in_=red1[64:128, :],
                                    accum_op=mybir.AluOpType.add)
                # scale/bias on rows 0:64, then duplicate
                s1t1 = small.tile([128, 2], F32)
                _bn_scale_bias(nc, s1t1, comb1, w32, BN1_OFF, 131072.0,
                               eps_t, small, rows=64)
                nc.gpsimd.dma_start(out=s1t1[64:128, :], in_=s1t1[0:64, :])

                # apply BN1 + relu -> h1n (f32r), x padded to 66 (ix' = ix+1)
                h1n = ds1.tile([128, 2 * 64 * 66], F32R)
                h1nr3 = h1n.rearrange("p (sh y x) -> p sh y x",
                                      sh=2, y=64, x=66)
                for sh in range(2):
                    for xc in (0, 65):
                        h1n_pads = bass.AP(tensor=h1n.tensor,
                                           offset=h1n.offset + 4224 * sh + xc,
                                           ap=[h1n.ap[0], [66, 64]])
                        nc.vector.tensor_copy(out=h1n_pads, in_=z32[:, 0:64])
                h1r4 = h1.rearrange("p (sh y x) -> p sh y x", sh=2, y=64, x=64)
                for sh in range(2):
                    nc.scalar.activation(out=h1nr3[:, sh, :, 1:65],
                                         in_=h1r4[:, sh, :, :], func=AF.Relu,
                                         scale=s1t1[:, 0:1], bias=s1t1[:, 1:2])

                # conv2: depthwise 3x3 stride 2 -> d2 [128, (sh, 32, 32)]
                # row iy = 2*oy + dy - 1 (unpadded), col ix' = 2*ox + dx (padded)
                h1nr = h1n.rearrange(
                    "p (sh y2 ty x2 tx) -> p sh y2 ty x2 tx",
                    sh=2, y2=32, ty=2, x2=33, tx=2)
                d2 = ds1.tile([128, 2048], F32R)
                for sh in range(2):
                    pd2 = psB.tile([128, 1024], F32, tag="b", name="pd2")
                    pd2r = pd2.rearrange("p (h y x) -> p h y x", h=2, y=16, x=32)
                    for h in range(2):
                        for ti, (dy, dx) in enumerate(TAPS):
                            oy0, oy1 = _clip(h * 16, h * 16 + 16,
                                             1 if dy == 0 else 0, 32)
                            if dy == 1:
                                ys, par = oy0, 0
                            elif dy == 0:
                                ys, par = oy0 - 1, 1
                            else:
                                ys, par = oy0, 1
                            rhs = h1nr[:, sh, ys:ys + (oy1 - oy0), par,
                                       dx // 2:dx // 2 + 32, dx % 2]
                            outp = pd2r[:, h, oy0 - h * 16:oy1 - h * 16, :]
                            t = TAPS.index((dy, dx))
                            nc.tensor.matmul(
                                outp, wcols(W2D_OFF + t * 128, 128), rhs,
                                start=(ti == 0), stop=(ti == len(TAPS) - 1))
                    nc.scalar.copy(out=d2[:, sh * 1024:(sh + 1) * 1024], in_=pd2)

                # conv3: 1x1, 64 -> 128 ; h3 [128=outc, (s, 1024px)]
                h3 = small.tile([128, 4096], F32)
                for a in range(2):
                    for nb in range(4):
                        pc3 = psA.tile([128, 512], F32, tag="a",
                                       name=f"pc3_{a}_{nb}")
                        nc.tensor.matmul(
                            pc3,
                            w_sb[64 * a:64 * a + 64, W3_OFF:W3_OFF + 128],
                            d2[64 * a:64 * a + 64, nb * 512:(nb + 1) * 512],
                            start=True, stop=True)
                        s_full = 2 * a + nb // 2
                        dst = h3[:, s_full * 1024 + (nb % 2) * 512:
                                 s_full * 1024 + (nb % 2) * 512 + 512]
                        if nb % 2 == 0:
                            nc.scalar.copy(out=dst, in_=pc3)
                        else:
                            nc.vector.tensor_copy(out=dst, in_=pc3)

                # BN2 stats + allreduce
                st2 = small.tile([128, 8, 6], F32)
                for i in range(8):
                    nc.vector.bn_stats(out=st2[:, i, :],
                                       in_=h3[:, i * 512:(i + 1) * 512])
                mv2 = small.tile([128, 2], F32)
                nc.vector.bn_aggr(out=mv2, in_=st2)
                sums2 = small.tile([128, 2], F32)
                tmp2 = small.tile([128, 1], F32)
                nc.vector.tensor_scalar_mul(out=sums2[:, 0:1], in0=mv2[:, 0:1],
                                            scalar1=4096.0)
                nc.vector.tensor_mul(out=tmp2, in0=mv2[:, 0:1], in1=mv2[:, 0:1])
                nc.vector.tensor_add(out=tmp2, in0=tmp2, in1=mv2[:, 1:2])
                nc.vector.tensor_scalar_mul(out=sums2[:, 1:2], in0=tmp2,
                                            scalar1=4096.0)
                bn2_in = dram.tile([128, 2], F32)
                bn2_out = dram.tile([128, 2], F32)
                nc.gpsimd.dma_start(out=bn2_in, in_=sums2)
                nc.gpsimd.collective_compute(
                    "AllReduce", mybir.AluOpType.add,
                    replica_groups=[list(range(N_CORES))],
                    ins=[bn2_in.opt()], outs=[bn2_out.opt()])
                red2 = small.tile([128, 2], F32)
                nc.gpsimd.dma_start(out=red2, in_=bn2_out)
                s2t2 = small.tile([128, 2], F32)
                _bn_scale_bias(nc, s2t2, red2, w32, BN2_OFF, 32768.0,
                               eps_t, small, rows=128)

            # ---------------- main loop ----------------
            with (
                tc.tile_pool(name="xp", bufs=XP_BUFS) as xp,
                tc.tile_pool(name="dp", bufs=5) as dp,
                tc.tile_pool(name="pp", bufs=12) as pp,
                tc.tile_pool(name="stp", bufs=4) as stp,
            ):
                def new_x_tile(name):
                    # pad rows (-1, 32) and columns (0, 33) of every xp slot
                    # were zeroed once below; applies only write the interior
                    # (rows 1..32, cols 1..32 of the 34x34 grid).
                    return xp.tile([128, 34 * 34], F16, tag="X", name=name)

                # one-time zeroing of the pad columns of all X slots: the
                # dummies are simultaneously live (kept alive by the reads
                # below), so by pigeonhole they cover all slots.
                _dummies = []
                for i in range(XP_BUFS):
                    zt = xp.tile([128, 34 * 34], F16, tag="X", name=f"xz{i}")
                    # pad rows -1 and 32 (contiguous 34-elem spans)
                    nc.vector.tensor_copy(out=zt[:, 0:34], in_=z16[:, 0:34])
                    nc.vector.tensor_copy(out=zt[:, 1122:1156],
                                          in_=z16[:, 0:34])
                    for xc in (0, 33):
                        pads = bass.AP(tensor=zt.tensor, offset=zt.offset + xc,
                                       ap=[zt.ap[0], [34, 34]])
                        nc.vector.tensor_copy(out=pads, in_=z16[:, 0:34])
                    _dummies.append(zt)
                _pad_scratch = small.tile([128, 1], F16)
                for zt in _dummies:
                    nc.scalar.copy(out=_pad_scratch, in_=zt[:, 0:1])

                Xcur = {}
                for s in range(BPC):
                    xt = new_x_tile(f"X1_0_{s}")
                    xtr = xt.rearrange("p (y x) -> p y x", y=34, x=34)
                    h3r = h3.rearrange("p (s y x) -> p s y x", s=4, y=32, x=32)
                    nc.scalar.activation(out=xtr[:, 1:33, 1:33],
                                         in_=h3r[:, s, :, :],
                                         func=AF.Relu,
                                         scale=s2t2[:, 0:1], bias=s2t2[:, 1:2])
                    Xcur[(0, s)] = xt

                pooled_in = small.tile([128, 2, 4, 4], F32)

                for L in range(1, LAYERS + 1):
                    gs_in = sorted({g for (g, _s) in Xcur})
                    mgs = [2, 3] if L == LAYERS else [0, 1, 2, 3]
                    pe_gs = [g for g in gs_in
                             if g in PE_GROUPS or len(gs_in) == 1]
                    sb_gs = [g for g in gs_in if g not in pe_gs]
                    Xnext = {}
                    d16s = {}
                    for s in range(BPC):
                        d16s[s] = dp.tile([128, 4096], F16, tag="D",
                                          name=f"D{L}_{s}")

                    def xv(g, s, dy, dx):
                        Xr = Xcur[(g, s)].rearrange("p (y x) -> p y x",
                                                    y=34, x=34)
                        return Xr[:, dy:dy + 32, dx:dx + 32]

                    # phase B: SBUF depthwise via 9 full-row products and a
                    # pairwise add tree per (sample, group)
                    def emit_prod(eng, out, xin, g, ti):
                        if eng == 'A':
                            nc.scalar.activation(out=out, in_=xin,
                                                 func=AF.Identity,
                                                 scale=kvec(g, ti))
                        elif eng == 'G':
                            nc.gpsimd.tensor_scalar_mul(out=out, in0=xin,
                                                        scalar1=kvec(g, ti))
                        else:
                            nc.vector.tensor_scalar_mul(out=out, in0=xin,
                                                        scalar1=kvec(g, ti))

                    def emit_add(eng, out, in0, in1):
                        if eng == 'G':
                            nc.gpsimd.tensor_add(out=out, in0=in0, in1=in1)
                        else:
                            nc.vector.tensor_add(out=out, in0=in0, in1=in1)

                    for s in range(BPC):
                        prods = {}
                        for g in sb_gs:
                            for ti, (dy, dx) in enumerate(TAPS):
                                prod = pp.tile([128, 1024], F16, tag="P",
                                               name=f"P{L}_{s}_{g}_{ti}")
                                emit_prod(DW_PROD[g][ti], prod,
                                          xv(g, s, dy, dx), g, ti)
                                prods[(g, ti)] = prod
                        for g in sb_gs:
                            for ai, (d, e) in enumerate(ADD_TREE):
                                emit_add(DW_ADD[g][ai], prods[(g, d)],
                                         prods[(g, d)], prods[(g, e)])
                        for g in sb_gs:
                            emit_add(DW_ADD[g][7],
                                     d16s[s][:, g * 1024:g * 1024 + 1024],
                                     prods[(g, 0)], prods[(g, 8)])

                    # phase A: PE depthwise (PSUM) + ACT copies to fp16 D
                    for s in range(BPC):
                        for g in pe_gs:
                            for h in range(2):
                                pdw = psA.tile([128, 512], F32, tag="a",
                                               name=f"pdw{L}_{s}_{g}_{h}")
                                pdwr = pdw.rearrange("p (y x) -> p y x",
                                                     y=16, x=32)
                                for ti, (dy, dx) in enumerate(TAPS):
                                    rhs = xv(g, s, dy, dx)[:, h * 16:
                                                           h * 16 + 16, :]
                                    t = TAPS.index((dy, dx))
                                    nc.tensor.matmul(
                                        pdwr,
                                        w16cols(DW16_OFF + (g * 9 + t) * 128,
                                                128),
                                        rhs,
                                        start=(ti == 0),
                                        stop=(ti == len(TAPS) - 1))
                                nc.scalar.copy(
                                    out=d16s[s][:, g * 1024 + h * 512:
                                                g * 1024 + h * 512 + 512],
                                    in_=pdw)

                    # phase C: mix + instnorm + apply
                    for s in range(BPC):
                        d16 = d16s[s]
                        for mg in mgs:
                            pm = psB.tile([128, 1024], F32, tag="b",
                                          name=f"pm{L}_{s}_{mg}")
                            for h in range(2):
                                for ki, kg in enumerate(gs_in):
                                    nc.tensor.matmul(
                                        pm[:, h * 512:h * 512 + 512],
                                        w16cols(MIX16_OFF + kg * 512
                                                + mg * 128, 128),
                                        d16[:, kg * 1024 + h * 512:
                                            kg * 1024 + h * 512 + 512],
                                        start=(ki == 0),
                                        stop=(ki == len(gs_in) - 1))
                            st = stp.tile([128, 2, 6], F32, tag="st")
                            nc.vector.bn_stats(out=st[:, 0, :],
                                               in_=pm[:, 0:512])
                            nc.vector.bn_stats(out=st[:, 1, :],
                                               in_=pm[:, 512:1024])
                            mv = stp.tile([128, 2], F32, tag="mv")
                            nc.vector.bn_aggr(out=mv, in_=st)
                            sc = stp.tile([128, 1], F32, tag="sc")
                            tt = stp.tile([128, 1], F32, tag="tt")
                            nc.scalar.activation(out=sc, in_=mv[:, 1:2],
                                                 func=AF.Sqrt, bias=eps_t)
                            nc.vector.reciprocal(out=sc, in_=sc)
                            nc.vector.tensor_scalar_mul(
                                out=sc, in0=sc,
                                scalar1=w32[:, NGB_OFF + 2 * mg:
                                            NGB_OFF + 2 * mg + 1])
                            nc.vector.tensor_mul(out=tt, in0=mv[:, 0:1],
                                                 in1=sc)
                            nc.vector.tensor_scalar(
                                out=tt, in0=tt, scalar1=-1.0,
                                scalar2=w32[:, NGB_OFF + 2 * mg + 1:
                                            NGB_OFF + 2 * mg + 2],
                                op0=mybir.AluOpType.mult,
                                op1=mybir.AluOpType.add)
                            if L < LAYERS:
                                xt = new_x_tile(f"X{L + 1}_{mg}_{s}")
                                Xnext[(mg, s)] = xt
                                xtr = xt.rearrange("p (y x) -> p y x",
                                                   y=34, x=34)
                                pmr2 = pm.rearrange("p (y x) -> p y x",
                                                    y=32, x=32)
                                nc.scalar.activation(
                                    out=xtr[:, 1:33, 1:33], in_=pmr2,
                                    func=AF.Relu, scale=sc, bias=tt)
                            else:
                                pmr = pm.rearrange("p (y x) -> p y x",
                                                   y=32, x=32)
                                nc.scalar.activation(
                                    out=pooled_in[:, mg - 2, s, :],
                                    in_=pmr[:, HALF - 1:HALF + 1,
                                            HALF - 1:HALF + 1],
                                    func=AF.Identity, scale=sc, bias=tt)
                    Xcur = Xnext

                # ---------------- readout ----------------
                tadd = small.tile([128, 2, 4], F32)
                tadd2 = small.tile([128, 2, 4], F32)
                pooled = small.tile([128, 2, 4], F32R)
                nc.vector.tensor_add(out=tadd, in0=pooled_in[:, :, :, 0],
                                     in1=pooled_in[:, :, :, 1])
                nc.vector.tensor_add(out=tadd2, in0=pooled_in[:, :, :, 2],
                                     in1=pooled_in[:, :, :, 3])
                nc.vector.tensor_add(out=pooled, in0=tadd, in1=tadd2)
                y_sb = small.tile([128, 4, 8], F32)
                for mo in range(8):
                    mlen = 128 if mo < 7 else OUT - 7 * 128
                    pf = psA.tile([128, 512], F32, tag="a", name=f"pf{mo}")
                    for kgi in range(2):
                        nc.tensor.matmul(
                            pf[0:mlen, 0:4],
                            w_sb[:, FCW_OFF + kgi * 1000 + mo * 128:
                                 FCW_OFF + kgi * 1000 + mo * 128 + mlen],
                            pooled[:, kgi, :],
                            start=(kgi == 0), stop=(kgi == 1))
                    nc.scalar.activation(
                        out=y_sb[0:mlen, :, mo], in_=pf[0:mlen, 0:4],
                        func=AF.Identity,
                        bias=w32[0:mlen, FCB_OFF + mo:FCB_OFF + mo + 1],
                        scale=1.0)
                for s in range(BPC):
                    dst1 = bass.AP(tensor=y4.tensor, offset=OUT * s,
                                   ap=[[1, 128], [128, 7]])
                    nc.sync.dma_start(out=dst1, in_=y_sb[:, s, 0:7])
                    dst2 = bass.AP(tensor=y4.tensor, offset=OUT * s + 896,
                                   ap=[[1, 104]])
                    nc.sync.dma_start(out=dst2, in_=y_sb[0:104, s, 7])

    nc.finalize()
    return nc


def di_col(dx):
    # column index of conv1 tap dx within w1x block (emission order 1,0,2)
    return {1: 0, 0: 1, 2: 2}[dx]


def _bn_scale_bias(nc, out_st, sums, w32, gb_off, n_tot, eps_t, pool, rows):
    """out_st[:rows, 0] = gamma*rsqrt(var+eps); out_st[:rows, 1] = beta - mu*scale."""
    r = slice(0, rows)
    mu = pool.tile([128, 1], F32, name=f"mu{gb_off}")
    ex2 = pool.tile([128, 1], F32, name=f"ex2{gb_off}")
    var = pool.tile([128, 1], F32, name=f"var{gb_off}")
    nc.vector.tensor_scalar_mul(out=mu[r], in0=sums[r, 0:1], scalar1=1.0 / n_tot)
    nc.vector.tensor_scalar_mul(out=ex2[r], in0=sums[r, 1:2], scalar1=1.0 / n_tot)
    nc.vector.tensor_mul(out=var[r], in0=mu[r], in1=mu[r])
    nc.vector.tensor_sub(out=var[r], in0=ex2[r], in1=var[r])
    nc.scalar.activation(out=var[r], in_=var[r], func=AF.Sqrt, bias=eps_t[r])
    nc.vector.reciprocal(out=var[r], in_=var[r])
    nc.vector.tensor_scalar_mul(out=out_st[r, 0:1], in0=var[r],
                                scalar1=w32[r, gb_off:gb_off + 1])
    nc.vector.tensor_mul(out=mu[r], in0=mu[r], in1=out_st[r, 0:1])
    nc.vector.tensor_scalar(out=out_st[r, 1:2], in0=mu[r], scalar1=-1.0,
                            scalar2=w32[r, gb_off + 1:gb_off + 2],
                            op0=mybir.AluOpType.mult,
                            op1=mybir.AluOpType.add)


def _pack_weights(ds_w1, ds_w2, ds_w3, conv_w, graph_w, fc_w, fc_b,
                  bn1_g, bn1_b, bn2_g, bn2_b, norm_g, norm_b):
    wts = np.zeros((128, WCOLS), np.float32)
    w16 = np.zeros((128, W16COLS), np.float16)
    # pruned graph weight
    k = int((1.0 - PRUNE) * DIM * DIM)
    a = np.abs(graph_w).ravel()
    thresh = np.partition(a, -k)[-k]
    w_eff = np.where(np.abs(graph_w) >= thresh, graph_w, 0.0).astype(np.float32)
    # conv1 taps, paired block-diag:
    # rows 64*q + 27*a + 3*dy + c, cols 64*a + o = w1[o, c, dy, dx]
    for dx in range(3):
        dc = di_col(dx)
        blk = np.zeros((128, 128), np.float32)
        for qq in range(2):
            for aa in range(2):
                for dy in range(3):
                    for c in range(3):
                        blk[64 * qq + 27 * aa + 3 * dy + c,
                            64 * aa:64 * aa + 64] = ds_w1[:, c, dy, dx]
        wts[:, W1X_OFF + dc * 128:W1X_OFF + (dc + 1) * 128] = blk
    # conv2 diag-dup taps
    for t, (dy, dx) in enumerate(TAPS):
        blk = np.zeros((128, 128), np.float32)
        d = ds_w2[:, 0, dy, dx]
        for aa in range(2):
            idx = np.arange(64)
            blk[64 * aa + idx, 64 * aa + idx] = d
        wts[:, W2D_OFF + t * 128:W2D_OFF + (t + 1) * 128] = blk
    # conv3: [64a + c, o] = w3[o, c]
    w3 = ds_w3[:, :, 0, 0]  # [128, 64]
    wts[0:64, W3_OFF:W3_OFF + 128] = w3.T
    wts[64:128, W3_OFF:W3_OFF + 128] = w3.T
    # main dw diag taps (fp16)
    for g in range(4):
        for t, (dy, dx) in enumerate(TAPS):
            blk = np.zeros((128, 128), np.float16)
            idx = np.arange(128)
            blk[idx, idx] = conv_w[g * 128:(g + 1) * 128, 0, dy, dx]
            off = DW16_OFF + (g * 9 + t) * 128
            w16[:, off:off + 128] = blk
    # dw k vectors for the DVE path (f32)
    for g in range(4):
        for t, (dy, dx) in enumerate(TAPS):
            wts[:, KV_OFF + g * 9 + t] = conv_w[g * 128:(g + 1) * 128, 0, dy, dx]
    # mix (fp16): [p, kg*512 + mg*128 + j] = w_eff[mg*128 + j, kg*128 + p]
    weT = w_eff.T  # [in, out]
    for kg in range(4):
        w16[:, MIX16_OFF + kg * 512:MIX16_OFF + (kg + 1) * 512] = \
            weT[kg * 128:(kg + 1) * 128, :].astype(np.float16)
    # fc: [p, kg*1000 + m] = 0.25 * fc_w[m, kg*128 + p]
    for kg in range(2):
        wts[:, FCW_OFF + kg * 1000:FCW_OFF + (kg + 1) * 1000] = \
            0.25 * fc_w[:, kg * 128:(kg + 1) * 128].T
    # fc bias [p, mo]
    fcb = np.zeros((128, 8), np.float32)
    fb = np.zeros(1024, np.float32)
    fb[:OUT] = fc_b
    fcb[:, :] = fb.reshape(8, 128).T
    wts[:, FCB_OFF:FCB_OFF + 8] = fcb
    # bn gammas/betas
    wts[0:64, BN1_OFF] = bn1_g
    wts[64:128, BN1_OFF] = bn1_g
    wts[0:64, BN1_OFF + 1] = bn1_b
    wts[64:128, BN1_OFF + 1] = bn1_b
    wts[:, BN2_OFF] = bn2_g
    wts[:, BN2_OFF + 1] = bn2_b
    for g in range(4):
        wts[:, NGB_OFF + 2 * g] = norm_g[g * 128:(g + 1) * 128]
        wts[:, NGB_OFF + 2 * g + 1] = norm_b[g * 128:(g + 1) * 128]
    return wts, w16


_nc_cache = None
last_results = None


def kernel(**inputs):
    global _nc_cache, last_results
    inputs = {k: np.asarray(v, np.float32) for k, v in inputs.items()}
    wts, w16 = _pack_weights(
        inputs["ds_w1"], inputs["ds_w2"], inputs["ds_w3"], inputs["conv_w"],
        inputs["graph_w"], inputs["fc_w"], inputs["fc_b"],
        inputs["bn1_g"], inputs["bn1_b"], inputs["bn2_g"], inputs["bn2_b"],
        inputs["norm_g"], inputs["norm_b"])
    x = inputs["x"]
    if _nc_cache is None:
        _nc_cache = build_nc()
    nc = _nc_cache
    in_maps = [{"x4": np.ascontiguousarray(x[c * BPC:(c + 1) * BPC]),
                "wts": wts, "wts16": w16} for c in range(N_CORES)]
    res = run_bass_kernel_spmd(nc, in_maps, core_ids=list(range(N_CORES)))
    last_results = res
    return np.concatenate([res.results[c]["y4"] for c in range(N_CORES)], axis=0)


# revision 22
# speedup vs baseline: 1.5726x; 1.0155x over previous
"""Trainium2 Bass kernel for nn_DiscreteTimeNeuralGraph.

Strategy (8 NeuronCores, batch-parallel, engine-balanced):
  - Shard the batch of 32 across 8 cores (4 samples each); weights replicated.
  - Downsample path on-device; BatchNorm batch stats via per-core partial
    sums + one tiny AllReduce each.
  - Main loop in fp16 storage (X, D, weights; fp32 PSUM accumulation):
    depthwise 3x3 conv groups 0-2 as rect-clipped diagonal matmuls on PE;
    group 3 computed on the Vector engine as tensor_scalar(mul, 4x mode) +
    tensor_tensor(add, 2x mode) chains writing fp16 SBUF directly.
    PSUM->SBUF depthwise results copied (and cast to fp16) on the Pool
    engine, freeing ACT for the instnorm applies.
    Channel mix as fp16 blocked matmuls; instance-norm stats on VectorE;
    instnorm+ReLU fused into one ScalarE activation producing fp16 X.
  - Pad-column zeroing via engine memsets (not DMA).
  - Readout: center 2x2 mean (folded into fc weights) + fc matmul (f32r).

Top-k threshold for the pruned graph weight is computed on host
(np.partition) -- it is weight preprocessing of a replicated input.
"""
import numpy as np

import concourse.bass as bass
import concourse.tile as tile
from concourse import bacc, mybir
from concourse.bass_utils import run_bass_kernel_spmd

F32 = mybir.dt.float32
F32R = mybir.dt.float32r
F16 = mybir.dt.float16
AF = mybir.ActivationFunctionType
ALU = mybir.AluOpType

N_CORES = 8
B = 32
BPC = B // N_CORES          # 4 samples per core
DIM = 512
DS = 128
FEAT = 256
LAYERS = 8
IMG = 128
OUT = 1000
EPS = 1e-5
HALF = IMG // 4 // 2 - 1    # 15
PRUNE = 0.9

# f32 mega-weight column layout ([128, WCOLS])
W1X_OFF = 0                  # 3 dx-taps x [128,128] for conv1
W2D_OFF = W1X_OFF + 3 * 128  # 9 taps x [128,128] diag-dup for conv2
W3_OFF = W2D_OFF + 9 * 128   # [128,128] conv3 (w3 stacked twice on K)
FCW_OFF = W3_OFF + 128       # 2 kg x [128, 1000] fc lhsT (x0.25 pooled)
FCB_OFF = FCW_OFF + 2 * 1000  # [128, 8] fc bias chunks
BN1_OFF = FCB_OFF + 8          # [128, 2] bn1 gamma/beta (dup across halves)
BN2_OFF = BN1_OFF + 2          # [128, 2]
NGB_OFF = BN2_OFF + 2          # [128, 8] instnorm gamma/beta per group
KV_OFF = NGB_OFF + 8           # [128, 36] dw k vectors (g*9+t)
WCOLS = KV_OFF + 36

# fp16 weight layout ([128, W16COLS])
DW16_OFF = 0                   # 36 taps x [128,128] diag (g*9+t)
MIX16_OFF = DW16_OFF + 36 * 128  # 4 kg x [128, 512] = w_eff.T blocks
W16COLS = MIX16_OFF + 4 * 512

XP_BUFS = 20

# engine split for the main-loop depthwise conv: per group, per tap-index
# 'P' = whole group on PE (psum); otherwise per-tap: 'V' = DVE mul+add pair,
# 'A' = ACT product + DVE add, 'G' = Pool fused scalar_tensor_tensor.
# Tap 0 (the full-coverage (1,1) tap) of a non-PE group always inits on DVE.
# PE groups do the depthwise as diagonal matmuls into PSUM; SBUF groups
# compute 9 full-row tap products (engines per PROD table) and combine them
# with a pairwise add tree (engines per ADD table, ops in fixed order:
# P0+=P1, P2+=P3, P4+=P5, P6+=P7, P0+=P2, P4+=P6, P0+=P4, d16=P0+P8).
PE_GROUPS = (0, 1)
DW_PROD = {
    2: ['V', 'A', 'A', 'A', 'A', 'A', 'A', 'A', 'V'],
    3: ['V', 'V', 'G', 'G', 'G', 'G', 'G', 'V', 'V'],
}
DW_ADD = {
    2: ['V', 'V', 'V', 'G', 'V', 'V', 'V', 'V'],
    3: ['V', 'V', 'V', 'G', 'V', 'V', 'G', 'V'],
}
ADD_TREE = [(0, 1), (2, 3), (4, 5), (6, 7), (0, 2), (4, 6), (0, 4)]

# tap order: full-coverage tap first (start=True zeroes the psum region)
TAPS = [(1, 1), (0, 0), (0, 1), (0, 2), (1, 0), (1, 2), (2, 0), (2, 1), (2, 2)]


def _clip(lo, hi, lo2, hi2):
    return max(lo, lo2), min(hi, hi2)


def build_nc():
    nc = bacc.Bacc(num_devices=N_CORES)
    x4 = nc.dram_tensor("x4", [BPC, 3, IMG, IMG], F32R, kind="ExternalInput").ap()
    wts = nc.dram_tensor("wts", [128, WCOLS], F32R, kind="ExternalInput").ap()
    wts16 = nc.dram_tensor("wts16", [128, W16COLS], F16,
                           kind="ExternalInput").ap()
    y4 = nc.dram_tensor("y4", [BPC, OUT], F32, kind="ExternalOutput").ap()

    with tile.TileContext(nc) as tc:
        with (
            tc.tile_pool(name="wp", bufs=1) as wp,
            tc.tile_pool(name="wp16", bufs=1) as wp16,
            tc.tile_pool(name="small", bufs=1) as small,
            tc.tile_pool(name="psA", bufs=3, space="PSUM") as psA,
            tc.tile_pool(name="psB", bufs=2, space="PSUM") as psB,
            tc.tile_pool(name="dram", bufs=1, space="DRAM") as dram,
        ):
            w_sb = wp.tile([128, WCOLS], F32R)
            nc.sync.dma_start(out=w_sb, in_=wts)
            w32 = w_sb.bitcast(F32)
            w16 = wp16.tile([128, W16COLS], F16)
            nc.sync.dma_start(out=w16, in_=wts16)

            def wcols(off, n):
                return w_sb[:, off:off + n]

            def w16cols(off, n):
                return w16[:, off:off + n]

            def kvec(g, t):
                return w32[:, KV_OFF + g * 9 + t:KV_OFF + g * 9 + t + 1]

            eps_t = small.tile([128, 1], F32)
            nc.vector.memset(eps_t, EPS)
            z32 = small.tile([128, 64], F32)
            nc.vector.memset(z32, 0.0)
            z16 = small.tile([128, 64], F16)
            nc.vector.memset(z16, 0.0)

            # ---------------- downsample ----------------
            with tc.tile_pool(name="ds1", bufs=1) as ds1:
                # im2col9: partition p = 32*s + 3*dy + c ; free = (oy 64, ix' 130)
                # ix' = ix + 1 (x padded by 1 on both sides)
                im9 = ds1.tile([128, 64 * 130], F32R)
                im9r = im9.rearrange("p (y x) -> p y x", y=64, x=130)
                # zero the x pads (cols 0 and 129)
                for xc in (0, 129):
                    im9_pads = bass.AP(tensor=im9.tensor,
                                       offset=im9.offset + xc,
                                       ap=[im9.ap[0], [130, 64]])
                    nc.vector.tensor_copy(out=im9_pads, in_=z32[:, 0:64])
                # row oy=0 is out of range for dy=0 taps: zero it everywhere
                # first (dy=1/2 loads overwrite their row 0 afterwards; cols
                # 0/129 are the x-pads zeroed above)
                nc.vector.tensor_copy(out=im9[:, 1:65], in_=z32[:, 0:64])
                nc.vector.tensor_copy(out=im9[:, 65:129], in_=z32[:, 0:64])
                # x rows: iy = 2*oy + dy - 1
                # partition base: sample s -> 64*(s%2) + 27*(s//2)
                x4r = x4.rearrange("s c (y2 two) x -> s c y2 two x", two=2)
                for s in range(BPC):
                    for dy in range(3):
                        p0 = 64 * (s % 2) + 27 * (s // 2) + 3 * dy
                        if dy == 0:
                            # oy in [1,64): iy = 2*(oy-1)+1
                            nc.sync.dma_start(
                                out=im9r[p0:p0 + 3, 1:64, 1:129],
                                in_=x4r[s, :, 0:63, 1, :])
                        elif dy == 1:
                            nc.sync.dma_start(
                                out=im9r[p0:p0 + 3, :, 1:129],
                                in_=x4r[s, :, :, 0, :])
                        else:
                            nc.sync.dma_start(
                                out=im9r[p0:p0 + 3, :, 1:129],
                                in_=x4r[s, :, :, 1, :])

                # conv1: out h1 [128 = 64*(s//2)+ch, (s%2, oy 64, ox 64)]
                h1 = ds1.tile([128, 8192], F32)
                h1r = h1.rearrange("p (sh y x) -> p sh y x", sh=2, y=64, x=64)
                # im2col x-read: ix' = 2*ox + dx (x2 = ox + dx//2, tx = dx%2)
                # paired matmul: K=54 block-diag covers samples (q, q+2):
                # out partitions 0-63 <- sample q, 64-127 <- sample q+2.
                im9x = im9.rearrange("p (y x2 two) -> p y x2 two", x2=65, two=2)
                for q in range(2):
                    for yb in range(4):           # 16-oy blocks
                        for h in range(2):
                            pc1 = psA.tile([128, 512], F32, tag="a",
                                           name="pc1")
                            pc1r = pc1.rearrange("p (y x) -> p y x", y=8, x=64)
                            oy0 = yb * 16 + h * 8
                            for di, dx in enumerate([1, 0, 2]):
                                rhs = im9x[64 * q:64 * q + 54, oy0:oy0 + 8,
                                           dx // 2:dx // 2 + 64, dx % 2]
                                lhsT = w_sb[64 * q:64 * q + 54,
                                            W1X_OFF + di_col(dx) * 128:
                                            W1X_OFF + di_col(dx) * 128 + 128]
                                nc.tensor.matmul(pc1r, lhsT, rhs,
                                                 start=(di == 0), stop=(di == 2),
                                                 tile_position=(64 * q, 0))
                            if (q + yb + h) % 2 == 0:
                                nc.scalar.copy(
                                    out=h1r[:, q, oy0:oy0 + 8, :], in_=pc1)
                            else:
                                nc.vector.tensor_copy(
                                    out=h1r[:, q, oy0:oy0 + 8, :], in_=pc1)

                # BN1 partial stats
                st1 = small.tile([128, 16, 6], F32)
                for i in range(16):
                    nc.vector.bn_stats(out=st1[:, i, :],
                                       in_=h1[:, i * 512:(i + 1) * 512])
                mv1 = small.tile([128, 2], F32)
                nc.vector.bn_aggr(out=mv1, in_=st1)
                sums1 = small.tile([128, 2], F32)
                tmp1 = small.tile([128, 1], F32)
                nc.vector.tensor_scalar_mul(out=sums1[:, 0:1], in0=mv1[:, 0:1],
                                            scalar1=8192.0)
                nc.vector.tensor_mul(out=tmp1, in0=mv1[:, 0:1], in1=mv1[:, 0:1])
                nc.vector.tensor_add(out=tmp1, in0=tmp1, in1=mv1[:, 1:2])
                nc.vector.tensor_scalar_mul(out=sums1[:, 1:2], in0=tmp1,
                                            scalar1=8192.0)
                bn1_in = dram.tile([128, 2], F32)
                bn1_out = dram.tile([8, 128, 2], F32)
                nc.gpsimd.dma_start(out=bn1_in, in_=sums1)
                nc.gpsimd.collective_compute(
                    "AllGather", mybir.AluOpType.bypass,
                    replica_groups=[list(range(N_CORES))],
                    ins=[bn1_in.opt()], outs=[bn1_out.opt()])
                ga1 = small.tile([128, 8, 2], F32)
                nc.gpsimd.dma_start(out=ga1, in_=bn1_out.rearrange(
                    "r p t -> p r t"))
                g4 = small.tile([128, 4, 2], F32)
                nc.vector.tensor_add(out=g4, in0=ga1[:, 0:4, :],
                                     in1=ga1[:, 4:8, :])
                g2t = small.tile([128, 2, 2], F32)
                nc.vector.tensor_add(out=g2t, in0=g4[:, 0:2, :],
                                     in1=g4[:, 2:4, :])
                red1 = small.tile([128, 2], F32)
                nc.vector.tensor_add(out=red1, in0=g2t[:, 0, :],
                                     in1=g2t[:, 1, :])
                comb1 = small.tile([128, 2], F32)
                nc.gpsimd.dma_start(out=comb1[0:64, :], in_=red1[0:64, :])
                nc.gpsimd.dma_start(out=comb1[0:64, :], in_=red1[64:128, :],
                                    accum_op=mybir.AluOpType.add)
                # scale/bias on rows 0:64, then duplicate
                s1t1 = small.tile([128, 2], F32)
                _bn_scale_bias(nc, s1t1, comb1, w32, BN1_OFF, 131072.0,
                               eps_t, small, rows=64)
                nc.gpsimd.dma_start(out=s1t1[64:128, :], in_=s1t1[0:64, :])

                # apply BN1 + relu -> h1n (f32r), x padded to 66 (ix' = ix+1)
                h1n = ds1.tile([128, 2 * 64 * 66], F32R)
                h1nr3 = h1n.rearrange("p (sh y x) -> p sh y x",
                                      sh=2, y=64, x=66)
                for sh in range(2):
                    for xc in (0, 65):
                        h1n_pads = bass.AP(tensor=h1n.tensor,
                                           offset=h1n.offset + 4224 * sh + xc,
                                           ap=[h1n.ap[0], [66, 64]])
                        nc.vector.tensor_copy(out=h1n_pads, in_=z32[:, 0:64])
                h1r4 = h1.rearrange("p (sh y x) -> p sh y x", sh=2, y=64, x=64)
                for sh in range(2):
                    nc.scalar.activation(out=h1nr3[:, sh, :, 1:65],
                                         in_=h1r4[:, sh, :, :], func=AF.Relu,
                                         scale=s1t1[:, 0:1], bias=s1t1[:, 1:2])

                # conv2: depthwise 3x3 stride 2 -> d2 [128, (sh, 32, 32)]
                # row iy = 2*oy + dy - 1 (unpadded), col ix' = 2*ox + dx (padded)
                h1nr = h1n.rearrange(
                    "p (sh y2 ty x2 tx) -> p sh y2 ty x2 tx",
                    sh=2, y2=32, ty=2, x2=33, tx=2)
                d2 = ds1.tile([128, 2048], F32R)
                for sh in range(2):
                    pd2 = psB.tile([128, 1024], F32, tag="b", name="pd2")
                    pd2r = pd2.rearrange("p (h y x) -> p h y x", h=2, y=16, x=32)
                    for h in range(2):
                        for ti, (dy, dx) in enumerate(TAPS):
                            oy0, oy1 = _clip(h * 16, h * 16 + 16,
                                             1 if dy == 0 else 0, 32)
                            if dy == 1:
                                ys, par = oy0, 0
                            elif dy == 0:
                                ys, par = oy0 - 1, 1
                            else:
                                ys, par = oy0, 1
                            rhs = h1nr[:, sh, ys:ys + (oy1 - oy0), par,
                                       dx // 2:dx // 2 + 32, dx % 2]
                            outp = pd2r[:, h, oy0 - h * 16:oy1 - h * 16, :]
                            t = TAPS.index((dy, dx))
                            nc.tensor.matmul(
                                outp, wcols(W2D_OFF + t * 128, 128), rhs,
                                start=(ti == 0), stop=(ti == len(TAPS) - 1))
                    nc.scalar.copy(out=d2[:, sh * 1024:(sh + 1) * 1024], in_=pd2)

                # conv3: 1x1, 64 -> 128 ; h3 [128=outc, (s, 1024px)]
                h3 = small.tile([128, 4096], F32)
                for a in range(2):
                    for nb in range(4):
                        pc3 = psA.tile([128, 512], F32, tag="a",
                                       name=f"pc3_{a}_{nb}")
                        nc.tensor.matmul(
                            pc3,
                            w_sb[64 * a:64 * a + 64, W3_OFF:W3_OFF + 128],
                            d2[64 * a:64 * a + 64, nb * 512:(nb + 1) * 512],
                            start=True, stop=True)
                        s_full = 2 * a + nb // 2
                        dst = h3[:, s_full * 1024 + (nb % 2) * 512:
                                 s_full * 1024 + (nb % 2) * 512 + 512]
                        if nb % 2 == 0:
                            nc.scalar.copy(out=dst, in_=pc3)
                        else:
                            nc.vector.tensor_copy(out=dst, in_=pc3)

                # BN2 stats + allreduce
                st2 = small.tile([128, 8, 6], F32)
                for i in range(8):
                    nc.vector.bn_stats(out=st2[:, i, :],
                                       in_=h3[:, i * 512:(i + 1) * 512])
                mv2 = small.tile([128, 2], F32)
                nc.vector.bn_aggr(out=mv2, in_=st2)
                sums2 = small.tile([128, 2], F32)
                tmp2 = small.tile([128, 1], F32)
                nc.vector.tensor_scalar_mul(out=sums2[:, 0:1], in0=mv2[:, 0:1],
                                            scalar1=4096.0)
                nc.vector.tensor_mul(out=tmp2, in0=mv2[:, 0:1], in1=mv2[:, 0:1])
                nc.vector.tensor_add(out=tmp2, in0=tmp2, in1=mv2[:, 1:2])
                nc.vector.tensor_scalar_mul(out=sums2[:, 1:2], in0=tmp2,
                                            scalar1=4096.0)
                bn2_in = dram.tile([256, 1], F32)
                bn2_out = dram.tile([8, 256, 1], F32)
                nc.gpsimd.dma_start(out=bn2_in, in_=sums2)
                nc.gpsimd.collective_compute(
                    "AllGather", mybir.AluOpType.bypass,
                    replica_groups=[list(range(N_CORES))],
                    ins=[bn2_in.opt()], outs=[bn2_out.opt()])
                gb1 = small.tile([128, 8, 2], F32)
                nc.gpsimd.dma_start(out=gb1, in_=bn2_out.rearrange(
                    "r (p t) one -> p r (t one)", p=128, t=2))
                h4s = small.tile([128, 4, 2], F32)
                nc.vector.tensor_add(out=h4s, in0=gb1[:, 0:4, :],
                                     in1=gb1[:, 4:8, :])
                h2s = small.tile([128, 2, 2], F32)
                nc.vector.tensor_add(out=h2s, in0=h4s[:, 0:2, :],
                                     in1=h4s[:, 2:4, :])
                red2 = small.tile([128, 2], F32)
                nc.vector.tensor_add(out=red2, in0=h2s[:, 0, :],
                                     in1=h2s[:, 1, :])
                s2t2 = small.tile([128, 2], F32)
                _bn_scale_bias(nc, s2t2, red2, w32, BN2_OFF, 32768.0,
                               eps_t, small, rows=128)

            # ---------------- main loop ----------------
            with (
                tc.tile_pool(name="xp", bufs=XP_BUFS) as xp,
                tc.tile_pool(name="dp", bufs=5) as dp,
                tc.tile_pool(name="pp", bufs=12) as pp,
                tc.tile_pool(name="stp", bufs=4) as stp,
            ):
                def new_x_tile(name):
                    # pad rows (-1, 32) and columns (0, 33) of every xp slot
                    # were zeroed once below; applies only write the interior
                    # (rows 1..32, cols 1..32 of the 34x34 grid).
                    return xp.tile([128, 34 * 34], F16, tag="X", name=name)

                # one-time zeroing of the pad columns of all X slots: the
                # dummies are simultaneously live (kept alive by the reads
                # below), so by pigeonhole they cover all slots.
                _dummies = []
                for i in range(XP_BUFS):
                    zt = xp.tile([128, 34 * 34], F16, tag="X", name=f"xz{i}")
                    # pad rows -1 and 32 (contiguous 34-elem spans)
                    nc.vector.tensor_copy(out=zt[:, 0:34], in_=z16[:, 0:34])
                    nc.vector.tensor_copy(out=zt[:, 1122:1156],
                                          in_=z16[:, 0:34])
                    for xc in (0, 33):
                        pads = bass.AP(tensor=zt.tensor, offset=zt.offset + xc,
                                       ap=[zt.ap[0], [34, 34]])
                        nc.vector.tensor_copy(out=pads, in_=z16[:, 0:34])
                    _dummies.append(zt)
                _pad_scratch = small.tile([128, 1], F16)
                for zt in _dummies:
                    nc.scalar.copy(out=_pad_scratch, in_=zt[:, 0:1])

                Xcur = {}
                for s in range(BPC):
                    xt = new_x_tile(f"X1_0_{s}")
                    xtr = xt.rearrange("p (y x) -> p y x", y=34, x=34)
                    h3r = h3.rearrange("p (s y x) -> p s y x", s=4, y=32, x=32)
                    nc.scalar.activation(out=xtr[:, 1:33, 1:33],
                                         in_=h3r[:, s, :, :],
                                         func=AF.Relu,
                                         scale=s2t2[:, 0:1], bias=s2t2[:, 1:2])
                    Xcur[(0, s)] = xt

                pooled_in = small.tile([128, 2, 4, 4], F32)

                for L in range(1, LAYERS + 1):
                    gs_in = sorted({g for (g, _s) in Xcur})
                    mgs = [2, 3] if L == LAYERS else [0, 1, 2, 3]
                    pe_gs = [g for g in gs_in
                             if g in PE_GROUPS or len(gs_in) == 1]
                    sb_gs = [g for g in gs_in if g not in pe_gs]
                    Xnext = {}
                    d16s = {}
                    for s in range(BPC):
                        d16s[s] = dp.tile([128, 4096], F16, tag="D",
                                          name=f"D{L}_{s}")

                    def xv(g, s, dy, dx):
                        Xr = Xcur[(g, s)].rearrange("p (y x) -> p y x",
                                                    y=34, x=34)
                        return Xr[:, dy:dy + 32, dx:dx + 32]

                    # phase B: SBUF depthwise via 9 full-row products and a
                    # pairwise add tree per (sample, group)
                    def emit_prod(eng, out, xin, g, ti):
                        if eng == 'A':
                            nc.scalar.activation(out=out, in_=xin,
                                                 func=AF.Identity,
                                                 scale=kvec(g, ti))
                        elif eng == 'G':
                            nc.gpsimd.tensor_scalar_mul(out=out, in0=xin,
                                                        scalar1=kvec(g, ti))
                        else:
                            nc.vector.tensor_scalar_mul(out=out, in0=xin,
                                                        scalar1=kvec(g, ti))

                    def emit_add(eng, out, in0, in1):
                        if eng == 'G':
                            nc.gpsimd.tensor_add(out=out, in0=in0, in1=in1)
                        else:
                            nc.vector.tensor_add(out=out, in0=in0, in1=in1)

                    for s in range(BPC):
                        prods = {}
                        for g in sb_gs:
                            for ti, (dy, dx) in enumerate(TAPS):
                                prod = pp.tile([128, 1024], F16, tag="P",
                                               name=f"P{L}_{s}_{g}_{ti}")
                                emit_prod(DW_PROD[g][ti], prod,
                                          xv(g, s, dy, dx), g, ti)
                                prods[(g, ti)] = prod
                        for g in sb_gs:
                            for ai, (d, e) in enumerate(ADD_TREE):
                                emit_add(DW_ADD[g][ai], prods[(g, d)],
                                         prods[(g, d)], prods[(g, e)])
                        for g in sb_gs:
                            emit_add(DW_ADD[g][7],
                                     d16s[s][:, g * 1024:g * 1024 + 1024],
                                     prods[(g, 0)], prods[(g, 8)])

                    # phase A: PE depthwise (PSUM) + ACT copies to fp16 D
                    for s in range(BPC):
                        for g in pe_gs:
                            for h in range(2):
                                pdw = psA.tile([128, 512], F32, tag="a",
                                               name=f"pdw{L}_{s}_{g}_{h}")
                                pdwr = pdw.rearrange("p (y x) -> p y x",
                                                     y=16, x=32)
                                for ti, (dy, dx) in enumerate(TAPS):
                                    rhs = xv(g, s, dy, dx)[:, h * 16:
                                                           h * 16 + 16, :]
                                    t = TAPS.index((dy, dx))
                                    nc.tensor.matmul(
                                        pdwr,
                                        w16cols(DW16_OFF + (g * 9 + t) * 128,
                                                128),
                                        rhs,
                                        start=(ti == 0),
                                        stop=(ti == len(TAPS) - 1))
                                nc.scalar.copy(
                                    out=d16s[s][:, g * 1024 + h * 512:
                                                g * 1024 + h * 512 + 512],
                                    in_=pdw)

                    # phase C: mix + instnorm + apply
                    for s in range(BPC):
                        d16 = d16s[s]
                        for mg in mgs:
                            pm = psB.tile([128, 1024], F32, tag="b",
                                          name=f"pm{L}_{s}_{mg}")
                            for h in range(2):
                                for ki, kg in enumerate(gs_in):
                                    nc.tensor.matmul(
                                        pm[:, h * 512:h * 512 + 512],
                                        w16cols(MIX16_OFF + kg * 512
                                                + mg * 128, 128),
                                        d16[:, kg * 1024 + h * 512:
                                            kg * 1024 + h * 512 + 512],
                                        start=(ki == 0),
                                        stop=(ki == len(gs_in) - 1))
                            st = stp.tile([128, 2, 6], F32, tag="st")
                            nc.vector.bn_stats(out=st[:, 0, :],
                                               in_=pm[:, 0:512])
                            nc.vector.bn_stats(out=st[:, 1, :],
                                               in_=pm[:, 512:1024])
                            mv = stp.tile([128, 2], F32, tag="mv")
                            nc.vector.bn_aggr(out=mv, in_=st)
                            sc = stp.tile([128, 1], F32, tag="sc")
                            tt = stp.tile([128, 1], F32, tag="tt")
                            nc.scalar.activation(out=sc, in_=mv[:, 1:2],
                                                 func=AF.Sqrt, bias=eps_t)
                            nc.vector.reciprocal(out=sc, in_=sc)
                            nc.vector.tensor_scalar_mul(
                                out=sc, in0=sc,
                                scalar1=w32[:, NGB_OFF + 2 * mg:
                                            NGB_OFF + 2 * mg + 1])
                            nc.vector.tensor_mul(out=tt, in0=mv[:, 0:1],
                                                 in1=sc)
                            nc.vector.tensor_scalar(
                                out=tt, in0=tt, scalar1=-1.0,
                                scalar2=w32[:, NGB_OFF + 2 * mg + 1:
                                            NGB_OFF + 2 * mg + 2],
                                op0=mybir.AluOpType.mult,
                                op1=mybir.AluOpType.add)
                            if L < LAYERS:
                                xt = new_x_tile(f"X{L + 1}_{mg}_{s}")
                                Xnext[(mg, s)] = xt
                                xtr = xt.rearrange("p (y x) -> p y x",
                                                   y=34, x=34)
                                pmr2 = pm.rearrange("p (y x) -> p y x",
                                                    y=32, x=32)
                                nc.scalar.activation(
                                    out=xtr[:, 1:33, 1:33], in_=pmr2,
                                    func=AF.Relu, scale=sc, bias=tt)
                            else:
                                pmr = pm.rearrange("p (y x) -> p y x",
                                                   y=32, x=32)
                                nc.scalar.activation(
                                    out=pooled_in[:, mg - 2, s, :],
                                    in_=pmr[:, HALF - 1:HALF + 1,
                                            HALF - 1:HALF + 1],
                                    func=AF.Identity, scale=sc, bias=tt)
                    Xcur = Xnext

                # ---------------- readout ----------------
                tadd = small.tile([128, 2, 4], F32)
                tadd2 = small.tile([128, 2, 4], F32)
                pooled = small.tile([128, 2, 4], F32R)
                nc.vector.tensor_add(out=tadd, in0=pooled_in[:, :, :, 0],
                                     in1=pooled_in[:, :, :, 1])
                nc.vector.tensor_add(out=tadd2, in0=pooled_in[:, :, :, 2],
                                     in1=pooled_in[:, :, :, 3])
                nc.vector.tensor_add(out=pooled, in0=tadd, in1=tadd2)
                y_sb = small.tile([128, 4, 8], F32)
                for mo in range(8):
                    mlen = 128 if mo < 7 else OUT - 7 * 128
                    pf = psA.tile([128, 512], F32, tag="a", name=f"pf{mo}")
                    for kgi in range(2):
                        nc.tensor.matmul(
                            pf[0:mlen, 0:4],
                            w_sb[:, FCW_OFF + kgi * 1000 + mo * 128:
                                 FCW_OFF + kgi * 1000 + mo * 128 + mlen],
                            pooled[:, kgi, :],
                            start=(kgi == 0), stop=(kgi == 1))
                    nc.scalar.activation(
                        out=y_sb[0:mlen, :, mo], in_=pf[0:mlen, 0:4],
                        func=AF.Identity,
                        bias=w32[0:mlen, FCB_OFF + mo:FCB_OFF + mo + 1],
                        scale=1.0)
                for s in range(BPC):
                    dst1 = bass.AP(tensor=y4.tensor, offset=OUT * s,
                                   ap=[[1, 128], [128, 7]])
                    nc.sync.dma_start(out=dst1, in_=y_sb[:, s, 0:7])
                    dst2 = bass.AP(tensor=y4.tensor, offset=OUT * s + 896,
                                   ap=[[1, 104]])
                    nc.sync.dma_start(out=dst2, in_=y_sb[0:104, s, 7])

    nc.finalize()
    return nc


def di_col(dx):
    # column index of conv1 tap dx within w1x block (emission order 1,0,2)
    return {1: 0, 0: 1, 2: 2}[dx]


def _bn_scale_bias(nc, out_st, sums, w32, gb_off, n_tot, eps_t, pool, rows):
    """out_st[:rows, 0] = gamma*rsqrt(var+eps); out_st[:rows, 1] = beta - mu*scale."""
    r = slice(0, rows)
    mu = pool.tile([128, 1], F32, name=f"mu{gb_off}")
    ex2 = pool.tile([128, 1], F32, name=f"ex2{gb_off}")
    var = pool.tile([128, 1], F32, name=f"var{gb_off}")
    nc.vector.tensor_scalar_mul(out=mu[r], in0=sums[r, 0:1], scalar1=1.0 / n_tot)
    nc.vector.tensor_scalar_mul(out=ex2[r], in0=sums[r, 1:2], scalar1=1.0 / n_tot)
    nc.vector.tensor_mul(out=var[r], in0=mu[r], in1=mu[r])
    nc.vector.tensor_sub(out=var[r], in0=ex2[r], in1=var[r])
    nc.scalar.activation(out=var[r], in_=var[r], func=AF.Sqrt, bias=eps_t[r])
    nc.vector.reciprocal(out=var[r], in_=var[r])
    nc.vector.tensor_scalar_mul(out=out_st[r, 0:1], in0=var[r],
                                scalar1=w32[r, gb_off:gb_off + 1])
    nc.vector.tensor_mul(out=mu[r], in0=mu[r], in1=out_st[r, 0:1])
    nc.vector.tensor_scalar(out=out_st[r, 1:2], in0=mu[r], scalar1=-1.0,
                            scalar2=w32[r, gb_off + 1:gb_off + 2],
                            op0=mybir.AluOpType.mult,
                            op1=mybir.AluOpType.add)


def _pack_weights(ds_w1, ds_w2, ds_w3, conv_w, graph_w, fc_w, fc_b,
                  bn1_g, bn1_b, bn2_g, bn2_b, norm_g, norm_b):
    wts = np.zeros((128, WCOLS), np.float32)
    w16 = np.zeros((128, W16COLS), np.float16)
    # pruned graph weight
    k = int((1.0 - PRUNE) * DIM * DIM)
    a = np.abs(graph_w).ravel()
    thresh = np.partition(a, -k)[-k]
    w_eff = np.where(np.abs(graph_w) >= thresh, graph_w, 0.0).astype(np.float32)
    # conv1 taps, paired block-diag:
    # rows 64*q + 27*a + 3*dy + c, cols 64*a + o = w1[o, c, dy, dx]
    for dx in range(3):
        dc = di_col(dx)
        blk = np.zeros((128, 128), np.float32)
        for qq in range(2):
            for aa in range(2):
                for dy in range(3):
                    for c in range(3):
                        blk[64 * qq + 27 * aa + 3 * dy + c,
                            64 * aa:64 * aa + 64] = ds_w1[:, c, dy, dx]
        wts[:, W1X_OFF + dc * 128:W1X_OFF + (dc + 1) * 128] = blk
    # conv2 diag-dup taps
    for t, (dy, dx) in enumerate(TAPS):
        blk = np.zeros((128, 128), np.float32)
        d = ds_w2[:, 0, dy, dx]
        for aa in range(2):
            idx = np.arange(64)
            blk[64 * aa + idx, 64 * aa + idx] = d
        wts[:, W2D_OFF + t * 128:W2D_OFF + (t + 1) * 128] = blk
    # conv3: [64a + c, o] = w3[o, c]
    w3 = ds_w3[:, :, 0, 0]  # [128, 64]
    wts[0:64, W3_OFF:W3_OFF + 128] = w3.T
    wts[64:128, W3_OFF:W3_OFF + 128] = w3.T
    # main dw diag taps (fp16)
    for g in range(4):
        for t, (dy, dx) in enumerate(TAPS):
            blk = np.zeros((128, 128), np.float16)
            idx = np.arange(128)
            blk[idx, idx] = conv_w[g * 128:(g + 1) * 128, 0, dy, dx]
            off = DW16_OFF + (g * 9 + t) * 128
            w16[:, off:off + 128] = blk
    # dw k vectors for the DVE path (f32)
    for g in range(4):
        for t, (dy, dx) in enumerate(TAPS):
            wts[:, KV_OFF + g * 9 + t] = conv_w[g * 128:(g + 1) * 128, 0, dy, dx]
    # mix (fp16): [p, kg*512 + mg*128 + j] = w_eff[mg*128 + j, kg*128 + p]
    weT = w_eff.T  # [in, out]
    for kg in range(4):
        w16[:, MIX16_OFF + kg * 512:MIX16_OFF + (kg + 1) * 512] = \
            weT[kg * 128:(kg + 1) * 128, :].astype(np.float16)
    # fc: [p, kg*1000 + m] = 0.25 * fc_w[m, kg*128 + p]
    for kg in range(2):
        wts[:, FCW_OFF + kg * 1000:FCW_OFF + (kg + 1) * 1000] = \
            0.25 * fc_w[:, kg * 128:(kg + 1) * 128].T
    # fc bias [p, mo]
    fcb = np.zeros((128, 8), np.float32)
    fb = np.zeros(1024, np.float32)
    fb[:OUT] = fc_b
    fcb[:, :] = fb.reshape(8, 128).T
    wts[:, FCB_OFF:FCB_OFF + 8] = fcb
    # bn gammas/betas
    wts[0:64, BN1_OFF] = bn1_g
    wts[64:128, BN1_OFF] = bn1_g
    wts[0:64, BN1_OFF + 1] = bn1_b
    wts[64:128, BN1_OFF + 1] = bn1_b
    wts[:, BN2_OFF] = bn2_g
    wts[:, BN2_OFF + 1] = bn2_b
    for g in range(4):
        wts[:, NGB_OFF + 2 * g] = norm_g[g * 128:(g + 1) * 128]
        wts[:, NGB_OFF + 2 * g + 1] = norm_b[g * 128:(g + 1) * 128]
    return wts, w16


_nc_cache = None
last_results = None


def kernel(**inputs):
    global _nc_cache, last_results
    inputs = {k: np.asarray(v, np.float32) for k, v in inputs.items()}
    wts, w16 = _pack_weights(
        inputs["ds_w1"], inputs["ds_w2"], inputs["ds_w3"], inputs["conv_w"],
        inputs["graph_w"], inputs["fc_w"], inputs["fc_b"],
        inputs["bn1_g"], inputs["bn1_b"], inputs["bn2_g"], inputs["bn2_b"],
        inputs["norm_g"], inputs["norm_b"])
    x = inputs["x"]
    if _nc_cache is None:
        _nc_cache = build_nc()
    nc = _nc_cache
    in_maps = [{"x4": np.ascontiguousarray(x[c * BPC:(c + 1) * BPC]),
                "wts": wts, "wts16": w16} for c in range(N_CORES)]
    res = run_bass_kernel_spmd(nc, in_maps, core_ids=list(range(N_CORES)))
    last_results = res
    return np.concatenate([res.results[c]["y4"] for c in range(N_CORES)], axis=0)


# revision 27
# speedup vs baseline: 1.5888x; 1.0103x over previous
"""Trainium2 Bass kernel for nn_DiscreteTimeNeuralGraph.

Strategy (8 NeuronCores, batch-parallel, engine-balanced):
  - Shard the batch of 32 across 8 cores (4 samples each); weights replicated.
  - Downsample path on-device; BatchNorm batch stats via per-core partial
    sums + one tiny AllReduce each.
  - Main loop in fp16 storage (X, D, weights; fp32 PSUM accumulation):
    depthwise 3x3 conv groups 0-2 as rect-clipped diagonal matmuls on PE;
    group 3 computed on the Vector engine as tensor_scalar(mul, 4x mode) +
    tensor_tensor(add, 2x mode) chains writing fp16 SBUF directly.
    PSUM->SBUF depthwise results copied (and cast to fp16) on the Pool
    engine, freeing ACT for the instnorm applies.
    Channel mix as fp16 blocked matmuls; instance-norm stats on VectorE;
    instnorm+ReLU fused into one ScalarE activation producing fp16 X.
  - Pad-column zeroing via engine memsets (not DMA).
  - Readout: center 2x2 mean (folded into fc weights) + fc matmul (f32r).

Top-k threshold for the pruned graph weight is computed on host
(np.partition) -- it is weight preprocessing of a replicated input.
"""
import numpy as np

import concourse.bass as bass
import concourse.tile as tile
from concourse import bacc, mybir
from concourse.bass_utils import run_bass_kernel_spmd

F32 = mybir.dt.float32
F32R = mybir.dt.float32r
F16 = mybir.dt.float16
AF = mybir.ActivationFunctionType
ALU = mybir.AluOpType

N_CORES = 8
B = 32
BPC = B // N_CORES          # 4 samples per core
DIM = 512
DS = 128
FEAT = 256
LAYERS = 8
IMG = 128
OUT = 1000
EPS = 1e-5
HALF = IMG // 4 // 2 - 1    # 15
PRUNE = 0.9

# f32 mega-weight column layout ([128, WCOLS])
W1X_OFF = 0                  # 3 dx-taps x [128,128] for conv1
W2D_OFF = W1X_OFF + 3 * 128  # 9 taps x [128,128] diag-dup for conv2
W3_OFF = W2D_OFF + 9 * 128   # [128,128] conv3 (w3 stacked twice on K)
FCW_OFF = W3_OFF + 128       # 2 kg x [128, 1000] fc lhsT (x0.25 pooled)
FCB_OFF = FCW_OFF + 2 * 1000  # [128, 8] fc bias chunks
BN1_OFF = FCB_OFF + 8          # [128, 2] bn1 gamma/beta (dup across halves)
BN2_OFF = BN1_OFF + 2          # [128, 2]
NGB_OFF = BN2_OFF + 2          # [128, 8] instnorm gamma/beta per group
KV_OFF = NGB_OFF + 8           # [128, 36] dw k vectors (g*9+t)
WCOLS = KV_OFF + 36

# fp16 weight layout ([128, W16COLS])
DW16_OFF = 0                   # 36 taps x [128,128] diag (g*9+t)
MIX16_OFF = DW16_OFF + 36 * 128  # 4 kg x [128, 512] = w_eff.T blocks
W16COLS = MIX16_OFF + 4 * 512

XP_BUFS = 22

# engine split for the main-loop depthwise conv: per group, per tap-index
# 'P' = whole group on PE (psum); otherwise per-tap: 'V' = DVE mul+add pair,
# 'A' = ACT product + DVE add, 'G' = Pool fused scalar_tensor_tensor.
# Tap 0 (the full-coverage (1,1) tap) of a non-PE group always inits on DVE.
# PE groups do the depthwise as diagonal matmuls into PSUM; SBUF groups
# compute 9 full-row tap products (engines per PROD table) and combine them
# with a pairwise add tree (engines per ADD table, ops in fixed order:
# P0+=P1, P2+=P3, P4+=P5, P6+=P7, P0+=P2, P4+=P6, P0+=P4, d16=P0+P8).
PE_GROUPS = (0, 1)
DW_PROD = {
    2: ['V', 'A', 'A', 'A', 'A', 'A', 'A', 'A', 'V'],
    3: ['V', 'V', 'G', 'G', 'G', 'G', 'G', 'V', 'V'],
}
DW_ADD = {
    2: ['V', 'V', 'V', 'G', 'V', 'V', 'V', 'V'],
    3: ['V', 'V', 'V', 'G', 'V', 'V', 'G', 'V'],
}
ADD_TREE = [(0, 1), (2, 3), (4, 5), (6, 7), (0, 2), (4, 6), (0, 4)]

# tap order: full-coverage tap first (start=True zeroes the psum region)
TAPS = [(1, 1), (0, 0), (0, 1), (0, 2), (1, 0), (1, 2), (2, 0), (2, 1), (2, 2)]


def _clip(lo, hi, lo2, hi2):
    return max(lo, lo2), min(hi, hi2)


def build_nc():
    nc = bacc.Bacc(num_devices=N_CORES)
    x4 = nc.dram_tensor("x4", [BPC, 3, IMG, IMG], F32R, kind="ExternalInput").ap()
    wts = nc.dram_tensor("wts", [128, WCOLS], F32R, kind="ExternalInput").ap()
    wts16 = nc.dram_tensor("wts16", [128, W16COLS], F16,
                           kind="ExternalInput").ap()
    y4 = nc.dram_tensor("y4", [BPC, OUT], F32, kind="ExternalOutput").ap()

    with tile.TileContext(nc) as tc:
        with (
            tc.tile_pool(name="wp", bufs=1) as wp,
            tc.tile_pool(name="wp16", bufs=1) as wp16,
            tc.tile_pool(name="small", bufs=1) as small,
            tc.tile_pool(name="psA", bufs=2, space="PSUM") as psA,
            tc.tile_pool(name="psB", bufs=3, space="PSUM") as psB,
            tc.tile_pool(name="dram", bufs=1, space="DRAM") as dram,
        ):
            w_sb = wp.tile([128, WCOLS], F32R)
            # ds conv weights first so conv1 can start early; rest after
            nc.sync.dma_start(out=w_sb[:, 0:FCW_OFF], in_=wts[:, 0:FCW_OFF])
            nc.sync.dma_start(out=w_sb[:, FCW_OFF:WCOLS],
                              in_=wts[:, FCW_OFF:WCOLS])
            w32 = w_sb.bitcast(F32)
            w16 = wp16.tile([128, W16COLS], F16)
            nc.sync.dma_start(out=w16, in_=wts16)

            def wcols(off, n):
                return w_sb[:, off:off + n]

            def w16cols(off, n):
                return w16[:, off:off + n]

            def kvec(g, t):
                return w32[:, KV_OFF + g * 9 + t:KV_OFF + g * 9 + t + 1]

            eps_t = small.tile([128, 1], F32)
            nc.vector.memset(eps_t, EPS)
            z32 = small.tile([128, 64], F32)
            nc.vector.memset(z32, 0.0)
            z16 = small.tile([128, 64], F16)
            nc.vector.memset(z16, 0.0)

            # ---------------- downsample ----------------
            with tc.tile_pool(name="ds1", bufs=1) as ds1:
                # im2col9: partition p = 32*s + 3*dy + c ; free = (oy 64, ix' 130)
                # ix' = ix + 1 (x padded by 1 on both sides)
                im9 = ds1.tile([128, 64 * 130], F32R)
                im9r = im9.rearrange("p (y x) -> p y x", y=64, x=130)
                # zero the x pads (cols 0 and 129)
                for xc in (0, 129):
                    im9_pads = bass.AP(tensor=im9.tensor,
                                       offset=im9.offset + xc,
                                       ap=[im9.ap[0], [130, 64]])
                    nc.vector.tensor_copy(out=im9_pads, in_=z32[:, 0:64])
                # row oy=0 is out of range for dy=0 taps: zero it everywhere
                # first (dy=1/2 loads overwrite their row 0 afterwards; cols
                # 0/129 are the x-pads zeroed above)
                nc.vector.tensor_copy(out=im9[:, 1:65], in_=z32[:, 0:64])
                nc.vector.tensor_copy(out=im9[:, 65:129], in_=z32[:, 0:64])
                # x rows: iy = 2*oy + dy - 1
                # partition base: sample s -> 64*(s%2) + 27*(s//2)
                x4r = x4.rearrange("s c (y2 two) x -> s c y2 two x", two=2)
                for s in range(BPC):
                    for dy in range(3):
                        p0 = 64 * (s % 2) + 27 * (s // 2) + 3 * dy
                        if dy == 0:
                            # oy in [1,64): iy = 2*(oy-1)+1
                            nc.sync.dma_start(
                                out=im9r[p0:p0 + 3, 1:64, 1:129],
                                in_=x4r[s, :, 0:63, 1, :])
                        elif dy == 1:
                            nc.sync.dma_start(
                                out=im9r[p0:p0 + 3, :, 1:129],
                                in_=x4r[s, :, :, 0, :])
                        else:
                            nc.sync.dma_start(
                                out=im9r[p0:p0 + 3, :, 1:129],
                                in_=x4r[s, :, :, 1, :])

                # conv1: out h1 [128 = 64*(s//2)+ch, (s%2, oy 64, ox 64)]
                h1 = ds1.tile([128, 8192], F32)
                h1r = h1.rearrange("p (sh y x) -> p sh y x", sh=2, y=64, x=64)
                # im2col x-read: ix' = 2*ox + dx (x2 = ox + dx//2, tx = dx%2)
                # paired matmul: K=54 block-diag covers samples (q, q+2):
                # out partitions 0-63 <- sample q, 64-127 <- sample q+2.
                im9x = im9.rearrange("p (y x2 two) -> p y x2 two", x2=65, two=2)
                for q in range(2):
                    for yb in range(4):           # 16-oy blocks
                        for h in range(2):
                            pc1 = psA.tile([128, 512], F32, tag="a",
                                           name="pc1")
                            pc1r = pc1.rearrange("p (y x) -> p y x", y=8, x=64)
                            oy0 = yb * 16 + h * 8
                            for di, dx in enumerate([1, 0, 2]):
                                rhs = im9x[64 * q:64 * q + 54, oy0:oy0 + 8,
                                           dx // 2:dx // 2 + 64, dx % 2]
                                lhsT = w_sb[64 * q:64 * q + 54,
                                            W1X_OFF + di_col(dx) * 128:
                                            W1X_OFF + di_col(dx) * 128 + 128]
                                nc.tensor.matmul(pc1r, lhsT, rhs,
                                                 start=(di == 0), stop=(di == 2),
                                                 tile_position=(64 * q, 0))
                            if (q + yb + h) % 2 == 0:
                                nc.scalar.copy(
                                    out=h1r[:, q, oy0:oy0 + 8, :], in_=pc1)
                            else:
                                nc.vector.tensor_copy(
                                    out=h1r[:, q, oy0:oy0 + 8, :], in_=pc1)

                # BN1 partial stats
                st1 = small.tile([128, 16, 6], F32)
                for i in range(16):
                    nc.vector.bn_stats(out=st1[:, i, :],
                                       in_=h1[:, i * 512:(i + 1) * 512])
                mv1 = small.tile([128, 2], F32)
                nc.vector.bn_aggr(out=mv1, in_=st1)
                sums1 = small.tile([128, 2], F32)
                tmp1 = small.tile([128, 1], F32)
                nc.vector.tensor_scalar_mul(out=sums1[:, 0:1], in0=mv1[:, 0:1],
                                            scalar1=8192.0)
                nc.vector.tensor_mul(out=tmp1, in0=mv1[:, 0:1], in1=mv1[:, 0:1])
                nc.vector.tensor_add(out=tmp1, in0=tmp1, in1=mv1[:, 1:2])
                nc.vector.tensor_scalar_mul(out=sums1[:, 1:2], in0=tmp1,
                                            scalar1=8192.0)
                # stage as [q, j, t]: channel q, partition-half j, stat t
                bn1_in = dram.tile([64, 2, 2], F32)
                bn1_out = dram.tile([8, 64, 2, 2], F32)
                nc.gpsimd.dma_start(
                    out=bn1_in.rearrange("q j t -> j q t"), in_=sums1)
                nc.gpsimd.collective_compute(
                    "AllGather", mybir.AluOpType.bypass,
                    replica_groups=[list(range(N_CORES))],
                    ins=[bn1_in.opt()], outs=[bn1_out.opt()])
                # readback: partition p gets (j = channel-half, rank) slots of
                # channel p%64; two parallel DMAs fill rows 0:64 and 64:128
                # with identical data, so the partition-halves sum (channel
                # stats) and the gamma scale land on all 128 rows directly.
                ga1 = small.tile([128, 8, 4], F32)
                src_ap = bn1_out.rearrange("r q j t -> q r (j t)")
                nc.gpsimd.dma_start(out=ga1[0:64], in_=src_ap)
                nc.gpsimd.dma_start(out=ga1[64:128], in_=src_ap)
                g4 = small.tile([128, 8, 2], F32)
                nc.vector.tensor_add(out=g4, in0=ga1[:, :, 0:2],
                                     in1=ga1[:, :, 2:4])
                g2t = small.tile([128, 4, 2], F32)
                nc.vector.tensor_add(out=g2t, in0=g4[:, 0:4, :],
                                     in1=g4[:, 4:8, :])
                g1t = small.tile([128, 2, 2], F32)
                nc.vector.tensor_add(out=g1t, in0=g2t[:, 0:2, :],
                                     in1=g2t[:, 2:4, :])
                red1 = small.tile([128, 2], F32)
                nc.vector.tensor_add(out=red1, in0=g1t[:, 0, :],
                                     in1=g1t[:, 1, :])

                s1t1 = small.tile([128, 2], F32)
                _bn_scale_bias(nc, s1t1, red1, w32, BN1_OFF, 131072.0,
                               eps_t, small, rows=128)

                # apply BN1 + relu -> h1n (f32r), x padded to 66 (ix' = ix+1)
                h1n = ds1.tile([128, 2 * 64 * 66], F32R)
                h1nr3 = h1n.rearrange("p (sh y x) -> p sh y x",
                                      sh=2, y=64, x=66)
                for sh in range(2):
                    for xc in (0, 65):
                        h1n_pads = bass.AP(tensor=h1n.tensor,
                                           offset=h1n.offset + 4224 * sh + xc,
                                           ap=[h1n.ap[0], [66, 64]])
                        nc.vector.tensor_copy(out=h1n_pads, in_=z32[:, 0:64])
                h1r4 = h1.rearrange("p (sh y x) -> p sh y x", sh=2, y=64, x=64)
                for sh in range(2):
                    nc.scalar.activation(out=h1nr3[:, sh, :, 1:65],
                                         in_=h1r4[:, sh, :, :], func=AF.Relu,
                                         scale=s1t1[:, 0:1], bias=s1t1[:, 1:2])

                # conv2: depthwise 3x3 stride 2 -> d2 [128, (sh, 32, 32)]
                # row iy = 2*oy + dy - 1 (unpadded), col ix' = 2*ox + dx (padded)
                h1nr = h1n.rearrange(
                    "p (sh y2 ty x2 tx) -> p sh y2 ty x2 tx",
                    sh=2, y2=32, ty=2, x2=33, tx=2)
                d2 = ds1.tile([128, 2048], F32R)
                for sh in range(2):
                    pd2 = psB.tile([128, 1024], F32, tag="b", name="pd2")
                    pd2r = pd2.rearrange("p (h y x) -> p h y x", h=2, y=16, x=32)
                    for h in range(2):
                        for ti, (dy, dx) in enumerate(TAPS):
                            oy0, oy1 = _clip(h * 16, h * 16 + 16,
                                             1 if dy == 0 else 0, 32)
                            if dy == 1:
                                ys, par = oy0, 0
                            elif dy == 0:
                                ys, par = oy0 - 1, 1
                            else:
                                ys, par = oy0, 1
                            rhs = h1nr[:, sh, ys:ys + (oy1 - oy0), par,
                                       dx // 2:dx // 2 + 32, dx % 2]
                            outp = pd2r[:, h, oy0 - h * 16:oy1 - h * 16, :]
                            t = TAPS.index((dy, dx))
                            nc.tensor.matmul(
                                outp, wcols(W2D_OFF + t * 128, 128), rhs,
                                start=(ti == 0), stop=(ti == len(TAPS) - 1))
                    nc.scalar.copy(out=d2[:, sh * 1024:(sh + 1) * 1024], in_=pd2)

                # conv3: 1x1, 64 -> 128 ; h3 [128=outc, (s, 1024px)]
                h3 = small.tile([128, 4096], F32)
                for a in range(2):
                    for nb in range(4):
                        pc3 = psA.tile([128, 512], F32, tag="a",
                                       name=f"pc3_{a}_{nb}")
                        nc.tensor.matmul(
                            pc3,
                            w_sb[64 * a:64 * a + 64, W3_OFF:W3_OFF + 128],
                            d2[64 * a:64 * a + 64, nb * 512:(nb + 1) * 512],
                            start=True, stop=True)
                        s_full = 2 * a + nb // 2
                        dst = h3[:, s_full * 1024 + (nb % 2) * 512:
                                 s_full * 1024 + (nb % 2) * 512 + 512]
                        if nb % 2 == 0:
                            nc.scalar.copy(out=dst, in_=pc3)
                        else:
                            nc.vector.tensor_copy(out=dst, in_=pc3)

                # BN2 stats + allreduce
                st2 = small.tile([128, 8, 6], F32)
                for i in range(8):
                    nc.vector.bn_stats(out=st2[:, i, :],
                                       in_=h3[:, i * 512:(i + 1) * 512])
                mv2 = small.tile([128, 2], F32)
                nc.vector.bn_aggr(out=mv2, in_=st2)
                sums2 = small.tile([128, 2], F32)
                tmp2 = small.tile([128, 1], F32)
                nc.vector.tensor_scalar_mul(out=sums2[:, 0:1], in0=mv2[:, 0:1],
                                            scalar1=4096.0)
                nc.vector.tensor_mul(out=tmp2, in0=mv2[:, 0:1], in1=mv2[:, 0:1])
                nc.vector.tensor_add(out=tmp2, in0=tmp2, in1=mv2[:, 1:2])
                nc.vector.tensor_scalar_mul(out=sums2[:, 1:2], in0=tmp2,
                                            scalar1=4096.0)
                bn2_in = dram.tile([256, 1], F32)
                bn2_out = dram.tile([8, 256, 1], F32)
                nc.gpsimd.dma_start(out=bn2_in, in_=sums2)
                nc.gpsimd.collective_compute(
                    "AllGather", mybir.AluOpType.bypass,
                    replica_groups=[list(range(N_CORES))],
                    ins=[bn2_in.opt()], outs=[bn2_out.opt()])
                gb1 = small.tile([128, 8, 2], F32)
                nc.gpsimd.dma_start(out=gb1, in_=bn2_out.rearrange(
                    "r (p t) one -> p r (t one)", p=128, t=2))
                h4s = small.tile([128, 4, 2], F32)
                nc.vector.tensor_add(out=h4s, in0=gb1[:, 0:4, :],
                                     in1=gb1[:, 4:8, :])
                h2s = small.tile([128, 2, 2], F32)
                nc.vector.tensor_add(out=h2s, in0=h4s[:, 0:2, :],
                                     in1=h4s[:, 2:4, :])
                red2 = small.tile([128, 2], F32)
                nc.vector.tensor_add(out=red2, in0=h2s[:, 0, :],
                                     in1=h2s[:, 1, :])
                s2t2 = small.tile([128, 2], F32)
                _bn_scale_bias(nc, s2t2, red2, w32, BN2_OFF, 32768.0,
                               eps_t, small, rows=128)

            # ---------------- main loop ----------------
            with (
                tc.tile_pool(name="xp", bufs=XP_BUFS) as xp,
                tc.tile_pool(name="dp", bufs=5) as dp,
                tc.tile_pool(name="pp", bufs=12) as pp,
                tc.tile_pool(name="stp", bufs=4) as stp,
            ):
                def new_x_tile(name):
                    # pad rows (-1, 32) and columns (0, 33) of every xp slot
                    # were zeroed once below; applies only write the interior
                    # (rows 1..32, cols 1..32 of the 34x34 grid).
                    return xp.tile([128, 34 * 34], F16, tag="X", name=name)

                # one-time zeroing of the pad columns of all X slots: the
                # dummies are simultaneously live (kept alive by the reads
                # below), so by pigeonhole they cover all slots.
                _dummies = []
                for i in range(XP_BUFS):
                    zt = xp.tile([128, 34 * 34], F16, tag="X", name=f"xz{i}")
                    # pad rows -1 and 32 (contiguous 34-elem spans)
                    nc.vector.tensor_copy(out=zt[:, 0:34], in_=z16[:, 0:34])
                    nc.vector.tensor_copy(out=zt[:, 1122:1156],
                                          in_=z16[:, 0:34])
                    for xc in (0, 33):
                        pads = bass.AP(tensor=zt.tensor, offset=zt.offset + xc,
                                       ap=[zt.ap[0], [34, 34]])
                        nc.vector.tensor_copy(out=pads, in_=z16[:, 0:34])
                    _dummies.append(zt)
                _pad_scratch = small.tile([128, 1], F16)
                for zt in _dummies:
                    nc.scalar.copy(out=_pad_scratch, in_=zt[:, 0:1])

                Xcur = {}
                for s in range(BPC):
                    xt = new_x_tile(f"X1_0_{s}")
                    xtr = xt.rearrange("p (y x) -> p y x", y=34, x=34)
                    h3r = h3.rearrange("p (s y x) -> p s y x", s=4, y=32, x=32)
                    nc.scalar.activation(out=xtr[:, 1:33, 1:33],
                                         in_=h3r[:, s, :, :],
                                         func=AF.Relu,
                                         scale=s2t2[:, 0:1], bias=s2t2[:, 1:2])
                    Xcur[(0, s)] = xt

                pooled_in = small.tile([128, 2, 4, 4], F32)

                for L in range(1, LAYERS + 1):
                    gs_in = sorted({g for (g, _s) in Xcur})
                    mgs = [2, 3] if L == LAYERS else [0, 1, 2, 3]
                    pe_gs = [g for g in gs_in
                             if g in PE_GROUPS or len(gs_in) == 1]
                    sb_gs = [g for g in gs_in if g not in pe_gs]
                    Xnext = {}
                    d16s = {}
                    for s in range(BPC):
                        d16s[s] = dp.tile([128, 4096], F16, tag="D",
                                          name=f"D{L}_{s}")

                    def xv(g, s, dy, dx):
                        Xr = Xcur[(g, s)].rearrange("p (y x) -> p y x",
                                                    y=34, x=34)
                        return Xr[:, dy:dy + 32, dx:dx + 32]

                    # phase B: SBUF depthwise via 9 full-row products and a
                    # pairwise add tree per (sample, group)
                    def emit_prod(eng, out, xin, g, ti):
                        if eng == 'A':
                            nc.scalar.activation(out=out, in_=xin,
                                                 func=AF.Identity,
                                                 scale=kvec(g, ti))
                        elif eng == 'G':
                            nc.gpsimd.tensor_scalar_mul(out=out, in0=xin,
                                                        scalar1=kvec(g, ti))
                        else:
                            nc.vector.tensor_scalar_mul(out=out, in0=xin,
                                                        scalar1=kvec(g, ti))

                    def emit_add(eng, out, in0, in1):
                        if eng == 'G':
                            nc.gpsimd.tensor_add(out=out, in0=in0, in1=in1)
                        else:
                            nc.vector.tensor_add(out=out, in0=in0, in1=in1)

                    for s in range(BPC):
                        prods = {}
                        for g in sb_gs:
                            for ti, (dy, dx) in enumerate(TAPS):
                                prod = pp.tile([128, 1024], F16, tag="P",
                                               name=f"P{L}_{s}_{g}_{ti}")
                                emit_prod(DW_PROD[g][ti], prod,
                                          xv(g, s, dy, dx), g, ti)
                                prods[(g, ti)] = prod
                        for g in sb_gs:
                            for ai, (d, e) in enumerate(ADD_TREE):
                                emit_add(DW_ADD[g][ai], prods[(g, d)],
                                         prods[(g, d)], prods[(g, e)])
                        for g in sb_gs:
                            emit_add(DW_ADD[g][7],
                                     d16s[s][:, g * 1024:g * 1024 + 1024],
                                     prods[(g, 0)], prods[(g, 8)])

                    # phase A: PE depthwise (PSUM) + ACT copies to fp16 D
                    for s in range(BPC):
                        for g in pe_gs:
                            for h in range(2):
                                pdw = psA.tile([128, 512], F32, tag="a",
                                               name=f"pdw{L}_{s}_{g}_{h}")
                                pdwr = pdw.rearrange("p (y x) -> p y x",
                                                     y=16, x=32)
                                for ti, (dy, dx) in enumerate(TAPS):
                                    rhs = xv(g, s, dy, dx)[:, h * 16:
                                                           h * 16 + 16, :]
                                    t = TAPS.index((dy, dx))
                                    nc.tensor.matmul(
                                        pdwr,
                                        w16cols(DW16_OFF + (g * 9 + t) * 128,
                                                128),
                                        rhs,
                                        start=(ti == 0),
                                        stop=(ti == len(TAPS) - 1))
                                nc.scalar.copy(
                                    out=d16s[s][:, g * 1024 + h * 512:
                                                g * 1024 + h * 512 + 512],
                                    in_=pdw)

                    # phase C: mix + instnorm + apply
                    for s in range(BPC):
                        d16 = d16s[s]
                        for mg in mgs:
                            pm = psB.tile([128, 1024], F32, tag="b",
                                          name=f"pm{L}_{s}_{mg}")
                            for h in range(2):
                                for ki, kg in enumerate(gs_in):
                                    nc.tensor.matmul(
                                        pm[:, h * 512:h * 512 + 512],
                                        w16cols(MIX16_OFF + kg * 512
                                                + mg * 128, 128),
                                        d16[:, kg * 1024 + h * 512:
                                            kg * 1024 + h * 512 + 512],
                                        start=(ki == 0),
                                        stop=(ki == len(gs_in) - 1))
                            st = stp.tile([128, 2, 6], F32, tag="st")
                            nc.vector.bn_stats(out=st[:, 0, :],
                                               in_=pm[:, 0:512])
                            nc.vector.bn_stats(out=st[:, 1, :],
                                               in_=pm[:, 512:1024])
                            mv = stp.tile([128, 2], F32, tag="mv")
                            nc.vector.bn_aggr(out=mv, in_=st)
                            sc = stp.tile([128, 1], F32, tag="sc")
                            tt = stp.tile([128, 1], F32, tag="tt")
                            nc.scalar.activation(out=sc, in_=mv[:, 1:2],
                                                 func=AF.Sqrt, bias=eps_t)
                            nc.vector.reciprocal(out=sc, in_=sc)
                            nc.vector.tensor_scalar_mul(
                                out=sc, in0=sc,
                                scalar1=w32[:, NGB_OFF + 2 * mg:
                                            NGB_OFF + 2 * mg + 1])
                            nc.vector.tensor_mul(out=tt, in0=mv[:, 0:1],
                                                 in1=sc)
                            nc.vector.tensor_scalar(
                                out=tt, in0=tt, scalar1=-1.0,
                                scalar2=w32[:, NGB_OFF + 2 * mg + 1:
                                            NGB_OFF + 2 * mg + 2],
                                op0=mybir.AluOpType.mult,
                                op1=mybir.AluOpType.add)
                            if L < LAYERS:
                                xt = new_x_tile(f"X{L + 1}_{mg}_{s}")
                                Xnext[(mg, s)] = xt
                                xtr = xt.rearrange("p (y x) -> p y x",
                                                   y=34, x=34)
                                pmr2 = pm.rearrange("p (y x) -> p y x",
                                                    y=32, x=32)
                                nc.scalar.activation(
                                    out=xtr[:, 1:33, 1:33], in_=pmr2,
                                    func=AF.Relu, scale=sc, bias=tt)
                            else:
                                pmr = pm.rearrange("p (y x) -> p y x",
                                                   y=32, x=32)
                                nc.scalar.activation(
                                    out=pooled_in[:, mg - 2, s, :],
                                    in_=pmr[:, HALF - 1:HALF + 1,
                                            HALF - 1:HALF + 1],
                                    func=AF.Identity, scale=sc, bias=tt)
                    Xcur = Xnext

                # ---------------- readout ----------------
                tadd = small.tile([128, 2, 4], F32)
                tadd2 = small.tile([128, 2, 4], F32)
                pooled = small.tile([128, 2, 4], F32R)
                nc.vector.tensor_add(out=tadd, in0=pooled_in[:, :, :, 0],
                                     in1=pooled_in[:, :, :, 1])
                nc.vector.tensor_add(out=tadd2, in0=pooled_in[:, :, :, 2],
                                     in1=pooled_in[:, :, :, 3])
                nc.vector.tensor_add(out=pooled, in0=tadd, in1=tadd2)
                y_sb = small.tile([128, 4, 8], F32)
                for mo in range(8):
                    mlen = 128 if mo < 7 else OUT - 7 * 128
                    pf = psA.tile([128, 512], F32, tag="a", name=f"pf{mo}")
                    for kgi in range(2):
                        nc.tensor.matmul(
                            pf[0:mlen, 0:4],
                            w_sb[:, FCW_OFF + kgi * 1000 + mo * 128:
                                 FCW_OFF + kgi * 1000 + mo * 128 + mlen],
                            pooled[:, kgi, :],
                            start=(kgi == 0), stop=(kgi == 1))
                    nc.scalar.activation(
                        out=y_sb[0:mlen, :, mo], in_=pf[0:mlen, 0:4],
                        func=AF.Identity,
                        bias=w32[0:mlen, FCB_OFF + mo:FCB_OFF + mo + 1],
                        scale=1.0)
                for s in range(BPC):
                    dst1 = bass.AP(tensor=y4.tensor, offset=OUT * s,
                                   ap=[[1, 128], [128, 7]])
                    nc.sync.dma_start(out=dst1, in_=y_sb[:, s, 0:7])
                    dst2 = bass.AP(tensor=y4.tensor, offset=OUT * s + 896,
                                   ap=[[1, 104]])
                    nc.sync.dma_start(out=dst2, in_=y_sb[0:104, s, 7])

    nc.finalize()
    return nc


def di_col(dx):
    # column index of conv1 tap dx within w1x block (emission order 1,0,2)
    return {1: 0, 0: 1, 2: 2}[dx]


def _bn_scale_bias(nc, out_st, sums, w32, gb_off, n_tot, eps_t, pool, rows):
    """out_st[:rows, 0] = gamma*rsqrt(var+eps); out_st[:rows, 1] = beta - mu*scale."""
    r = slice(0, rows)
    mu = pool.tile([128, 1], F32, name=f"mu{gb_off}")
    ex2 = pool.tile([128, 1], F32, name=f"ex2{gb_off}")
    var = pool.tile([128, 1], F32, name=f"var{gb_off}")
    nc.vector.tensor_scalar_mul(out=mu[r], in0=sums[r, 0:1], scalar1=1.0 / n_tot)
    nc.vector.tensor_scalar_mul(out=ex2[r], in0=sums[r, 1:2], scalar1=1.0 / n_tot)
    nc.vector.tensor_mul(out=var[r], in0=mu[r], in1=mu[r])
    nc.vector.tensor_sub(out=var[r], in0=ex2[r], in1=var[r])
    nc.scalar.activation(out=var[r], in_=var[r], func=AF.Sqrt, bias=eps_t[r])
    nc.vector.reciprocal(out=var[r], in_=var[r])
    nc.vector.tensor_scalar_mul(out=out_st[r, 0:1], in0=var[r],
                                scalar1=w32[r, gb_off:gb_off + 1])
    nc.vector.tensor_mul(out=mu[r], in0=mu[r], in1=out_st[r, 0:1])
    nc.vector.tensor_scalar(out=out_st[r, 1:2], in0=mu[r], scalar1=-1.0,
                            scalar2=w32[r, gb_off + 1:gb_off + 2],
                            op0=mybir.AluOpType.mult,
                            op1=mybir.AluOpType.add)


def _pack_weights(ds_w1, ds_w2, ds_w3, conv_w, graph_w, fc_w, fc_b,
                  bn1_g, bn1_b, bn2_g, bn2_b, norm_g, norm_b):
    wts = np.zeros((128, WCOLS), np.float32)
    w16 = np.zeros((128, W16COLS), np.float16)
    # pruned graph weight
    k = int((1.0 - PRUNE) * DIM * DIM)
    a = np.abs(graph_w).ravel()
    thresh = np.partition(a, -k)[-k]
    w_eff = np.where(np.abs(graph_w) >= thresh, graph_w, 0.0).astype(np.float32)
    # conv1 taps, paired block-diag:
    # rows 64*q + 27*a + 3*dy + c, cols 64*a + o = w1[o, c, dy, dx]
    for dx in range(3):
        dc = di_col(dx)
        blk = np.zeros((128, 128), np.float32)
        for qq in range(2):
            for aa in range(2):
                for dy in range(3):
                    for c in range(3):
                        blk[64 * qq + 27 * aa + 3 * dy + c,
                            64 * aa:64 * aa + 64] = ds_w1[:, c, dy, dx]
        wts[:, W1X_OFF + dc * 128:W1X_OFF + (dc + 1) * 128] = blk
    # conv2 diag-dup taps
    for t, (dy, dx) in enumerate(TAPS):
        blk = np.zeros((128, 128), np.float32)
        d = ds_w2[:, 0, dy, dx]
        for aa in range(2):
            idx = np.arange(64)
            blk[64 * aa + idx, 64 * aa + idx] = d
        wts[:, W2D_OFF + t * 128:W2D_OFF + (t + 1) * 128] = blk
    # conv3: [64a + c, o] = w3[o, c]
    w3 = ds_w3[:, :, 0, 0]  # [128, 64]
    wts[0:64, W3_OFF:W3_OFF + 128] = w3.T
    wts[64:128, W3_OFF:W3_OFF + 128] = w3.T
    # main dw diag taps (fp16)
    for g in range(4):
        for t, (dy, dx) in enumerate(TAPS):
            blk = np.zeros((128, 128), np.float16)
            idx = np.arange(128)
            blk[idx, idx] = conv_w[g * 128:(g + 1) * 128, 0, dy, dx]
            off = DW16_OFF + (g * 9 + t) * 128
            w16[:, off:off + 128] = blk
    # dw k vectors for the DVE path (f32)
    for g in range(4):
        for t, (dy, dx) in enumerate(TAPS):
            wts[:, KV_OFF + g * 9 + t] = conv_w[g * 128:(g + 1) * 128, 0, dy, dx]
    # mix (fp16): [p, kg*512 + mg*128 + j] = w_eff[mg*128 + j, kg*128 + p]
    weT = w_eff.T  # [in, out]
    for kg in range(4):
        w16[:, MIX16_OFF + kg * 512:MIX16_OFF + (kg + 1) * 512] = \
            weT[kg * 128:(kg + 1) * 128, :].astype(np.float16)
    # fc: [p, kg*1000 + m] = 0.25 * fc_w[m, kg*128 + p]
    for kg in range(2):
        wts[:, FCW_OFF + kg * 1000:FCW_OFF + (kg + 1) * 1000] = \
            0.25 * fc_w[:, kg * 128:(kg + 1) * 128].T
    # fc bias [p, mo]
    fcb = np.zeros((128, 8), np.float32)
    fb = np.zeros(1024, np.float32)
    fb[:OUT] = fc_b
    fcb[:, :] = fb.reshape(8, 128).T
    wts[:, FCB_OFF:FCB_OFF + 8] = fcb
    # bn gammas/betas
    wts[0:64, BN1_OFF] = bn1_g
    wts[64:128, BN1_OFF] = bn1_g
    wts[0:64, BN1_OFF + 1] = bn1_b
    wts[64:128, BN1_OFF + 1] = bn1_b
    wts[:, BN2_OFF] = bn2_g
    wts[:, BN2_OFF + 1] = bn2_b
    for g in range(4):
        wts[:, NGB_OFF + 2 * g] = norm_g[g * 128:(g + 1) * 128]
        wts[:, NGB_OFF + 2 * g + 1] = norm_b[g * 128:(g + 1) * 128]
    return wts, w16


_nc_cache = None
last_results = None


def kernel(**inputs):
    global _nc_cache, last_results
    inputs = {k: np.asarray(v, np.float32) for k, v in inputs.items()}
    wts, w16 = _pack_weights(
        inputs["ds_w1"], inputs["ds_w2"], inputs["ds_w3"], inputs["conv_w"],
        inputs["graph_w"], inputs["fc_w"], inputs["fc_b"],
        inputs["bn1_g"], inputs["bn1_b"], inputs["bn2_g"], inputs["bn2_b"],
        inputs["norm_g"], inputs["norm_b"])
    x = inputs["x"]
    if _nc_cache is None:
        _nc_cache = build_nc()
    nc = _nc_cache
    in_maps = [{"x4": np.ascontiguousarray(x[c * BPC:(c + 1) * BPC]),
                "wts": wts, "wts16": w16} for c in range(N_CORES)]
    res = run_bass_kernel_spmd(nc, in_maps, core_ids=list(range(N_CORES)))
    last_results = res
    return np.concatenate([res.results[c]["y4"] for c in range(N_CORES)], axis=0)


# revision 29
# speedup vs baseline: 1.6635x; 1.0470x over previous
"""Trainium2 Bass kernel for nn_DiscreteTimeNeuralGraph.

Strategy (8 NeuronCores, batch-parallel, engine-balanced):
  - Shard the batch of 32 across 8 cores (4 samples each); weights replicated.
  - Downsample path on-device; BatchNorm batch stats via per-core partial
    sums + one tiny AllReduce each.
  - Main loop in fp16 storage (X, D, weights; fp32 PSUM accumulation):
    depthwise 3x3 conv groups 0-2 as rect-clipped diagonal matmuls on PE;
    group 3 computed on the Vector engine as tensor_scalar(mul, 4x mode) +
    tensor_tensor(add, 2x mode) chains writing fp16 SBUF directly.
    PSUM->SBUF depthwise results copied (and cast to fp16) on the Pool
    engine, freeing ACT for the instnorm applies.
    Channel mix as fp16 blocked matmuls; instance-norm stats on VectorE;
    instnorm+ReLU fused into one ScalarE activation producing fp16 X.
  - Pad-column zeroing via engine memsets (not DMA).
  - Readout: center 2x2 mean (folded into fc weights) + fc matmul (f32r).

Top-k threshold for the pruned graph weight is computed on host
(np.partition) -- it is weight preprocessing of a replicated input.
"""
import numpy as np

import concourse.bass as bass
import concourse.tile as tile
from concourse import bacc, mybir
from concourse.bass_utils import run_bass_kernel_spmd

F32 = mybir.dt.float32
F32R = mybir.dt.float32r
F16 = mybir.dt.float16
AF = mybir.ActivationFunctionType
ALU = mybir.AluOpType

N_CORES = 8
B = 32
BPC = B // N_CORES          # 4 samples per core
DIM = 512
DS = 128
FEAT = 256
LAYERS = 8
IMG = 128
OUT = 1000
EPS = 1e-5
HALF = IMG // 4 // 2 - 1    # 15
PRUNE = 0.9

# f32 mega-weight column layout ([128, WCOLS])
W1X_OFF = 0                  # 3 dx-taps x [128,128] for conv1
W2D_OFF = W1X_OFF + 3 * 128  # 9 taps x [128,128] diag-dup for conv2
W3_OFF = W2D_OFF + 9 * 128   # [128,128] conv3 (w3 stacked twice on K)
FCW_OFF = W3_OFF + 128       # 2 kg x [128, 1000] fc lhsT (x0.25 pooled)
FCB_OFF = FCW_OFF + 2 * 1000  # [128, 8] fc bias chunks
BN1_OFF = FCB_OFF + 8          # [128, 2] bn1 gamma/beta (dup across halves)
BN2_OFF = BN1_OFF + 2          # [128, 2]
NGB_OFF = BN2_OFF + 2          # [128, 8] instnorm gamma/beta per group
KV_OFF = NGB_OFF + 8           # [128, 36] dw k vectors (g*9+t)
WCOLS = KV_OFF + 36

# fp16 weight layout ([128, W16COLS])
DW16_OFF = 0                   # 36 taps x [128,128] diag (g*9+t)
MIX16_OFF = DW16_OFF + 36 * 128  # 4 kg x [128, 512] = w_eff.T blocks
W16COLS = MIX16_OFF + 4 * 512

XP_BUFS = 22

# engine split for the main-loop depthwise conv: per group, per tap-index
# 'P' = whole group on PE (psum); otherwise per-tap: 'V' = DVE mul+add pair,
# 'A' = ACT product + DVE add, 'G' = Pool fused scalar_tensor_tensor.
# Tap 0 (the full-coverage (1,1) tap) of a non-PE group always inits on DVE.
# PE groups do the depthwise as diagonal matmuls into PSUM; SBUF groups
# compute 9 full-row tap products (engines per PROD table) and combine them
# with a pairwise add tree (engines per ADD table, ops in fixed order:
# P0+=P1, P2+=P3, P4+=P5, P6+=P7, P0+=P2, P4+=P6, P0+=P4, d16=P0+P8).
PE_GROUPS = (0, 1)
DW_PROD = {
    2: ['V', 'A', 'A', 'A', 'A', 'A', 'A', 'A', 'V'],
    3: ['V', 'V', 'A', 'G', 'G', 'G', 'G', 'V', 'V'],
}
DW_ADD = {
    2: ['V', 'V', 'V', 'G', 'V', 'V', 'V', 'V'],
    3: ['V', 'V', 'V', 'G', 'V', 'V', 'G', 'V'],
}
ADD_TREE = [(0, 1), (2, 3), (4, 5), (6, 7), (0, 2), (4, 6), (0, 4)]

# tap order: full-coverage tap first (start=True zeroes the psum region)
TAPS = [(1, 1), (0, 0), (0, 1), (0, 2), (1, 0), (1, 2), (2, 0), (2, 1), (2, 2)]


def _clip(lo, hi, lo2, hi2):
    return max(lo, lo2), min(hi, hi2)


def build_nc():
    nc = bacc.Bacc(num_devices=N_CORES)
    x4 = nc.dram_tensor("x4", [BPC, 3, IMG, IMG], F32R, kind="ExternalInput").ap()
    wts = nc.dram_tensor("wts", [128, WCOLS], F32R, kind="ExternalInput").ap()
    wts16 = nc.dram_tensor("wts16", [128, W16COLS], F16,
                           kind="ExternalInput").ap()
    y4 = nc.dram_tensor("y4", [BPC, OUT], F32, kind="ExternalOutput").ap()

    with tile.TileContext(nc) as tc:
        with (
            tc.tile_pool(name="wp", bufs=1) as wp,
            tc.tile_pool(name="wp16", bufs=1) as wp16,
            tc.tile_pool(name="small", bufs=1) as small,
            tc.tile_pool(name="psA", bufs=2, space="PSUM") as psA,
            tc.tile_pool(name="psB", bufs=3, space="PSUM") as psB,
            tc.tile_pool(name="dram", bufs=1, space="DRAM") as dram,
        ):
            w_sb = wp.tile([128, WCOLS], F32R)
            w32 = w_sb.bitcast(F32)
            w16 = wp16.tile([128, W16COLS], F16)

            def load_weights(which):
                # emission points chosen so the serial DMA device serves
                # the downsample path first (input loads go even earlier)
                if which == 'head':
                    nc.sync.dma_start(out=w_sb[:, 0:FCW_OFF],
                                      in_=wts[:, 0:FCW_OFF])
                    nc.sync.dma_start(out=w_sb[:, FCB_OFF:WCOLS],
                                      in_=wts[:, FCB_OFF:WCOLS])
                    nc.sync.dma_start(out=w16, in_=wts16)
                    nc.sync.dma_start(out=w_sb[:, FCW_OFF:FCB_OFF],
                                      in_=wts[:, FCW_OFF:FCB_OFF])
                elif which == 'main':
                    pass
                else:
                    pass

            def wcols(off, n):
                return w_sb[:, off:off + n]

            def w16cols(off, n):
                return w16[:, off:off + n]

            def kvec(g, t):
                return w32[:, KV_OFF + g * 9 + t:KV_OFF + g * 9 + t + 1]

            eps_t = small.tile([128, 1], F32)
            nc.vector.memset(eps_t, EPS)
            z32 = small.tile([128, 64], F32)
            nc.vector.memset(z32, 0.0)
            z16 = small.tile([128, 64], F16)
            nc.vector.memset(z16, 0.0)

            # ---------------- downsample ----------------
            with tc.tile_pool(name="ds1", bufs=1) as ds1:
                # im2col9: partition p = 32*s + 3*dy + c ; free = (oy 64, ix' 130)
                # ix' = ix + 1 (x padded by 1 on both sides)
                im9 = ds1.tile([128, 64 * 130], F32R)
                im9r = im9.rearrange("p (y x) -> p y x", y=64, x=130)
                # zero the x pads (cols 0 and 129)
                for xc in (0, 129):
                    im9_pads = bass.AP(tensor=im9.tensor,
                                       offset=im9.offset + xc,
                                       ap=[im9.ap[0], [130, 64]])
                    nc.vector.tensor_copy(out=im9_pads, in_=z32[:, 0:64])
                # row oy=0 is out of range for dy=0 taps: zero it everywhere
                # first (dy=1/2 loads overwrite their row 0 afterwards; cols
                # 0/129 are the x-pads zeroed above)
                nc.vector.tensor_copy(out=im9[:, 1:65], in_=z32[:, 0:64])
                nc.vector.tensor_copy(out=im9[:, 65:129], in_=z32[:, 0:64])
                # x rows: iy = 2*oy + dy - 1
                # partition base: sample s -> 64*(s%2) + 27*(s//2)
                x4r = x4.rearrange("s c (y2 two) x -> s c y2 two x", two=2)
                for s in range(BPC):
                    for dy in range(3):
                        p0 = 64 * (s % 2) + 27 * (s // 2) + 3 * dy
                        if dy == 0:
                            # oy in [1,64): iy = 2*(oy-1)+1
                            nc.sync.dma_start(
                                out=im9r[p0:p0 + 3, 1:64, 1:129],
                                in_=x4r[s, :, 0:63, 1, :])
                        elif dy == 1:
                            nc.sync.dma_start(
                                out=im9r[p0:p0 + 3, :, 1:129],
                                in_=x4r[s, :, :, 0, :])
                        else:
                            nc.sync.dma_start(
                                out=im9r[p0:p0 + 3, :, 1:129],
                                in_=x4r[s, :, :, 1, :])

                load_weights('head')
                # conv1: out h1 [128 = 64*(s//2)+ch, (s%2, oy 64, ox 64)]
                h1 = ds1.tile([128, 8192], F32)
                h1r = h1.rearrange("p (sh y x) -> p sh y x", sh=2, y=64, x=64)
                # im2col x-read: ix' = 2*ox + dx (x2 = ox + dx//2, tx = dx%2)
                # paired matmul: K=54 block-diag covers samples (q, q+2):
                # out partitions 0-63 <- sample q, 64-127 <- sample q+2.
                im9x = im9.rearrange("p (y x2 two) -> p y x2 two", x2=65, two=2)
                for q in range(2):
                    for yb in range(4):           # 16-oy blocks
                        for h in range(2):
                            pc1 = psA.tile([128, 512], F32, tag="a",
                                           name="pc1")
                            pc1r = pc1.rearrange("p (y x) -> p y x", y=8, x=64)
                            oy0 = yb * 16 + h * 8
                            for di, dx in enumerate([1, 0, 2]):
                                rhs = im9x[64 * q:64 * q + 54, oy0:oy0 + 8,
                                           dx // 2:dx // 2 + 64, dx % 2]
                                lhsT = w_sb[64 * q:64 * q + 54,
                                            W1X_OFF + di_col(dx) * 128:
                                            W1X_OFF + di_col(dx) * 128 + 128]
                                nc.tensor.matmul(pc1r, lhsT, rhs,
                                                 start=(di == 0), stop=(di == 2),
                                                 tile_position=(64 * q, 0))
                            if (q + yb + h) % 2 == 0:
                                nc.scalar.copy(
                                    out=h1r[:, q, oy0:oy0 + 8, :], in_=pc1)
                            else:
                                nc.vector.tensor_copy(
                                    out=h1r[:, q, oy0:oy0 + 8, :], in_=pc1)

                # BN1 partial stats
                st1 = small.tile([128, 16, 6], F32)
                for i in range(16):
                    nc.vector.bn_stats(out=st1[:, i, :],
                                       in_=h1[:, i * 512:(i + 1) * 512])
                mv1 = small.tile([128, 2], F32)
                nc.vector.bn_aggr(out=mv1, in_=st1)
                sums1 = small.tile([128, 2], F32)
                tmp1 = small.tile([128, 1], F32)
                nc.vector.tensor_scalar_mul(out=sums1[:, 0:1], in0=mv1[:, 0:1],
                                            scalar1=8192.0)
                nc.vector.tensor_mul(out=tmp1, in0=mv1[:, 0:1], in1=mv1[:, 0:1])
                nc.vector.tensor_add(out=tmp1, in0=tmp1, in1=mv1[:, 1:2])
                nc.vector.tensor_scalar_mul(out=sums1[:, 1:2], in0=tmp1,
                                            scalar1=8192.0)
                # stage as [q, j, t]: channel q, partition-half j, stat t
                bn1_in = dram.tile([64, 2, 2], F32)
                bn1_out = dram.tile([8, 64, 2, 2], F32)
                nc.gpsimd.dma_start(
                    out=bn1_in.rearrange("q j t -> j q t"), in_=sums1)
                nc.gpsimd.collective_compute(
                    "AllGather", mybir.AluOpType.bypass,
                    replica_groups=[list(range(N_CORES))],
                    ins=[bn1_in.opt()], outs=[bn1_out.opt()])
                # readback: partition p gets (j = channel-half, rank) slots of
                # channel p%64; two parallel DMAs fill rows 0:64 and 64:128
                # with identical data, so the partition-halves sum (channel
                # stats) and the gamma scale land on all 128 rows directly.
                ga1 = small.tile([128, 8, 4], F32)
                src_ap = bn1_out.rearrange("r q j t -> q r (j t)")
                nc.gpsimd.dma_start(out=ga1[0:64], in_=src_ap)
                nc.gpsimd.dma_start(out=ga1[64:128], in_=src_ap)
                g4 = small.tile([128, 8, 2], F32)
                nc.vector.tensor_add(out=g4, in0=ga1[:, :, 0:2],
                                     in1=ga1[:, :, 2:4])
                g2t = small.tile([128, 4, 2], F32)
                nc.vector.tensor_add(out=g2t, in0=g4[:, 0:4, :],
                                     in1=g4[:, 4:8, :])
                g1t = small.tile([128, 2, 2], F32)
                nc.vector.tensor_add(out=g1t, in0=g2t[:, 0:2, :],
                                     in1=g2t[:, 2:4, :])
                red1 = small.tile([128, 2], F32)
                nc.vector.tensor_add(out=red1, in0=g1t[:, 0, :],
                                     in1=g1t[:, 1, :])

                s1t1 = small.tile([128, 2], F32)
                _bn_scale_bias(nc, s1t1, red1, w32, BN1_OFF, 131072.0,
                               eps_t, small, rows=128)

                # apply BN1 + relu -> h1n (f32r), x padded to 66 (ix' = ix+1)
                h1n = ds1.tile([128, 2 * 64 * 66], F32R)
                h1nr3 = h1n.rearrange("p (sh y x) -> p sh y x",
                                      sh=2, y=64, x=66)
                for sh in range(2):
                    for xc in (0, 65):
                        h1n_pads = bass.AP(tensor=h1n.tensor,
                                           offset=h1n.offset + 4224 * sh + xc,
                                           ap=[h1n.ap[0], [66, 64]])
                        nc.vector.tensor_copy(out=h1n_pads, in_=z32[:, 0:64])
                h1r4 = h1.rearrange("p (sh y x) -> p sh y x", sh=2, y=64, x=64)
                for sh in range(2):
                    for rh in range(2):
                        nc.scalar.activation(
                            out=h1nr3[:, sh, rh * 32:rh * 32 + 32, 1:65],
                            in_=h1r4[:, sh, rh * 32:rh * 32 + 32, :],
                            func=AF.Relu,
                            scale=s1t1[:, 0:1], bias=s1t1[:, 1:2])

                # conv2: depthwise 3x3 stride 2 -> d2 [128, (sh, 32, 32)]
                # row iy = 2*oy + dy - 1 (unpadded), col ix' = 2*ox + dx (padded)
                h1nr = h1n.rearrange(
                    "p (sh y2 ty x2 tx) -> p sh y2 ty x2 tx",
                    sh=2, y2=32, ty=2, x2=33, tx=2)
                d2 = ds1.tile([128, 2048], F32R)
                for sh in range(2):
                    pd2 = psB.tile([128, 1024], F32, tag="b", name="pd2")
                    pd2r = pd2.rearrange("p (h y x) -> p h y x", h=2, y=16, x=32)
                    for h in range(2):
                        for ti, (dy, dx) in enumerate(TAPS):
                            oy0, oy1 = _clip(h * 16, h * 16 + 16,
                                             1 if dy == 0 else 0, 32)
                            if dy == 1:
                                ys, par = oy0, 0
                            elif dy == 0:
                                ys, par = oy0 - 1, 1
                            else:
                                ys, par = oy0, 1
                            rhs = h1nr[:, sh, ys:ys + (oy1 - oy0), par,
                                       dx // 2:dx // 2 + 32, dx % 2]
                            outp = pd2r[:, h, oy0 - h * 16:oy1 - h * 16, :]
                            t = TAPS.index((dy, dx))
                            nc.tensor.matmul(
                                outp, wcols(W2D_OFF + t * 128, 128), rhs,
                                start=(ti == 0), stop=(ti == len(TAPS) - 1))
                    nc.scalar.copy(out=d2[:, sh * 1024:(sh + 1) * 1024], in_=pd2)

                load_weights('main')
                # conv3: 1x1, 64 -> 128 ; h3 [128=outc, (s, 1024px)]
                h3 = small.tile([128, 4096], F32)
                for a in range(2):
                    for nb in range(4):
                        pc3 = psA.tile([128, 512], F32, tag="a",
                                       name=f"pc3_{a}_{nb}")
                        nc.tensor.matmul(
                            pc3,
                            w_sb[64 * a:64 * a + 64, W3_OFF:W3_OFF + 128],
                            d2[64 * a:64 * a + 64, nb * 512:(nb + 1) * 512],
                            start=True, stop=True)
                        s_full = 2 * a + nb // 2
                        dst = h3[:, s_full * 1024 + (nb % 2) * 512:
                                 s_full * 1024 + (nb % 2) * 512 + 512]
                        if nb % 2 == 0:
                            nc.scalar.copy(out=dst, in_=pc3)
                        else:
                            nc.vector.tensor_copy(out=dst, in_=pc3)

                # BN2 stats + allreduce
                st2 = small.tile([128, 8, 6], F32)
                for i in range(8):
                    nc.vector.bn_stats(out=st2[:, i, :],
                                       in_=h3[:, i * 512:(i + 1) * 512])
                mv2 = small.tile([128, 2], F32)
                nc.vector.bn_aggr(out=mv2, in_=st2)
                sums2 = small.tile([128, 2], F32)
                tmp2 = small.tile([128, 1], F32)
                nc.vector.tensor_scalar_mul(out=sums2[:, 0:1], in0=mv2[:, 0:1],
                                            scalar1=4096.0)
                nc.vector.tensor_mul(out=tmp2, in0=mv2[:, 0:1], in1=mv2[:, 0:1])
                nc.vector.tensor_add(out=tmp2, in0=tmp2, in1=mv2[:, 1:2])
                nc.vector.tensor_scalar_mul(out=sums2[:, 1:2], in0=tmp2,
                                            scalar1=4096.0)
                bn2_in = dram.tile([256, 1], F32)
                bn2_out = dram.tile([8, 256, 1], F32)
                nc.gpsimd.dma_start(out=bn2_in, in_=sums2)
                nc.gpsimd.collective_compute(
                    "AllGather", mybir.AluOpType.bypass,
                    replica_groups=[list(range(N_CORES))],
                    ins=[bn2_in.opt()], outs=[bn2_out.opt()])
                gb1 = small.tile([128, 8, 2], F32)
                nc.gpsimd.dma_start(out=gb1, in_=bn2_out.rearrange(
                    "r (p t) one -> p r (t one)", p=128, t=2))
                h4s = small.tile([128, 4, 2], F32)
                nc.vector.tensor_add(out=h4s, in0=gb1[:, 0:4, :],
                                     in1=gb1[:, 4:8, :])
                h2s = small.tile([128, 2, 2], F32)
                nc.vector.tensor_add(out=h2s, in0=h4s[:, 0:2, :],
                                     in1=h4s[:, 2:4, :])
                red2 = small.tile([128, 2], F32)
                nc.vector.tensor_add(out=red2, in0=h2s[:, 0, :],
                                     in1=h2s[:, 1, :])
                s2t2 = small.tile([128, 2], F32)
                _bn_scale_bias(nc, s2t2, red2, w32, BN2_OFF, 32768.0,
                               eps_t, small, rows=128)

            # ---------------- main loop ----------------
            with (
                tc.tile_pool(name="xp", bufs=XP_BUFS) as xp,
                tc.tile_pool(name="dp", bufs=5) as dp,
                tc.tile_pool(name="pp", bufs=12) as pp,
                tc.tile_pool(name="stp", bufs=4) as stp,
            ):
                def new_x_tile(name):
                    # pad rows (-1, 32) and columns (0, 33) of every xp slot
                    # were zeroed once below; applies only write the interior
                    # (rows 1..32, cols 1..32 of the 34x34 grid).
                    return xp.tile([128, 34 * 34], F16, tag="X", name=name)

                # one-time zeroing of the pad columns of all X slots: the
                # dummies are simultaneously live (kept alive by the reads
                # below), so by pigeonhole they cover all slots.
                _dummies = []
                for i in range(XP_BUFS):
                    zt = xp.tile([128, 34 * 34], F16, tag="X", name=f"xz{i}")
                    # pad rows -1 and 32 (contiguous 34-elem spans)
                    nc.vector.tensor_copy(out=zt[:, 0:34], in_=z16[:, 0:34])
                    nc.vector.tensor_copy(out=zt[:, 1122:1156],
                                          in_=z16[:, 0:34])
                    for xc in (0, 33):
                        pads = bass.AP(tensor=zt.tensor, offset=zt.offset + xc,
                                       ap=[zt.ap[0], [34, 34]])
                        nc.vector.tensor_copy(out=pads, in_=z16[:, 0:34])
                    _dummies.append(zt)
                _pad_scratch = small.tile([128, 1], F16)
                for zt in _dummies:
                    nc.scalar.copy(out=_pad_scratch, in_=zt[:, 0:1])

                Xcur = {}
                for s in range(BPC):
                    xt = new_x_tile(f"X1_0_{s}")
                    xtr = xt.rearrange("p (y x) -> p y x", y=34, x=34)
                    h3r = h3.rearrange("p (s y x) -> p s y x", s=4, y=32, x=32)
                    nc.scalar.activation(out=xtr[:, 1:33, 1:33],
                                         in_=h3r[:, s, :, :],
                                         func=AF.Relu,
                                         scale=s2t2[:, 0:1], bias=s2t2[:, 1:2])
                    Xcur[(0, s)] = xt

                pooled_in = small.tile([128, 2, 4, 4], F32)

                for L in range(1, LAYERS + 1):
                    gs_in = sorted({g for (g, _s) in Xcur})
                    mgs = [2, 3] if L == LAYERS else [0, 1, 2, 3]
                    pe_gs = [g for g in gs_in
                             if g in PE_GROUPS or len(gs_in) == 1]
                    sb_gs = [g for g in gs_in if g not in pe_gs]
                    Xnext = {}
                    d16s = {}
                    for s in range(BPC):
                        d16s[s] = dp.tile([128, 4096], F16, tag="D",
                                          name=f"D{L}_{s}")

                    def xv(g, s, dy, dx):
                        Xr = Xcur[(g, s)].rearrange("p (y x) -> p y x",
                                                    y=34, x=34)
                        return Xr[:, dy:dy + 32, dx:dx + 32]

                    # phase B: SBUF depthwise via 9 full-row products and a
                    # pairwise add tree per (sample, group)
                    def emit_prod(eng, out, xin, g, ti):
                        if eng == 'A':
                            nc.scalar.activation(out=out, in_=xin,
                                                 func=AF.Identity,
                                                 scale=kvec(g, ti))
                        elif eng == 'G':
                            nc.gpsimd.tensor_scalar_mul(out=out, in0=xin,
                                                        scalar1=kvec(g, ti))
                        else:
                            nc.vector.tensor_scalar_mul(out=out, in0=xin,
                                                        scalar1=kvec(g, ti))

                    def emit_add(eng, out, in0, in1):
                        if eng == 'G':
                            nc.gpsimd.tensor_add(out=out, in0=in0, in1=in1)
                        else:
                            nc.vector.tensor_add(out=out, in0=in0, in1=in1)

                    for s in range(BPC):
                        prods = {}
                        for g in sb_gs:
                            for ti, (dy, dx) in enumerate(TAPS):
                                prod = pp.tile([128, 1024], F16, tag="P",
                                               name=f"P{L}_{s}_{g}_{ti}")
                                emit_prod(DW_PROD[g][ti], prod,
                                          xv(g, s, dy, dx), g, ti)
                                prods[(g, ti)] = prod
                        for g in sb_gs:
                            for ai, (d, e) in enumerate(ADD_TREE):
                                emit_add(DW_ADD[g][ai], prods[(g, d)],
                                         prods[(g, d)], prods[(g, e)])
                        for g in sb_gs:
                            emit_add(DW_ADD[g][7],
                                     d16s[s][:, g * 1024:g * 1024 + 1024],
                                     prods[(g, 0)], prods[(g, 8)])

                    # phase A: PE depthwise (PSUM) + ACT copies to fp16 D
                    for s in range(BPC):
                        for g in pe_gs:
                            for h in range(2):
                                pdw = psA.tile([128, 512], F32, tag="a",
                                               name=f"pdw{L}_{s}_{g}_{h}")
                                pdwr = pdw.rearrange("p (y x) -> p y x",
                                                     y=16, x=32)
                                for ti, (dy, dx) in enumerate(TAPS):
                                    rhs = xv(g, s, dy, dx)[:, h * 16:
                                                           h * 16 + 16, :]
                                    t = TAPS.index((dy, dx))
                                    nc.tensor.matmul(
                                        pdwr,
                                        w16cols(DW16_OFF + (g * 9 + t) * 128,
                                                128),
                                        rhs,
                                        start=(ti == 0),
                                        stop=(ti == len(TAPS) - 1))
                                nc.scalar.copy(
                                    out=d16s[s][:, g * 1024 + h * 512:
                                                g * 1024 + h * 512 + 512],
                                    in_=pdw)

                    # phase C: mix + instnorm + apply
                    for s in range(BPC):
                        d16 = d16s[s]
                        for mg in mgs:
                            pm = psB.tile([128, 1024], F32, tag="b",
                                          name=f"pm{L}_{s}_{mg}")
                            for h in range(2):
                                for ki, kg in enumerate(gs_in):
                                    nc.tensor.matmul(
                                        pm[:, h * 512:h * 512 + 512],
                                        w16cols(MIX16_OFF + kg * 512
                                                + mg * 128, 128),
                                        d16[:, kg * 1024 + h * 512:
                                            kg * 1024 + h * 512 + 512],
                                        start=(ki == 0),
                                        stop=(ki == len(gs_in) - 1))
                            st = stp.tile([128, 2, 6], F32, tag="st")
                            nc.vector.bn_stats(out=st[:, 0, :],
                                               in_=pm[:, 0:512])
                            nc.vector.bn_stats(out=st[:, 1, :],
                                               in_=pm[:, 512:1024])
                            mv = stp.tile([128, 2], F32, tag="mv")
                            nc.vector.bn_aggr(out=mv, in_=st)
                            sc = stp.tile([128, 1], F32, tag="sc")
                            tt = stp.tile([128, 1], F32, tag="tt")
                            nc.scalar.activation(out=sc, in_=mv[:, 1:2],
                                                 func=AF.Sqrt, bias=eps_t)
                            nc.vector.reciprocal(out=sc, in_=sc)
                            nc.vector.tensor_scalar_mul(
                                out=sc, in0=sc,
                                scalar1=w32[:, NGB_OFF + 2 * mg:
                                            NGB_OFF + 2 * mg + 1])
                            nc.vector.tensor_mul(out=tt, in0=mv[:, 0:1],
                                                 in1=sc)
                            nc.vector.tensor_scalar(
                                out=tt, in0=tt, scalar1=-1.0,
                                scalar2=w32[:, NGB_OFF + 2 * mg + 1:
                                            NGB_OFF + 2 * mg + 2],
                                op0=mybir.AluOpType.mult,
                                op1=mybir.AluOpType.add)
                            if L < LAYERS:
                                xt = new_x_tile(f"X{L + 1}_{mg}_{s}")
                                Xnext[(mg, s)] = xt
                                xtr = xt.rearrange("p (y x) -> p y x",
                                                   y=34, x=34)
                                pmr2 = pm.rearrange("p (y x) -> p y x",
                                                    y=32, x=32)
                                nc.scalar.activation(
                                    out=xtr[:, 1:33, 1:33], in_=pmr2,
                                    func=AF.Relu, scale=sc, bias=tt)
                            else:
                                pmr = pm.rearrange("p (y x) -> p y x",
                                                   y=32, x=32)
                                nc.scalar.activation(
                                    out=pooled_in[:, mg - 2, s, :],
                                    in_=pmr[:, HALF - 1:HALF + 1,
                                            HALF - 1:HALF + 1],
                                    func=AF.Identity, scale=sc, bias=tt)
                    Xcur = Xnext

                # ---------------- readout ----------------
                load_weights('fc')
                tadd = small.tile([128, 2, 4], F32)
                tadd2 = small.tile([128, 2, 4], F32)
                pooled = small.tile([128, 2, 4], F32R)
                nc.vector.tensor_add(out=tadd, in0=pooled_in[:, :, :, 0],
                                     in1=pooled_in[:, :, :, 1])
                nc.vector.tensor_add(out=tadd2, in0=pooled_in[:, :, :, 2],
                                     in1=pooled_in[:, :, :, 3])
                nc.vector.tensor_add(out=pooled, in0=tadd, in1=tadd2)
                y_sb = small.tile([128, 4, 8], F32)
                for mo in range(8):
                    mlen = 128 if mo < 7 else OUT - 7 * 128
                    pf = psA.tile([128, 512], F32, tag="a", name=f"pf{mo}")
                    for kgi in range(2):
                        nc.tensor.matmul(
                            pf[0:mlen, 0:4],
                            w_sb[:, FCW_OFF + kgi * 1000 + mo * 128:
                                 FCW_OFF + kgi * 1000 + mo * 128 + mlen],
                            pooled[:, kgi, :],
                            start=(kgi == 0), stop=(kgi == 1))
                    nc.scalar.activation(
                        out=y_sb[0:mlen, :, mo], in_=pf[0:mlen, 0:4],
                        func=AF.Identity,
                        bias=w32[0:mlen, FCB_OFF + mo:FCB_OFF + mo + 1],
                        scale=1.0)
                for s in range(BPC):
                    dst1 = bass.AP(tensor=y4.tensor, offset=OUT * s,
                                   ap=[[1, 128], [128, 7]])
                    nc.sync.dma_start(out=dst1, in_=y_sb[:, s, 0:7])
                    dst2 = bass.AP(tensor=y4.tensor, offset=OUT * s + 896,
                                   ap=[[1, 104]])
                    nc.sync.dma_start(out=dst2, in_=y_sb[0:104, s, 7])

    nc.finalize()
    return nc


def di_col(dx):
    # column index of conv1 tap dx within w1x block (emission order 1,0,2)
    return {1: 0, 0: 1, 2: 2}[dx]


def _bn_scale_bias(nc, out_st, sums, w32, gb_off, n_tot, eps_t, pool, rows):
    """out_st[:rows, 0] = gamma*rsqrt(var+eps); out_st[:rows, 1] = beta - mu*scale."""
    r = slice(0, rows)
    mu = pool.tile([128, 1], F32, name=f"mu{gb_off}")
    ex2 = pool.tile([128, 1], F32, name=f"ex2{gb_off}")
    var = pool.tile([128, 1], F32, name=f"var{gb_off}")
    nc.vector.tensor_scalar_mul(out=mu[r], in0=sums[r, 0:1], scalar1=1.0 / n_tot)
    nc.vector.tensor_scalar_mul(out=ex2[r], in0=sums[r, 1:2], scalar1=1.0 / n_tot)
    nc.vector.tensor_mul(out=var[r], in0=mu[r], in1=mu[r])
    nc.vector.tensor_sub(out=var[r], in0=ex2[r], in1=var[r])
    nc.scalar.activation(out=var[r], in_=var[r], func=AF.Sqrt, bias=eps_t[r])
    nc.vector.reciprocal(out=var[r], in_=var[r])
    nc.vector.tensor_scalar_mul(out=out_st[r, 0:1], in0=var[r],
                                scalar1=w32[r, gb_off:gb_off + 1])
    nc.vector.tensor_mul(out=mu[r], in0=mu[r], in1=out_st[r, 0:1])
    nc.vector.tensor_scalar(out=out_st[r, 1:2], in0=mu[r], scalar1=-1.0,
                            scalar2=w32[r, gb_off + 1:gb_off + 2],
                            op0=mybir.AluOpType.mult,
                            op1=mybir.AluOpType.add)


def _pack_weights(ds_w1, ds_w2, ds_w3, conv_w, graph_w, fc_w, fc_b,
                  bn1_g, bn1_b, bn2_g, bn2_b, norm_g, norm_b):
    wts = np.zeros((128, WCOLS), np.float32)
    w16 = np.zeros((128, W16COLS), np.float16)
    # pruned graph weight
    k = int((1.0 - PRUNE) * DIM * DIM)
    a = np.abs(graph_w).ravel()
    thresh = np.partition(a, -k)[-k]
    w_eff = np.where(np.abs(graph_w) >= thresh, graph_w, 0.0).astype(np.float32)
    # conv1 taps, paired block-diag:
    # rows 64*q + 27*a + 3*dy + c, cols 64*a + o = w1[o, c, dy, dx]
    for dx in range(3):
        dc = di_col(dx)
        blk = np.zeros((128, 128), np.float32)
        for qq in range(2):
            for aa in range(2):
                for dy in range(3):
                    for c in range(3):
                        blk[64 * qq + 27 * aa + 3 * dy + c,
                            64 * aa:64 * aa + 64] = ds_w1[:, c, dy, dx]
        wts[:, W1X_OFF + dc * 128:W1X_OFF + (dc + 1) * 128] = blk
    # conv2 diag-dup taps
    for t, (dy, dx) in enumerate(TAPS):
        blk = np.zeros((128, 128), np.float32)
        d = ds_w2[:, 0, dy, dx]
        for aa in range(2):
            idx = np.arange(64)
            blk[64 * aa + idx, 64 * aa + idx] = d
        wts[:, W2D_OFF + t * 128:W2D_OFF + (t + 1) * 128] = blk
    # conv3: [64a + c, o] = w3[o, c]
    w3 = ds_w3[:, :, 0, 0]  # [128, 64]
    wts[0:64, W3_OFF:W3_OFF + 128] = w3.T
    wts[64:128, W3_OFF:W3_OFF + 128] = w3.T
    # main dw diag taps (fp16)
    for g in range(4):
        for t, (dy, dx) in enumerate(TAPS):
            blk = np.zeros((128, 128), np.float16)
            idx = np.arange(128)
            blk[idx, idx] = conv_w[g * 128:(g + 1) * 128, 0, dy, dx]
            off = DW16_OFF + (g * 9 + t) * 128
            w16[:, off:off + 128] = blk
    # dw k vectors for the DVE path (f32)
    for g in range(4):
        for t, (dy, dx) in enumerate(TAPS):
            wts[:, KV_OFF + g * 9 + t] = conv_w[g * 128:(g + 1) * 128, 0, dy, dx]
    # mix (fp16): [p, kg*512 + mg*128 + j] = w_eff[mg*128 + j, kg*128 + p]
    weT = w_eff.T  # [in, out]
    for kg in range(4):
        w16[:, MIX16_OFF + kg * 512:MIX16_OFF + (kg + 1) * 512] = \
            weT[kg * 128:(kg + 1) * 128, :].astype(np.float16)
    # fc: [p, kg*1000 + m] = 0.25 * fc_w[m, kg*128 + p]
    for kg in range(2):
        wts[:, FCW_OFF + kg * 1000:FCW_OFF + (kg + 1) * 1000] = \
            0.25 * fc_w[:, kg * 128:(kg + 1) * 128].T
    # fc bias [p, mo]
    fcb = np.zeros((128, 8), np.float32)
    fb = np.zeros(1024, np.float32)
    fb[:OUT] = fc_b
    fcb[:, :] = fb.reshape(8, 128).T
    wts[:, FCB_OFF:FCB_OFF + 8] = fcb
    # bn gammas/betas
    wts[0:64, BN1_OFF] = bn1_g
    wts[64:128, BN1_OFF] = bn1_g
    wts[0:64, BN1_OFF + 1] = bn1_b
    wts[64:128, BN1_OFF + 1] = bn1_b
    wts[:, BN2_OFF] = bn2_g
    wts[:, BN2_OFF + 1] = bn2_b
    for g in range(4):
        wts[:, NGB_OFF + 2 * g] = norm_g[g * 128:(g + 1) * 128]
        wts[:, NGB_OFF + 2 * g + 1] = norm_b[g * 128:(g + 1) * 128]
    return wts, w16


_nc_cache = None
last_results = None


def kernel(**inputs):
    global _nc_cache, last_results
    inputs = {k: np.asarray(v, np.float32) for k, v in inputs.items()}
    wts, w16 = _pack_weights(
        inputs["ds_w1"], inputs["ds_w2"], inputs["ds_w3"], inputs["conv_w"],
        inputs["graph_w"], inputs["fc_w"], inputs["fc_b"],
        inputs["bn1_g"], inputs["bn1_b"], inputs["bn2_g"], inputs["bn2_b"],
        inputs["norm_g"], inputs["norm_b"])
    x = inputs["x"]
    if _nc_cache is None:
        _nc_cache = build_nc()
    nc = _nc_cache
    in_maps = [{"x4": np.ascontiguousarray(x[c * BPC:(c + 1) * BPC]),
                "wts": wts, "wts16": w16} for c in range(N_CORES)]
    res = run_bass_kernel_spmd(nc, in_maps, core_ids=list(range(N_CORES)))
    last_results = res
    return np.concatenate([res.results[c]["y4"] for c in range(N_CORES)], axis=0)


# revision 31
# speedup vs baseline: 1.6660x; 1.0015x over previous
"""Trainium2 Bass kernel for nn_DiscreteTimeNeuralGraph.

Strategy (8 NeuronCores, batch-parallel, engine-balanced):
  - Shard the batch of 32 across 8 cores (4 samples each); weights replicated.
  - Downsample path on-device; BatchNorm batch stats via per-core partial
    sums + one tiny AllReduce each.
  - Main loop in fp16 storage (X, D, weights; fp32 PSUM accumulation):
    depthwise 3x3 conv groups 0-2 as rect-clipped diagonal matmuls on PE;
    group 3 computed on the Vector engine as tensor_scalar(mul, 4x mode) +
    tensor_tensor(add, 2x mode) chains writing fp16 SBUF directly.
    PSUM->SBUF depthwise results copied (and cast to fp16) on the Pool
    engine, freeing ACT for the instnorm applies.
    Channel mix as fp16 blocked matmuls; instance-norm stats on VectorE;
    instnorm+ReLU fused into one ScalarE activation producing fp16 X.
  - Pad-column zeroing via engine memsets (not DMA).
  - Readout: center 2x2 mean (folded into fc weights) + fc matmul (f32r).

Top-k threshold for the pruned graph weight is computed on host
(np.partition) -- it is weight preprocessing of a replicated input.
"""
import numpy as np

import concourse.bass as bass
import concourse.tile as tile
from concourse import bacc, mybir
from concourse.bass_utils import run_bass_kernel_spmd

F32 = mybir.dt.float32
F32R = mybir.dt.float32r
F16 = mybir.dt.float16
AF = mybir.ActivationFunctionType
ALU = mybir.AluOpType

N_CORES = 8
B = 32
BPC = B // N_CORES          # 4 samples per core
DIM = 512
DS = 128
FEAT = 256
LAYERS = 8
IMG = 128
OUT = 1000
EPS = 1e-5
HALF = IMG // 4 // 2 - 1    # 15
PRUNE = 0.9

# f32 mega-weight column layout ([128, WCOLS])
W1X_OFF = 0                  # 3 dx-taps x [128,128] for conv1
W2D_OFF = W1X_OFF + 3 * 128  # 9 taps x [128,128] diag-dup for conv2
W3_OFF = W2D_OFF + 9 * 128   # [128,128] conv3 (w3 stacked twice on K)
FCW_OFF = W3_OFF + 128       # 2 kg x [128, 1000] fc lhsT (x0.25 pooled)
FCB_OFF = FCW_OFF + 2 * 1000  # [128, 8] fc bias chunks
BN1_OFF = FCB_OFF + 8          # [128, 2] bn1 gamma/beta (dup across halves)
BN2_OFF = BN1_OFF + 2          # [128, 2]
NGB_OFF = BN2_OFF + 2          # [128, 8] instnorm gamma/beta per group
KV_OFF = NGB_OFF + 8           # [128, 36] dw k vectors (g*9+t)
WCOLS = KV_OFF + 36

# fp16 weight layout ([128, W16COLS])
DW16_OFF = 0                   # 36 taps x [128,128] diag (g*9+t)
MIX16_OFF = DW16_OFF + 36 * 128  # 4 kg x [128, 512] = w_eff.T blocks
W16COLS = MIX16_OFF + 4 * 512

XP_BUFS = 22

# engine split for the main-loop depthwise conv: per group, per tap-index
# 'P' = whole group on PE (psum); otherwise per-tap: 'V' = DVE mul+add pair,
# 'A' = ACT product + DVE add, 'G' = Pool fused scalar_tensor_tensor.
# Tap 0 (the full-coverage (1,1) tap) of a non-PE group always inits on DVE.
# PE groups do the depthwise as diagonal matmuls into PSUM; SBUF groups
# compute 9 full-row tap products (engines per PROD table) and combine them
# with a pairwise add tree (engines per ADD table, ops in fixed order:
# P0+=P1, P2+=P3, P4+=P5, P6+=P7, P0+=P2, P4+=P6, P0+=P4, d16=P0+P8).
PE_GROUPS = (0, 1)
DW_PROD = {
    2: ['V', 'A', 'A', 'A', 'A', 'A', 'A', 'A', 'V'],
    3: ['V', 'V', 'A', 'G', 'G', 'G', 'G', 'V', 'V'],
}
DW_ADD = {
    2: ['V', 'V', 'V', 'G', 'V', 'V', 'V', 'V'],
    3: ['V', 'V', 'V', 'G', 'V', 'V', 'G', 'V'],
}
ADD_TREE = [(0, 1), (2, 3), (4, 5), (6, 7), (0, 2), (4, 6), (0, 4)]

# tap order: full-coverage tap first (start=True zeroes the psum region)
TAPS = [(1, 1), (0, 0), (0, 1), (0, 2), (1, 0), (1, 2), (2, 0), (2, 1), (2, 2)]


def _clip(lo, hi, lo2, hi2):
    return max(lo, lo2), min(hi, hi2)


def build_nc():
    nc = bacc.Bacc(num_devices=N_CORES)
    x4 = nc.dram_tensor("x4", [BPC, 3, IMG, IMG], F32R, kind="ExternalInput").ap()
    wts = nc.dram_tensor("wts", [128, WCOLS], F32R, kind="ExternalInput").ap()
    wts16 = nc.dram_tensor("wts16", [128, W16COLS], F16,
                           kind="ExternalInput").ap()
    y4 = nc.dram_tensor("y4", [BPC, OUT], F32, kind="ExternalOutput").ap()

    with tile.TileContext(nc) as tc:
        with (
            tc.tile_pool(name="wp", bufs=1) as wp,
            tc.tile_pool(name="wp16", bufs=1) as wp16,
            tc.tile_pool(name="small", bufs=1) as small,
            tc.tile_pool(name="psA", bufs=2, space="PSUM") as psA,
            tc.tile_pool(name="psB", bufs=3, space="PSUM") as psB,
            tc.tile_pool(name="dram", bufs=1, space="DRAM") as dram,
        ):
            w_sb = wp.tile([128, WCOLS], F32R)
            w32 = w_sb.bitcast(F32)
            w16 = wp16.tile([128, W16COLS], F16)

            def load_weights(which):
                # emission points chosen so the serial DMA device serves
                # the downsample path first (input loads go even earlier)
                if which == 'head':
                    nc.sync.dma_start(out=w_sb[:, 0:FCW_OFF],
                                      in_=wts[:, 0:FCW_OFF])
                    nc.sync.dma_start(out=w_sb[:, FCB_OFF:WCOLS],
                                      in_=wts[:, FCB_OFF:WCOLS])
                    nc.sync.dma_start(out=w16, in_=wts16)
                    nc.sync.dma_start(out=w_sb[:, FCW_OFF:FCB_OFF],
                                      in_=wts[:, FCW_OFF:FCB_OFF])
                elif which == 'main':
                    pass
                else:
                    pass

            def wcols(off, n):
                return w_sb[:, off:off + n]

            def w16cols(off, n):
                return w16[:, off:off + n]

            def kvec(g, t):
                return w32[:, KV_OFF + g * 9 + t:KV_OFF + g * 9 + t + 1]

            eps_t = small.tile([128, 1], F32)
            nc.vector.memset(eps_t, EPS)
            z32 = small.tile([128, 64], F32)
            nc.vector.memset(z32, 0.0)
            z16 = small.tile([128, 64], F16)
            nc.vector.memset(z16, 0.0)

            # ---------------- downsample ----------------
            with tc.tile_pool(name="ds1", bufs=1) as ds1:
                # im2col9: partition p = 32*s + 3*dy + c ; free = (oy 64, ix' 130)
                # ix' = ix + 1 (x padded by 1 on both sides)
                im9 = ds1.tile([128, 64 * 130], F32R)
                im9r = im9.rearrange("p (y x) -> p y x", y=64, x=130)
                # zero the x pads (cols 0 and 129)
                for xc in (0, 129):
                    im9_pads = bass.AP(tensor=im9.tensor,
                                       offset=im9.offset + xc,
                                       ap=[im9.ap[0], [130, 64]])
                    nc.vector.tensor_copy(out=im9_pads, in_=z32[:, 0:64])
                # row oy=0 is out of range for dy=0 taps: zero it everywhere
                # first (dy=1/2 loads overwrite their row 0 afterwards; cols
                # 0/129 are the x-pads zeroed above)
                nc.vector.tensor_copy(out=im9[:, 1:65], in_=z32[:, 0:64])
                nc.vector.tensor_copy(out=im9[:, 65:129], in_=z32[:, 0:64])
                # x rows: iy = 2*oy + dy - 1
                # partition base: sample s -> 64*(s%2) + 27*(s//2)
                x4r = x4.rearrange("s c (y2 two) x -> s c y2 two x", two=2)
                for si, s in enumerate((0, 2, 1, 3)):
                    if si == 2:
                        load_weights('head')
                    for dy in range(3):
                        p0 = 64 * (s % 2) + 27 * (s // 2) + 3 * dy
                        if dy == 0:
                            # oy in [1,64): iy = 2*(oy-1)+1
                            nc.sync.dma_start(
                                out=im9r[p0:p0 + 3, 1:64, 1:129],
                                in_=x4r[s, :, 0:63, 1, :])
                        elif dy == 1:
                            nc.sync.dma_start(
                                out=im9r[p0:p0 + 3, :, 1:129],
                                in_=x4r[s, :, :, 0, :])
                        else:
                            nc.sync.dma_start(
                                out=im9r[p0:p0 + 3, :, 1:129],
                                in_=x4r[s, :, :, 1, :])

                # conv1: out h1 [128 = 64*(s//2)+ch, (s%2, oy 64, ox 64)]
                h1 = ds1.tile([128, 8192], F32)
                h1r = h1.rearrange("p (sh y x) -> p sh y x", sh=2, y=64, x=64)
                # im2col x-read: ix' = 2*ox + dx (x2 = ox + dx//2, tx = dx%2)
                # paired matmul: K=54 block-diag covers samples (q, q+2):
                # out partitions 0-63 <- sample q, 64-127 <- sample q+2.
                im9x = im9.rearrange("p (y x2 two) -> p y x2 two", x2=65, two=2)
                st1 = small.tile([128, 16, 6], F32)
                for q in range(2):
                    for yb in range(4):           # 16-oy blocks
                        for h in range(2):
                            pc1 = psA.tile([128, 512], F32, tag="a",
                                           name="pc1")
                            pc1r = pc1.rearrange("p (y x) -> p y x", y=8, x=64)
                            oy0 = yb * 16 + h * 8
                            for di, dx in enumerate([1, 0, 2]):
                                rhs = im9x[64 * q:64 * q + 54, oy0:oy0 + 8,
                                           dx // 2:dx // 2 + 64, dx % 2]
                                lhsT = w_sb[64 * q:64 * q + 54,
                                            W1X_OFF + di_col(dx) * 128:
                                            W1X_OFF + di_col(dx) * 128 + 128]
                                nc.tensor.matmul(pc1r, lhsT, rhs,
                                                 start=(di == 0), stop=(di == 2),
                                                 tile_position=(64 * q, 0))
                            if (q + yb + h) % 2 == 0:
                                nc.scalar.copy(
                                    out=h1r[:, q, oy0:oy0 + 8, :], in_=pc1)
                            else:
                                nc.vector.tensor_copy(
                                    out=h1r[:, q, oy0:oy0 + 8, :], in_=pc1)
                            # BN1 partial stats, interleaved per chunk
                            ci = q * 8 + yb * 2 + h
                            nc.vector.bn_stats(
                                out=st1[:, ci, :],
                                in_=h1[:, ci * 512:(ci + 1) * 512])
                mv1 = small.tile([128, 2], F32)
                nc.vector.bn_aggr(out=mv1, in_=st1)
                sums1 = small.tile([128, 2], F32)
                tmp1 = small.tile([128, 1], F32)
                nc.vector.tensor_scalar_mul(out=sums1[:, 0:1], in0=mv1[:, 0:1],
                                            scalar1=8192.0)
                nc.vector.tensor_mul(out=tmp1, in0=mv1[:, 0:1], in1=mv1[:, 0:1])
                nc.vector.tensor_add(out=tmp1, in0=tmp1, in1=mv1[:, 1:2])
                nc.vector.tensor_scalar_mul(out=sums1[:, 1:2], in0=tmp1,
                                            scalar1=8192.0)
                # stage as [q, j, t]: channel q, partition-half j, stat t
                bn1_in = dram.tile([64, 2, 2], F32)
                bn1_out = dram.tile([8, 64, 2, 2], F32)
                nc.gpsimd.dma_start(
                    out=bn1_in.rearrange("q j t -> j q t"), in_=sums1)
                nc.gpsimd.collective_compute(
                    "AllGather", mybir.AluOpType.bypass,
                    replica_groups=[list(range(N_CORES))],
                    ins=[bn1_in.opt()], outs=[bn1_out.opt()])
                # readback: partition p gets (j = channel-half, rank) slots of
                # channel p%64; two parallel DMAs fill rows 0:64 and 64:128
                # with identical data, so the partition-halves sum (channel
                # stats) and the gamma scale land on all 128 rows directly.
                ga1 = small.tile([128, 8, 4], F32)
                src_ap = bn1_out.rearrange("r q j t -> q r (j t)")
                nc.gpsimd.dma_start(out=ga1[0:64], in_=src_ap)
                nc.gpsimd.dma_start(out=ga1[64:128], in_=src_ap)
                g4 = small.tile([128, 8, 2], F32)
                nc.vector.tensor_add(out=g4, in0=ga1[:, :, 0:2],
                                     in1=ga1[:, :, 2:4])
                g2t = small.tile([128, 4, 2], F32)
                nc.vector.tensor_add(out=g2t, in0=g4[:, 0:4, :],
                                     in1=g4[:, 4:8, :])
                g1t = small.tile([128, 2, 2], F32)
                nc.vector.tensor_add(out=g1t, in0=g2t[:, 0:2, :],
                                     in1=g2t[:, 2:4, :])
                red1 = small.tile([128, 2], F32)
                nc.vector.tensor_add(out=red1, in0=g1t[:, 0, :],
                                     in1=g1t[:, 1, :])

                s1t1 = small.tile([128, 2], F32)
                _bn_scale_bias(nc, s1t1, red1, w32, BN1_OFF, 131072.0,
                               eps_t, small, rows=128)

                # apply BN1 + relu -> h1n (f32r), x padded to 66 (ix' = ix+1)
                h1n = ds1.tile([128, 2 * 64 * 66], F32R)
                h1nr3 = h1n.rearrange("p (sh y x) -> p sh y x",
                                      sh=2, y=64, x=66)
                for sh in range(2):
                    for xc in (0, 65):
                        h1n_pads = bass.AP(tensor=h1n.tensor,
                                           offset=h1n.offset + 4224 * sh + xc,
                                           ap=[h1n.ap[0], [66, 64]])
                        nc.vector.tensor_copy(out=h1n_pads, in_=z32[:, 0:64])
                h1r4 = h1.rearrange("p (sh y x) -> p sh y x", sh=2, y=64, x=64)
                for sh in range(2):
                    for rh in range(2):
                        nc.scalar.activation(
                            out=h1nr3[:, sh, rh * 32:rh * 32 + 32, 1:65],
                            in_=h1r4[:, sh, rh * 32:rh * 32 + 32, :],
                            func=AF.Relu,
                            scale=s1t1[:, 0:1], bias=s1t1[:, 1:2])

                # conv2: depthwise 3x3 stride 2 -> d2 [128, (sh, 32, 32)]
                # row iy = 2*oy + dy - 1 (unpadded), col ix' = 2*ox + dx (padded)
                h1nr = h1n.rearrange(
                    "p (sh y2 ty x2 tx) -> p sh y2 ty x2 tx",
                    sh=2, y2=32, ty=2, x2=33, tx=2)
                d2 = ds1.tile([128, 2048], F32R)
                for sh in range(2):
                    pd2 = psB.tile([128, 1024], F32, tag="b", name="pd2")
                    pd2r = pd2.rearrange("p (h y x) -> p h y x", h=2, y=16, x=32)
                    for h in range(2):
                        for ti, (dy, dx) in enumerate(TAPS):
                            oy0, oy1 = _clip(h * 16, h * 16 + 16,
                                             1 if dy == 0 else 0, 32)
                            if dy == 1:
                                ys, par = oy0, 0
                            elif dy == 0:
                                ys, par = oy0 - 1, 1
                            else:
                                ys, par = oy0, 1
                            rhs = h1nr[:, sh, ys:ys + (oy1 - oy0), par,
                                       dx // 2:dx // 2 + 32, dx % 2]
                            outp = pd2r[:, h, oy0 - h * 16:oy1 - h * 16, :]
                            t = TAPS.index((dy, dx))
                            nc.tensor.matmul(
                                outp, wcols(W2D_OFF + t * 128, 128), rhs,
                                start=(ti == 0), stop=(ti == len(TAPS) - 1))
                    nc.scalar.copy(out=d2[:, sh * 1024:(sh + 1) * 1024], in_=pd2)

                load_weights('main')
                # conv3: 1x1, 64 -> 128 ; h3 [128=outc, (s, 1024px)]
                h3 = small.tile([128, 4096], F32)
                st2 = small.tile([128, 8, 6], F32)
                for a in range(2):
                    for nb in range(4):
                        pc3 = psA.tile([128, 512], F32, tag="a",
                                       name=f"pc3_{a}_{nb}")
                        nc.tensor.matmul(
                            pc3,
                            w_sb[64 * a:64 * a + 64, W3_OFF:W3_OFF + 128],
                            d2[64 * a:64 * a + 64, nb * 512:(nb + 1) * 512],
                            start=True, stop=True)
                        s_full = 2 * a + nb // 2
                        dst = h3[:, s_full * 1024 + (nb % 2) * 512:
                                 s_full * 1024 + (nb % 2) * 512 + 512]
                        if nb % 2 == 0:
                            nc.scalar.copy(out=dst, in_=pc3)
                        else:
                            nc.vector.tensor_copy(out=dst, in_=pc3)
                        ci2 = a * 4 + nb
                        nc.vector.bn_stats(
                            out=st2[:, ci2, :],
                            in_=h3[:, ci2 * 512:(ci2 + 1) * 512])
                mv2 = small.tile([128, 2], F32)
                nc.vector.bn_aggr(out=mv2, in_=st2)
                sums2 = small.tile([128, 2], F32)
                tmp2 = small.tile([128, 1], F32)
                nc.vector.tensor_scalar_mul(out=sums2[:, 0:1], in0=mv2[:, 0:1],
                                            scalar1=4096.0)
                nc.vector.tensor_mul(out=tmp2, in0=mv2[:, 0:1], in1=mv2[:, 0:1])
                nc.vector.tensor_add(out=tmp2, in0=tmp2, in1=mv2[:, 1:2])
                nc.vector.tensor_scalar_mul(out=sums2[:, 1:2], in0=tmp2,
                                            scalar1=4096.0)
                bn2_in = dram.tile([256, 1], F32)
                bn2_out = dram.tile([8, 256, 1], F32)
                nc.gpsimd.dma_start(out=bn2_in, in_=sums2)
                nc.gpsimd.collective_compute(
                    "AllGather", mybir.AluOpType.bypass,
                    replica_groups=[list(range(N_CORES))],
                    ins=[bn2_in.opt()], outs=[bn2_out.opt()])
                gb1 = small.tile([128, 8, 2], F32)
                nc.gpsimd.dma_start(out=gb1, in_=bn2_out.rearrange(
                    "r (p t) one -> p r (t one)", p=128, t=2))
                h4s = small.tile([128, 4, 2], F32)
                nc.vector.tensor_add(out=h4s, in0=gb1[:, 0:4, :],
                                     in1=gb1[:, 4:8, :])
                h2s = small.tile([128, 2, 2], F32)
                nc.vector.tensor_add(out=h2s, in0=h4s[:, 0:2, :],
                                     in1=h4s[:, 2:4, :])
                red2 = small.tile([128, 2], F32)
                nc.vector.tensor_add(out=red2, in0=h2s[:, 0, :],
                                     in1=h2s[:, 1, :])
                s2t2 = small.tile([128, 2], F32)
                _bn_scale_bias(nc, s2t2, red2, w32, BN2_OFF, 32768.0,
                               eps_t, small, rows=128)

            # ---------------- main loop ----------------
            with (
                tc.tile_pool(name="xp", bufs=XP_BUFS) as xp,
                tc.tile_pool(name="dp", bufs=5) as dp,
                tc.tile_pool(name="pp", bufs=12) as pp,
                tc.tile_pool(name="stp", bufs=4) as stp,
            ):
                def new_x_tile(name):
                    # pad rows (-1, 32) and columns (0, 33) of every xp slot
                    # were zeroed once below; applies only write the interior
                    # (rows 1..32, cols 1..32 of the 34x34 grid).
                    return xp.tile([128, 34 * 34], F16, tag="X", name=name)

                # one-time zeroing of the pad columns of all X slots: the
                # dummies are simultaneously live (kept alive by the reads
                # below), so by pigeonhole they cover all slots.
                _dummies = []
                for i in range(XP_BUFS):
                    zt = xp.tile([128, 34 * 34], F16, tag="X", name=f"xz{i}")
                    # pad rows -1 and 32 (contiguous 34-elem spans)
                    nc.vector.tensor_copy(out=zt[:, 0:34], in_=z16[:, 0:34])
                    nc.vector.tensor_copy(out=zt[:, 1122:1156],
                                          in_=z16[:, 0:34])
                    for xc in (0, 33):
                        pads = bass.AP(tensor=zt.tensor, offset=zt.offset + xc,
                                       ap=[zt.ap[0], [34, 34]])
                        nc.vector.tensor_copy(out=pads, in_=z16[:, 0:34])
                    _dummies.append(zt)
                _pad_scratch = small.tile([128, 1], F16)
                for zt in _dummies:
                    nc.scalar.copy(out=_pad_scratch, in_=zt[:, 0:1])

                Xcur = {}
                for s in range(BPC):
                    xt = new_x_tile(f"X1_0_{s}")
                    xtr = xt.rearrange("p (y x) -> p y x", y=34, x=34)
                    h3r = h3.rearrange("p (s y x) -> p s y x", s=4, y=32, x=32)
                    nc.scalar.activation(out=xtr[:, 1:33, 1:33],
                                         in_=h3r[:, s, :, :],
                                         func=AF.Relu,
                                         scale=s2t2[:, 0:1], bias=s2t2[:, 1:2])
                    Xcur[(0, s)] = xt

                pooled_in = small.tile([128, 2, 4, 4], F32)

                for L in range(1, LAYERS + 1):
                    gs_in = sorted({g for (g, _s) in Xcur})
                    mgs = [2, 3] if L == LAYERS else [0, 1, 2, 3]
                    pe_gs = [g for g in gs_in
                             if g in PE_GROUPS or len(gs_in) == 1]
                    sb_gs = [g for g in gs_in if g not in pe_gs]
                    Xnext = {}
                    d16s = {}
                    for s in range(BPC):
                        d16s[s] = dp.tile([128, 4096], F16, tag="D",
                                          name=f"D{L}_{s}")

                    def xv(g, s, dy, dx):
                        Xr = Xcur[(g, s)].rearrange("p (y x) -> p y x",
                                                    y=34, x=34)
                        return Xr[:, dy:dy + 32, dx:dx + 32]

                    # phase B: SBUF depthwise via 9 full-row products and a
                    # pairwise add tree per (sample, group)
                    def emit_prod(eng, out, xin, g, ti):
                        if eng == 'A':
                            nc.scalar.activation(out=out, in_=xin,
                                                 func=AF.Identity,
                                                 scale=kvec(g, ti))
                        elif eng == 'G':
                            nc.gpsimd.tensor_scalar_mul(out=out, in0=xin,
                                                        scalar1=kvec(g, ti))
                        else:
                            nc.vector.tensor_scalar_mul(out=out, in0=xin,
                                                        scalar1=kvec(g, ti))

                    def emit_add(eng, out, in0, in1):
                        if eng == 'G':
                            nc.gpsimd.tensor_add(out=out, in0=in0, in1=in1)
                        else:
                            nc.vector.tensor_add(out=out, in0=in0, in1=in1)

                    for s in range(BPC):
                        prods = {}
                        for g in sb_gs:
                            for ti, (dy, dx) in enumerate(TAPS):
                                prod = pp.tile([128, 1024], F16, tag="P",
                                               name=f"P{L}_{s}_{g}_{ti}")
                                emit_prod(DW_PROD[g][ti], prod,
                                          xv(g, s, dy, dx), g, ti)
                                prods[(g, ti)] = prod
                        for g in sb_gs:
                            for ai, (d, e) in enumerate(ADD_TREE):
                                emit_add(DW_ADD[g][ai], prods[(g, d)],
                                         prods[(g, d)], prods[(g, e)])
                        for g in sb_gs:
                            emit_add(DW_ADD[g][7],
                                     d16s[s][:, g * 1024:g * 1024 + 1024],
                                     prods[(g, 0)], prods[(g, 8)])

                    # phase A: PE depthwise (PSUM) + ACT copies to fp16 D
                    for s in range(BPC):
                        for g in pe_gs:
                            for h in range(2):
                                pdw = psA.tile([128, 512], F32, tag="a",
                                               name=f"pdw{L}_{s}_{g}_{h}")
                                pdwr = pdw.rearrange("p (y x) -> p y x",
                                                     y=16, x=32)
                                for ti, (dy, dx) in enumerate(TAPS):
                                    rhs = xv(g, s, dy, dx)[:, h * 16:
                                                           h * 16 + 16, :]
                                    t = TAPS.index((dy, dx))
                                    nc.tensor.matmul(
                                        pdwr,
                                        w16cols(DW16_OFF + (g * 9 + t) * 128,
                                                128),
                                        rhs,
                                        start=(ti == 0),
                                        stop=(ti == len(TAPS) - 1))
                                nc.scalar.copy(
                                    out=d16s[s][:, g * 1024 + h * 512:
                                                g * 1024 + h * 512 + 512],
                                    in_=pdw)

                    # phase C: mix + instnorm + apply
                    for s in range(BPC):
                        d16 = d16s[s]
                        for mg in mgs:
                            pm = psB.tile([128, 1024], F32, tag="b",
                                          name=f"pm{L}_{s}_{mg}")
                            for h in range(2):
                                for ki, kg in enumerate(gs_in):
                                    nc.tensor.matmul(
                                        pm[:, h * 512:h * 512 + 512],
                                        w16cols(MIX16_OFF + kg * 512
                                                + mg * 128, 128),
                                        d16[:, kg * 1024 + h * 512:
                                            kg * 1024 + h * 512 + 512],
                                        start=(ki == 0),
                                        stop=(ki == len(gs_in) - 1))
                            st = stp.tile([128, 2, 6], F32, tag="st")
                            nc.vector.bn_stats(out=st[:, 0, :],
                                               in_=pm[:, 0:512])
                            nc.vector.bn_stats(out=st[:, 1, :],
                                               in_=pm[:, 512:1024])
                            mv = stp.tile([128, 2], F32, tag="mv")
                            nc.vector.bn_aggr(out=mv, in_=st)
                            sc = stp.tile([128, 1], F32, tag="sc")
                            tt = stp.tile([128, 1], F32, tag="tt")
                            nc.scalar.activation(out=sc, in_=mv[:, 1:2],
                                                 func=AF.Sqrt, bias=eps_t)
                            nc.vector.reciprocal(out=sc, in_=sc)
                            nc.vector.tensor_scalar_mul(
                                out=sc, in0=sc,
                                scalar1=w32[:, NGB_OFF + 2 * mg:
                                            NGB_OFF + 2 * mg + 1])
                            nc.vector.tensor_mul(out=tt, in0=mv[:, 0:1],
                                                 in1=sc)
                            nc.vector.tensor_scalar(
                                out=tt, in0=tt, scalar1=-1.0,
                                scalar2=w32[:, NGB_OFF + 2 * mg + 1:
                                            NGB_OFF + 2 * mg + 2],
                                op0=mybir.AluOpType.mult,
                                op1=mybir.AluOpType.add)
                            if L < LAYERS:
                                xt = new_x_tile(f"X{L + 1}_{mg}_{s}")
                                Xnext[(mg, s)] = xt
                                xtr = xt.rearrange("p (y x) -> p y x",
                                                   y=34, x=34)
                                pmr2 = pm.rearrange("p (y x) -> p y x",
                                                    y=32, x=32)
                                nc.scalar.activation(
                                    out=xtr[:, 1:33, 1:33], in_=pmr2,
                                    func=AF.Relu, scale=sc, bias=tt)
                            else:
                                pmr = pm.rearrange("p (y x) -> p y x",
                                                   y=32, x=32)
                                nc.scalar.activation(
                                    out=pooled_in[:, mg - 2, s, :],
                                    in_=pmr[:, HALF - 1:HALF + 1,
                                            HALF - 1:HALF + 1],
                                    func=AF.Identity, scale=sc, bias=tt)
                    Xcur = Xnext

                # ---------------- readout ----------------
                load_weights('fc')
                tadd = small.tile([128, 2, 4], F32)
                tadd2 = small.tile([128, 2, 4], F32)
                pooled = small.tile([128, 2, 4], F32R)
                nc.vector.tensor_add(out=tadd, in0=pooled_in[:, :, :, 0],
                                     in1=pooled_in[:, :, :, 1])
                nc.vector.tensor_add(out=tadd2, in0=pooled_in[:, :, :, 2],
                                     in1=pooled_in[:, :, :, 3])
                nc.vector.tensor_add(out=pooled, in0=tadd, in1=tadd2)
                y_sb = small.tile([128, 4, 8], F32)
                for mo in range(8):
                    mlen = 128 if mo < 7 else OUT - 7 * 128
                    pf = psA.tile([128, 512], F32, tag="a", name=f"pf{mo}")
                    for kgi in range(2):
                        nc.tensor.matmul(
                            pf[0:mlen, 0:4],
                            w_sb[:, FCW_OFF + kgi * 1000 + mo * 128:
                                 FCW_OFF + kgi * 1000 + mo * 128 + mlen],
                            pooled[:, kgi, :],
                            start=(kgi == 0), stop=(kgi == 1))
                    nc.scalar.activation(
                        out=y_sb[0:mlen, :, mo], in_=pf[0:mlen, 0:4],
                        func=AF.Identity,
                        bias=w32[0:mlen, FCB_OFF + mo:FCB_OFF + mo + 1],
                        scale=1.0)
                for s in range(BPC):
                    dst1 = bass.AP(tensor=y4.tensor, offset=OUT * s,
                                   ap=[[1, 128], [128, 7]])
                    nc.sync.dma_start(out=dst1, in_=y_sb[:, s, 0:7])
                    dst2 = bass.AP(tensor=y4.tensor, offset=OUT * s + 896,
                                   ap=[[1, 104]])
                    nc.sync.dma_start(out=dst2, in_=y_sb[0:104, s, 7])

    nc.finalize()
    return nc


def di_col(dx):
    # column index of conv1 tap dx within w1x block (emission order 1,0,2)
    return {1: 0, 0: 1, 2: 2}[dx]


def _bn_scale_bias(nc, out_st, sums, w32, gb_off, n_tot, eps_t, pool, rows):
    """out_st[:rows, 0] = gamma*rsqrt(var+eps); out_st[:rows, 1] = beta - mu*scale."""
    r = slice(0, rows)
    mu = pool.tile([128, 1], F32, name=f"mu{gb_off}")
    ex2 = pool.tile([128, 1], F32, name=f"ex2{gb_off}")
    var = pool.tile([128, 1], F32, name=f"var{gb_off}")
    nc.vector.tensor_scalar_mul(out=mu[r], in0=sums[r, 0:1], scalar1=1.0 / n_tot)
    nc.vector.tensor_scalar_mul(out=ex2[r], in0=sums[r, 1:2], scalar1=1.0 / n_tot)
    nc.vector.tensor_mul(out=var[r], in0=mu[r], in1=mu[r])
    nc.vector.tensor_sub(out=var[r], in0=ex2[r], in1=var[r])
    nc.scalar.activation(out=var[r], in_=var[r], func=AF.Sqrt, bias=eps_t[r])
    nc.vector.reciprocal(out=var[r], in_=var[r])
    nc.vector.tensor_scalar_mul(out=out_st[r, 0:1], in0=var[r],
                                scalar1=w32[r, gb_off:gb_off + 1])
    nc.vector.tensor_mul(out=mu[r], in0=mu[r], in1=out_st[r, 0:1])
    nc.vector.tensor_scalar(out=out_st[r, 1:2], in0=mu[r], scalar1=-1.0,
                            scalar2=w32[r, gb_off + 1:gb_off + 2],
                            op0=mybir.AluOpType.mult,
                            op1=mybir.AluOpType.add)


def _pack_weights(ds_w1, ds_w2, ds_w3, conv_w, graph_w, fc_w, fc_b,
                  bn1_g, bn1_b, bn2_g, bn2_b, norm_g, norm_b):
    wts = np.zeros((128, WCOLS), np.float32)
    w16 = np.zeros((128, W16COLS), np.float16)
    # pruned graph weight
    k = int((1.0 - PRUNE) * DIM * DIM)
    a = np.abs(graph_w).ravel()
    thresh = np.partition(a, -k)[-k]
    w_eff = np.where(np.abs(graph_w) >= thresh, graph_w, 0.0).astype(np.float32)
    # conv1 taps, paired block-diag:
    # rows 64*q + 27*a + 3*dy + c, cols 64*a + o = w1[o, c, dy, dx]
    for dx in range(3):
        dc = di_col(dx)
        blk = np.zeros((128, 128), np.float32)
        for qq in range(2):
            for aa in range(2):
                for dy in range(3):
                    for c in range(3):
                        blk[64 * qq + 27 * aa + 3 * dy + c,
                            64 * aa:64 * aa + 64] = ds_w1[:, c, dy, dx]
        wts[:, W1X_OFF + dc * 128:W1X_OFF + (dc + 1) * 128] = blk
    # conv2 diag-dup taps
    for t, (dy, dx) in enumerate(TAPS):
        blk = np.zeros((128, 128), np.float32)
        d = ds_w2[:, 0, dy, dx]
        for aa in range(2):
            idx = np.arange(64)
            blk[64 * aa + idx, 64 * aa + idx] = d
        wts[:, W2D_OFF + t * 128:W2D_OFF + (t + 1) * 128] = blk
    # conv3: [64a + c, o] = w3[o, c]
    w3 = ds_w3[:, :, 0, 0]  # [128, 64]
    wts[0:64, W3_OFF:W3_OFF + 128] = w3.T
    wts[64:128, W3_OFF:W3_OFF + 128] = w3.T
    # main dw diag taps (fp16)
    for g in range(4):
        for t, (dy, dx) in enumerate(TAPS):
            blk = np.zeros((128, 128), np.float16)
            idx = np.arange(128)
            blk[idx, idx] = conv_w[g * 128:(g + 1) * 128, 0, dy, dx]
            off = DW16_OFF + (g * 9 + t) * 128
            w16[:, off:off + 128] = blk
    # dw k vectors for the DVE path (f32)
    for g in range(4):
        for t, (dy, dx) in enumerate(TAPS):
            wts[:, KV_OFF + g * 9 + t] = conv_w[g * 128:(g + 1) * 128, 0, dy, dx]
    # mix (fp16): [p, kg*512 + mg*128 + j] = w_eff[mg*128 + j, kg*128 + p]
    weT = w_eff.T  # [in, out]
    for kg in range(4):
        w16[:, MIX16_OFF + kg * 512:MIX16_OFF + (kg + 1) * 512] = \
            weT[kg * 128:(kg + 1) * 128, :].astype(np.float16)
    # fc: [p, kg*1000 + m] = 0.25 * fc_w[m, kg*128 + p]
    for kg in range(2):
        wts[:, FCW_OFF + kg * 1000:FCW_OFF + (kg + 1) * 1000] = \
            0.25 * fc_w[:, kg * 128:(kg + 1) * 128].T
    # fc bias [p, mo]
    fcb = np.zeros((128, 8), np.float32)
    fb = np.zeros(1024, np.float32)
    fb[:OUT] = fc_b
    fcb[:, :] = fb.reshape(8, 128).T
    wts[:, FCB_OFF:FCB_OFF + 8] = fcb
    # bn gammas/betas
    wts[0:64, BN1_OFF] = bn1_g
    wts[64:128, BN1_OFF] = bn1_g
    wts[0:64, BN1_OFF + 1] = bn1_b
    wts[64:128, BN1_OFF + 1] = bn1_b
    wts[:, BN2_OFF] = bn2_g
    wts[:, BN2_OFF + 1] = bn2_b
    for g in range(4):
        wts[:, NGB_OFF + 2 * g] = norm_g[g * 128:(g + 1) * 128]
        wts[:, NGB_OFF + 2 * g + 1] = norm_b[g * 128:(g + 1) * 128]
    return wts, w16


_nc_cache = None
last_results = None


def kernel(**inputs):
    global _nc_cache, last_results
    inputs = {k: np.asarray(v, np.float32) for k, v in inputs.items()}
    wts, w16 = _pack_weights(
        inputs["ds_w1"], inputs["ds_w2"], inputs["ds_w3"], inputs["conv_w"],
        inputs["graph_w"], inputs["fc_w"], inputs["fc_b"],
        inputs["bn1_g"], inputs["bn1_b"], inputs["bn2_g"], inputs["bn2_b"],
        inputs["norm_g"], inputs["norm_b"])
    x = inputs["x"]
    if _nc_cache is None:
        _nc_cache = build_nc()
    nc = _nc_cache
    in_maps = [{"x4": np.ascontiguousarray(x[c * BPC:(c + 1) * BPC]),
                "wts": wts, "wts16": w16} for c in range(N_CORES)]
    res = run_bass_kernel_spmd(nc, in_maps, core_ids=list(range(N_CORES)))
    last_results = res
    return np.concatenate([res.results[c]["y4"] for c in range(N_CORES)], axis=0)
